# revision 92
# baseline (speedup 1.0000x reference)
"""Trainium2 Bass kernel for the ViTDet-style transformer block
(windowed 14x14 attention w/ decomposed rel-pos, MLP, bottleneck convs).

Sharding: data-parallel over batch — 8 images, one per NeuronCore; no
collectives (windows/MLP/convs are intra-image).

On-chip layout is D-major (channels on partitions, tokens on free dim).
The decomposed rel-pos bias is folded into the score matmul by augmenting
the contraction dim: q rows carry [q (64) | q.Rh (14) | q.Rw (14)] and k
rows carry [k (64) | onehot(kh) | onehot(kw)], padded with zero rows to
K=128. Compute engines cannot cross partitions, so the odd head of each
pair lives at partitions 64..127 with its aug rows at 0..13 / 32..45,
letting every psum->sbuf copy stay partition-aligned. Softmax runs in the
S^T orientation (exp on ACT; key-sums via ones-matmul; 1/sum broadcast by
a K=1 matmul). 3x3 conv = 9 shifted accumulating matmuls over a
zero-padded 66x66 layout. PE transposes only at entry/exit.
"""

import sys

sys.path.insert(0, "/opt/trn_rl_repo")

import numpy as np
import ml_dtypes

import concourse.bass as bass
import concourse.mybir as mybir
from concourse.tile import TileContext
from concourse.bass_utils import run_bass_kernel_spmd

F32 = mybir.dt.float32
BF16 = mybir.dt.bfloat16
F8 = mybir.dt.float8e4
F8SC = 32.0  # fp8 weight pre-scale (undone at activation/copy time)
DR = mybir.MatmulPerfMode.DoubleRow
AF = mybir.ActivationFunctionType
ALU = mybir.AluOpType

DIM = 768
HEADS = 12
HDIM = 64
WS = 14
MLPD = 3072
BOT = 384
IMG = 64
GRID = 5
NW = GRID * GRID  # 25
NTOK = WS * WS  # 196
T = NW * NTOK  # 4900
C0, C1 = 126, 70  # window-token chunks (rows 0..8 / 9..13)
NK = DIM // 128  # 6
NT45 = 13  # token tiles of 392 (last one 196)
N_CORES = 8
PAD = IMG + 2  # 66

APPLY_AFFINE = True

# per-head-parity row layouts inside the [128, T] qa/ka tiles
QK_BASE = (0, 64)  # q/k content rows
RH_BASE = (64, 0)  # q.Rh / onehot(kh) rows (14)
RW_BASE = (96, 32)  # q.Rw / onehot(kw) rows (14)
Z_ROWS = (((78, 96), (110, 128)), ((14, 32), (46, 64)))  # zeroed rows


def wdims(w):
    wr, wc = divmod(w, GRID)
    ph = WS if wr < GRID - 1 else IMG - WS * (GRID - 1)
    pw = WS if wc < GRID - 1 else IMG - WS * (GRID - 1)
    return wr, wc, ph, pw


def split_multi_waits(nc):
    """walrus CoreV3 encodes at most one sync-wait per CTRL-class op; move
    extra waits onto preceding same-engine NoOps."""
    n = 0
    for f in nc.m.functions:
        for bb in f.blocks:
            out = []
            for inst in bb.instructions:
                si = getattr(inst, "sync_info", None)
                waits = list(si.on_wait) if si is not None else []
                if len(waits) > 1:
                    for i, wt in enumerate(waits[:-1]):
                        nop = mybir.InstNoOp(
                            name=f"{inst.name}-wsplit{i}", ins=[], outs=[]
                        )
                        nop.engine = inst.engine
                        nop.sync_info = mybir.SyncInfo(on_wait=[wt], on_update=[])
                        out.append(nop)
                        n += 1
                    inst.sync_info = mybir.SyncInfo(
                        on_wait=[waits[-1]], on_update=list(si.on_update)
                    )
                out.append(inst)
            bb.instructions = out
    return n


def _bcast_part(ap, n):
    return bass.AP(tensor=ap.tensor, offset=ap.offset, ap=[[0, n]] + list(ap.ap))


def _ln_rows(nc, ps, srow, ssq, csz, nch, eps_ap, tag):
    """psum row-sums -> bf16 rows (rstd, -mean*rstd), each [1, csz]."""
    mrow = ps.tile([1, 512], F32, bufs=1, tag=f"{tag}mr")
    nc.scalar.mul(out=mrow[:, :csz], in_=srow[:, :csz], mul=1.0 / nch)
    vrow = ps.tile([1, 512], F32, bufs=1, tag=f"{tag}vr")
    nc.scalar.mul(out=vrow[:, :csz], in_=ssq[:, :csz], mul=1.0 / nch)
    m2 = ps.tile([1, 512], F32, bufs=1, tag=f"{tag}m2")
    nc.vector.tensor_mul(out=m2[:, :csz], in0=mrow[:, :csz], in1=mrow[:, :csz])
    nc.vector.tensor_sub(out=vrow[:, :csz], in0=vrow[:, :csz], in1=m2[:, :csz])
    sdr = ps.tile([1, 512], F32, bufs=1, tag=f"{tag}sd")
    nc.scalar.activation(out=sdr[:, :csz], in_=vrow[:, :csz], func=AF.Sqrt,
                         bias=eps_ap)
    rs = ps.tile([1, 512], F32, bufs=1, tag=f"{tag}rs")
    nc.vector.reciprocal(out=rs[:, :csz], in_=sdr[:, :csz])
    nmr = ps.tile([1, 512], F32, bufs=1, tag=f"{tag}nm")
    nc.vector.tensor_mul(out=nmr[:, :csz], in0=mrow[:, :csz], in1=rs[:, :csz])
    nc.vector.tensor_scalar_mul(out=nmr[:, :csz], in0=nmr[:, :csz],
                                scalar1=-1.0)
    rs_b = ps.tile([1, 512], BF16, bufs=1, tag=f"{tag}rsb")
    nc.vector.tensor_copy(out=rs_b[:, :csz], in_=rs[:, :csz])
    nm_b = ps.tile([1, 512], BF16, bufs=1, tag=f"{tag}nmb")
    nc.vector.tensor_copy(out=nm_b[:, :csz], in_=nmr[:, :csz])
    return rs_b, nm_b


def build(stages=(1, 2, 3, 45, 6)):
    nc = bass.Bass("TRN2", target_bir_lowering=False)

    x = nc.dram_tensor("x", [IMG, IMG, DIM], F32, kind="ExternalInput")
    y = nc.dram_tensor("y", [IMG * IMG, DIM], BF16, kind="ExternalOutput")

    # weights pre-chunked on host to the SBUF layout [128, nchunk, cols]
    wqkv_d = nc.dram_tensor("wqkv", [128, 3 * NK, DIM], BF16,
                            kind="ExternalInput")
    wproj_d = nc.dram_tensor("wproj", [128, NK, DIM], BF16,
                             kind="ExternalInput")
    wfc1_d = nc.dram_tensor("wfc1", [128, NK, MLPD], BF16,
                            kind="ExternalInput")
    wfc2_d = nc.dram_tensor("wfc2", [128, 24, DIM], BF16,
                            kind="ExternalInput")
    w1c_d = nc.dram_tensor("w1c", [128, NK, BOT], BF16, kind="ExternalInput")
    w2t_d = nc.dram_tensor("w2t", [128, 3, 9, BOT], BF16,
                           kind="ExternalInput")
    w3c_d = nc.dram_tensor("w3c", [128, 3, DIM], BF16, kind="ExternalInput")
    rh_d = nc.dram_tensor("rh_all", [HDIM, WS * WS], BF16,
                          kind="ExternalInput")
    rw_d = nc.dram_tensor("rw_all", [HDIM, WS * WS], BF16,
                          kind="ExternalInput")
    kon_d = nc.dram_tensor("konst", [2 * WS, T], BF16, kind="ExternalInput")
    e2_d = nc.dram_tensor("e2", [HEADS, NK * 128], BF16, kind="ExternalInput")
    idf_d = nc.dram_tensor("idf", [128, 128], F32, kind="ExternalInput")
    idb_d = nc.dram_tensor("idb", [128, 128], BF16, kind="ExternalInput")
    n1wb_d = nc.dram_tensor("n1wb", [128, NK, 2], F32, kind="ExternalInput")
    n1wbi_d = nc.dram_tensor("n1wbi", [128, NK, 2], F32,
                             kind="ExternalInput")
    n2wb_d = nc.dram_tensor("n2wb", [128, NK, 2], F32, kind="ExternalInput")
    lnwb_d = nc.dram_tensor("lnwb", [128, 12, 2], F32, kind="ExternalInput")

    attn_d = nc.dram_tensor("attn_d", [DIM, T], BF16)

    with TileContext(nc) as tc:
        with tc.tile_pool(name="const", bufs=1) as constp:
            ident_f = constp.tile([128, 128], F32, tag="idf")
            nc.sync.dma_start(out=ident_f, in_=idf_d[:, :])
            ident_b = constp.tile([128, 128], BF16, tag="idb")
            nc.sync.dma_start(out=ident_b, in_=idb_d[:, :])
            onesb = constp.tile([128, 128], BF16, tag="onesb")
            nc.vector.memset(onesb, 1.0)
            eps5 = constp.tile([128, 1], F32, tag="eps5")
            nc.vector.memset(eps5, 1e-5)
            eps6 = constp.tile([128, 1], F32, tag="eps6")
            nc.vector.memset(eps6, 1e-6)

            with tc.tile_pool(name="srp", bufs=1) as srp:
                srows = srp.tile([HEADS, T], BF16, tag="srows")
                smrows = srp.tile([33, T], BF16, tag="smrows")
                x1_d = nc.dram_tensor("x1_d", [NK, 128, T], BF16)
                with tc.tile_pool(name="attw", bufs=1) as awp:
                    wqkv = awp.tile([128, 3 * NK, DIM], BF16, tag="wqkv")
                    nc.sync.dma_start(out=wqkv, in_=wqkv_d[:, :, :])
                    # rel-pos weights duplicated on partitions 0..63 and
                    # 64..127 so lhsT base matches the rhs base of either
                    # head parity
                    rh_s = awp.tile([128, WS * WS], BF16, tag="rh")
                    rw_s = awp.tile([128, WS * WS], BF16, tag="rw")
                    for b in (0, 64):
                        nc.sync.dma_start(out=rh_s[b : b + HDIM, :],
                                          in_=rh_d[:, :])
                        nc.sync.dma_start(out=rw_s[b : b + HDIM, :],
                                          in_=rw_d[:, :])
                    with tc.tile_pool(name="ln1t", bufs=1) as lp:
                        ln1t = [lp.tile([128, T], BF16, tag=f"ln1t{k}",
                                        name=f"ln1t{k}")
                                for k in range(NK)]
                        with tc.tile_pool(name="qa", bufs=1) as qap:
                            qa2 = [[qap.tile([128, T], BF16, tag=f"qa{h}{s}",
                                             name=f"qa{h}{s}")
                                    for h in range(2)] for s in range(2)]
                            ka2 = [[qap.tile([128, T], BF16, tag=f"ka{h}{s}",
                                             name=f"ka{h}{s}")
                                    for h in range(2)] for s in range(2)]
                            for s in range(2):
                                for h in range(2):
                                    zb = 64 - QK_BASE[h]
                                    nc.vector.memset(
                                        qa2[s][h][zb : zb + 64, :], 0.0)
                                    nc.vector.memset(
                                        ka2[s][h][zb : zb + 64, :], 0.0)
                                    nc.sync.dma_start(
                                        out=ka2[s][h][RH_BASE[h] :
                                                      RH_BASE[h] + WS, :],
                                        in_=kon_d[0:WS, :])
                                    nc.sync.dma_start(
                                        out=ka2[s][h][RW_BASE[h] :
                                                      RW_BASE[h] + WS, :],
                                        in_=kon_d[WS : 2 * WS, :])
                            if 1 in stages:
                                _s1(nc, tc, x, ident_b, ident_f, n1wb_d,
                                    ln1t, eps5, smrows, qa2, ka2, wqkv)
                            if 3 in stages:
                                _s3(nc, tc, ln1t, wqkv, rh_s, rw_s, qa2, ka2,
                                    onesb, attn_d, srows)
                        if 45 in stages:
                            _s4(nc, tc, wproj_d, n1wbi_d, onesb, ln1t,
                                attn_d, x1_d, srows, smrows, e2_d)
            with tc.tile_pool(name="xp", bufs=1) as xpp:
                x2p = [xpp.tile([128, PAD * PAD], BF16, tag=f"x2p{k}",
                                name=f"x2p{k}")
                       for k in range(NK)]
                for k in range(NK):
                    nc.gpsimd.memset(x2p[k], 0.0)
                if 45 in stages:
                    _s5(nc, tc, wfc1_d, wfc2_d, n2wb_d, onesb, x1_d, x2p,
                        eps5)
                if 6 in stages:
                    _s6(nc, tc, w1c_d, w2t_d, w3c_d, lnwb_d, onesb, ident_f,
                        ident_b, x2p, y, eps6)
    return nc


# ---------------------------------------------------------------- stage 1
def _s1(nc, tc, x, ident_b, ident_f, n1wb_d, ln1t, eps5, smrows,
        qa2, ka2, wqkv):
    with tc.tile_pool(name="s1c", bufs=1) as cp, \
         tc.tile_pool(name="s1", bufs=2) as p, \
         tc.tile_pool(name="s1s", bufs=4) as ps, \
         tc.tile_pool(name="s1ps", bufs=4, space="PSUM") as pp:
        n1wb = cp.tile([128, NK, 2], F32, tag="n1wb")
        nc.sync.dma_start(out=n1wb, in_=n1wb_d[:, :, :])
        for w in range(NW):
            wr, wc, ph, pw = wdims(w)
            xw0 = p.tile([C0, DIM], F32, tag="xw0", bufs=3)
            xw1 = p.tile([C1, DIM], F32, tag="xw1", bufs=3)
            if ph == WS and pw == WS:
                nc.gpsimd.dma_start(
                    out=xw0, in_=x[WS * wr : WS * wr + 9,
                                   WS * wc : WS * wc + WS, :])
                nc.gpsimd.dma_start(
                    out=xw1, in_=x[WS * wr + 9 : WS * wr + WS,
                                   WS * wc : WS * wc + WS, :])
            else:
                nc.vector.memset(xw0, 0.0)
                nc.vector.memset(xw1, 0.0)
                for i in range(ph):
                    src = x[WS * wr + i, WS * wc : WS * wc + pw, :]
                    t0 = i * WS
                    if t0 + pw <= C0:
                        nc.gpsimd.dma_start(out=xw0[t0 : t0 + pw, :], in_=src)
                    elif t0 >= C0:
                        nc.gpsimd.dma_start(
                            out=xw1[t0 - C0 : t0 - C0 + pw, :], in_=src)
                    else:
                        sp = C0 - t0
                        nc.gpsimd.dma_start(out=xw0[t0:C0, :],
                                            in_=src[:sp, :])
                        nc.gpsimd.dma_start(out=xw1[0 : pw - sp, :],
                                            in_=src[sp:pw, :])
            for ci, (xw, csz, coff) in enumerate(((xw0, C0, 0), (xw1, C1, C0))):
                stats = ps.tile([csz, 3, 6], F32, tag=f"st{ci}")
                xv = xw.rearrange("p (s d) -> p s d", s=3)
                for s in range(3):
                    nc.vector.bn_stats(out=stats[:, s, :], in_=xv[:, s, :])
                mv = ps.tile([csz, 2], F32, tag=f"mv{ci}")
                nc.vector.bn_aggr(out=mv, in_=stats)
                # per-token (std, mean) -> rows of smrows for the stage-4
                # x reconstruction
                sm = ps.tile([csz, 33], F32, tag=f"sm{ci}")
                nc.scalar.activation(out=sm[:, 0:1], in_=mv[:, 1:2],
                                     func=AF.Sqrt, bias=eps5[:csz, :])
                nc.vector.tensor_copy(out=sm[:, 32:33], in_=mv[:, 0:1])
                pst = pp.tile([33, C0], F32, tag="pst", bufs=2)
                nc.tensor.transpose(pst[:, :csz], sm, ident_f[:csz, :csz])
                nc.vector.tensor_copy(
                    out=smrows[0:1, w * NTOK + coff : w * NTOK + coff + csz],
                    in_=pst[0:1, :csz])
                nc.vector.tensor_copy(
                    out=smrows[32:33, w * NTOK + coff : w * NTOK + coff + csz],
                    in_=pst[32:33, :csz])
                rstd = ps.tile([csz, 1], F32, tag=f"rstd{ci}")
                nc.vector.reciprocal(out=rstd, in_=sm[:, 0:1])
                nmr = ps.tile([csz, 1], F32, tag=f"nmr{ci}")
                nc.vector.tensor_mul(out=nmr, in0=mv[:, 0:1], in1=rstd)
                nc.vector.tensor_scalar_mul(out=nmr, in0=nmr, scalar1=-1.0)
                lnw_t = p.tile([csz, DIM], BF16, tag=f"lnw{ci}",
                               bufs=2 if ci == 0 else 1)
                nc.scalar.activation(out=lnw_t, in_=xw, func=AF.Identity,
                                     scale=rstd, bias=nmr)
                for k in range(NK):
                    ptb = pp.tile([128, C0], BF16, tag="ptb")
                    nc.tensor.transpose(ptb[:, :csz],
                                        lnw_t[:, k * 128 : (k + 1) * 128],
                                        ident_b[:csz, :csz])
                    dst = ln1t[k][:, w * NTOK + coff :
                                  w * NTOK + coff + csz]
                    if k % 2:
                        nc.scalar.activation(
                            out=dst, in_=ptb[:, :csz], func=AF.Identity,
                            scale=n1wb[:, k, 0:1], bias=n1wb[:, k, 1:2])
                    else:
                        nc.vector.tensor_scalar(
                            out=dst, in0=ptb[:, :csz],
                            scalar1=n1wb[:, k, 0:1], scalar2=n1wb[:, k, 1:2],
                            op0=ALU.mult, op1=ALU.add)
            if w % 2 == 1 or w == NW - 1:
                nt = w // 2
                c0 = nt * 392
                csz = min(392, T - c0)
                for pair in range(2):
                    for qk, dst in ((0, qa2[pair]), (1, ka2[pair])):
                        pqk = pp.tile([128, 392], F32, tag="pqk", bufs=2)
                        for k in range(NK):
                            nc.tensor.matmul(
                                pqk[:, :csz],
                                lhsT=wqkv[:, qk * NK + k,
                                          pair * 128 : (pair + 1) * 128],
                                rhs=ln1t[k][:, c0 : c0 + csz],
                                start=(k == 0), stop=(k == NK - 1))
                        for h in range(2):
                            b = QK_BASE[h]
                            if h == 0:
                                nc.vector.tensor_copy(
                                    out=dst[h][b : b + HDIM, c0 : c0 + csz],
                                    in_=pqk[b : b + HDIM, :csz])
                            else:
                                nc.scalar.copy(
                                    out=dst[h][b : b + HDIM, c0 : c0 + csz],
                                    in_=pqk[b : b + HDIM, :csz])


# ---------------------------------------------------------------- stage 3
def _s3(nc, tc, ln1t, wqkv, rh_s, rw_s, qa2, ka2, onesb, attn_d, srows):
    with tc.tile_pool(name="s3", bufs=4) as p, \
         tc.tile_pool(name="s3o", bufs=2) as po_p, \
         tc.tile_pool(name="s3ps", bufs=2, space="PSUM") as pp, \
         tc.tile_pool(name="s3ps1", bufs=1, space="PSUM") as pp1:
        HB = HDIM + 1
        for pair in range(HEADS // 2):
            qa = qa2[pair % 2]
            ka = ka2[pair % 2]
            for nt in range(NT45 if pair >= 2 else 0):
                c0 = nt * 392
                csz = min(392, T - c0)
                for qk, dst in ((0, qa), (1, ka)):
                    pqk = pp.tile([128, 392], F32, tag="ps3a", bufs=3)
                    for k in range(NK):
                        nc.tensor.matmul(
                            pqk[:, :csz],
                            lhsT=wqkv[:, qk * NK + k,
                                      pair * 128 : (pair + 1) * 128],
                            rhs=ln1t[k][:, c0 : c0 + csz],
                            start=(k == 0), stop=(k == NK - 1))
                    for h in range(2):
                        b = QK_BASE[h]
                        nc.vector.tensor_copy(
                            out=dst[h][b : b + HDIM, c0 : c0 + csz],
                            in_=pqk[b : b + HDIM, :csz])
            for h in range(2):
                qb = QK_BASE[h]
                qa_v = qa[h].rearrange("p (w a b) -> p w a b", a=WS, b=WS)
                for qh in range(WS):
                    pr = pp.tile([WS, NW * WS], F32, tag="ps3a", bufs=3, name="pr")
                    nc.tensor.matmul(
                        pr.rearrange("p (w b) -> p w b", w=NW),
                        lhsT=rh_s[qb : qb + HDIM, qh * WS : (qh + 1) * WS],
                        rhs=qa_v[qb : qb + HDIM, :, qh, :],
                        start=True, stop=True)
                    if h == 0:
                        nc.vector.tensor_copy(
                            out=qa_v[RH_BASE[h] : RH_BASE[h] + WS, :, qh, :],
                            in_=pr.rearrange("p (w b) -> p w b", w=NW))
                    else:
                        nc.scalar.copy(
                            out=qa_v[RH_BASE[h] : RH_BASE[h] + WS, :, qh, :],
                            in_=pr.rearrange("p (w b) -> p w b", w=NW))
                for qw in range(WS):
                    pr2 = pp.tile([WS, NW * WS], F32, tag="ps3a", bufs=3, name="pr2")
                    nc.tensor.matmul(
                        pr2.rearrange("p (w a) -> p w a", w=NW),
                        lhsT=rw_s[qb : qb + HDIM, qw * WS : (qw + 1) * WS],
                        rhs=qa_v[qb : qb + HDIM, :, :, qw],
                        start=True, stop=True)
                    if h == 0:
                        nc.vector.tensor_copy(
                            out=qa_v[RW_BASE[h] : RW_BASE[h] + WS, :, :, qw],
                            in_=pr2.rearrange("p (w a) -> p w a", w=NW))
                    else:
                        nc.scalar.copy(
                            out=qa_v[RW_BASE[h] : RW_BASE[h] + WS, :, :, qw],
                            in_=pr2.rearrange("p (w a) -> p w a", w=NW))

            srh = po_p.tile([33, T], BF16, tag="srh", bufs=1)
            ao = None
            for w in range(NW):
                if w % 13 == 0:
                    ao = po_p.tile([128, 2548], BF16, tag="ao")
                    w0 = w
                vws = []
                for coff in (0, 98):
                    pv = pp.tile([98, 128], F32, tag="ps3a", bufs=3, name="pv")
                    for k in range(NK):
                        nc.tensor.matmul(
                            pv,
                            lhsT=ln1t[k][:, w * NTOK + coff :
                                         w * NTOK + coff + 98],
                            rhs=wqkv[:, 2 * NK + k,
                                     pair * 128 : (pair + 1) * 128],
                            start=(k == 0), stop=(k == NK - 1))
                    # [v_h0 (64) | 1 | v_h1 (64) | 1] so lhsT slices stay
                    # contiguous and carry the softmax-sums ones column
                    vw = p.tile([98, 2 * HB], BF16,
                                tag=f"vw{coff > 0}", name="vw")
                    vwv = vw.rearrange("p (h c) -> p h c", c=HB)
                    nc.vector.tensor_copy(
                        out=vwv[:, :, 0:HDIM],
                        in_=pv.rearrange("p (h c) -> p h c", c=HDIM))
                    nc.vector.memset(vwv[:, :, HDIM : HDIM + 1], 1.0)
                    vws.append(vw)
                vw0, vw1 = vws
                for h in range(2):
                    st = pp1.tile([98, 2, NTOK], F32, tag="st0", bufs=2)
                    nc.tensor.matmul(
                        st[:, 0, :], lhsT=ka[h][:, w * NTOK : w * NTOK + 98],
                        rhs=qa[h][:, w * NTOK : (w + 1) * NTOK],
                        start=True, stop=True)
                    nc.tensor.matmul(
                        st[:, 1, :],
                        lhsT=ka[h][:, w * NTOK + 98 : w * NTOK + 196],
                        rhs=qa[h][:, w * NTOK : (w + 1) * NTOK],
                        start=True, stop=True)
                    et = p.tile([98, 2, NTOK], BF16, tag="et0")
                    nc.scalar.activation(out=et, in_=st, func=AF.Exp)
                    # O plus a sums row via the ones column of v (row 64)
                    pO = pp1.tile([HB, NTOK], F32, tag="pO", bufs=3)
                    nc.tensor.matmul(pO,
                                     lhsT=vw0[:, h * HB : (h + 1) * HB],
                                     rhs=et[:, 0, :], start=True, stop=False)
                    nc.tensor.matmul(pO,
                                     lhsT=vw1[:, h * HB : (h + 1) * HB],
                                     rhs=et[:, 1, :], start=False, stop=True)
                    cof = (w - w0) * NTOK
                    if h == 0:
                        nc.vector.tensor_copy(
                            out=srh[0:1, w * NTOK : (w + 1) * NTOK],
                            in_=pO[HDIM : HDIM + 1, :])
                        nc.scalar.copy(
                            out=ao[0:HDIM, cof : cof + NTOK],
                            in_=pO[0:HDIM, :])
                    else:
                        nc.scalar.copy(
                            out=srh[32:33, w * NTOK : (w + 1) * NTOK],
                            in_=pO[HDIM : HDIM + 1, :])
                        nc.vector.tensor_copy(
                            out=ao[HDIM:128, cof : cof + NTOK],
                            in_=pO[0:HDIM, :])
                if w in (12, NW - 1):
                    nsz = (w - w0 + 1) * NTOK
                    nc.sync.dma_start(
                        out=attn_d[pair * 128 : (pair + 1) * 128,
                                   w0 * NTOK : w0 * NTOK + nsz],
                        in_=ao[:, :nsz])
            for h in range(2):
                nc.sync.dma_start(
                    out=srows[pair * 2 + h : pair * 2 + h + 1, :],
                    in_=srh[h * 32 : h * 32 + 1, :])


# ---------------------------------------------------------------- stage 4+5
def _s4(nc, tc, wproj_d, n1wbi_d, onesb, ln1t, attn_d, x1_d, srows, smrows,
        e2_d):
    # phase 1 (windowed tokens): proj + residual -> x1_d (window order).
    # x is reconstructed from ln1t via per-token (std, mean) rows.
    with tc.tile_pool(name="s4w", bufs=1) as wp, \
         tc.tile_pool(name="s4", bufs=2) as p, \
         tc.tile_pool(name="s4ps", bufs=2, space="PSUM") as pp, \
         tc.tile_pool(name="s4ps1", bufs=1, space="PSUM") as pp1:
        wproj = wp.tile([128, NK, DIM], BF16, tag="wproj")
        nc.sync.dma_start(out=wproj, in_=wproj_d[:, :, :])
        e2 = wp.tile([HEADS, NK * 128], BF16, tag="e2")
        nc.sync.dma_start(out=e2, in_=e2_d[:, :])
        n1wbi = wp.tile([128, NK, 2], F32, tag="n1wbi")
        nc.sync.dma_start(out=n1wbi, in_=n1wbi_d[:, :, :])
        for nt in range(NT45):
            c0 = nt * 392
            csz = min(392, T - c0)
            at = [p.tile([128, 392], BF16, tag=f"at{k}", name=f"at{k}")
                  for k in range(NK)]
            recb = p.tile([HEADS, 392], BF16, tag="recb")
            with nc.allow_low_precision(reason="softmax denominators"):
                nc.vector.reciprocal(out=recb[:, :csz],
                                     in_=srows[:, c0 : c0 + csz])
            pbs_p = pp1.tile([128, 392], F32, tag="pbs")
            nc.tensor.matmul(pbs_p[:, :csz], lhsT=onesb[0:1, :],
                             rhs=smrows[0:1, c0 : c0 + csz],
                             start=True, stop=True)


            for k in range(NK):
                nc.gpsimd.dma_start(
                    out=at[k][:, :csz],
                    in_=attn_d[k * 128 : (k + 1) * 128, c0 : c0 + csz])
                bck = pp.tile([128, 392], F32, tag="bck")
                nc.tensor.matmul(bck[:, :csz],
                                 lhsT=e2[:, k * 128 : (k + 1) * 128],
                                 rhs=recb[:, :csz], start=True, stop=True)
                nc.vector.tensor_mul(out=at[k][:, :csz], in0=at[k][:, :csz],
                                     in1=bck[:, :csz])
            x1 = p.tile([128, NK, 392], BF16, tag="x1")
            for m in range(NK):
                xs = p.tile([128, 392], F32, tag="xs")
                nc.scalar.activation(
                    out=xs[:, :csz], in_=ln1t[m][:, c0 : c0 + csz],
                    func=AF.Identity,
                    scale=n1wbi[:, m, 0:1], bias=n1wbi[:, m, 1:2])
                xsb = p.tile([128, 392], F32, tag="xsb")
                nc.vector.tensor_mul(out=xsb[:, :csz], in0=xs[:, :csz],
                                     in1=pbs_p[:, :csz])
                pj = pp.tile([128, 392], F32, tag="pj", bufs=3)
                nc.tensor.matmul(pj[:, :csz], lhsT=onesb[32:33, :],
                                 rhs=smrows[32:33, c0 : c0 + csz],
                                 start=True, stop=False)
                for k in range(NK):
                    nc.tensor.matmul(
                        pj[:, :csz],
                        lhsT=wproj[:, k, m * 128 : (m + 1) * 128],
                        rhs=at[k][:, :csz], start=False, stop=(k == NK - 1))
                nc.vector.tensor_add(out=x1[:, m, :csz], in0=pj[:, :csz],
                                     in1=xsb[:, :csz])
            nc.sync.dma_start(
                out=x1_d[:, :, c0 : c0 + csz].rearrange("k p c -> p k c"),
                in_=x1[:, :, :csz])


def _s5(nc, tc, wfc1_d, wfc2_d, n2wb_d, onesb, x1_d, x2p, eps5):
    # phase 2 (window order, 13 tiles of 392): LN2 + MLP + residual -> x2_d
    with tc.tile_pool(name="s5w", bufs=1) as wp, \
         tc.tile_pool(name="s5", bufs=2) as p, \
         tc.tile_pool(name="s5h", bufs=2) as hp, \
         tc.tile_pool(name="s5s", bufs=3) as ps, \
         tc.tile_pool(name="s5ps", bufs=2, space="PSUM") as pp, \
         tc.tile_pool(name="s5ps1", bufs=1, space="PSUM") as pp1:
        f1 = wp.tile([128, NK, MLPD], BF16, tag="f1")
        nc.sync.dma_start(out=f1, in_=wfc1_d[:, :, :])
        f2 = wp.tile([128, 24, DIM], BF16, tag="f2")
        nc.sync.dma_start(out=f2, in_=wfc2_d[:, :, :])
        n2wb = wp.tile([128, NK, 2], F32, tag="n2wb")
        nc.sync.dma_start(out=n2wb, in_=n2wb_d[:, :, :])
        for nt in range(NT45):
            c0 = nt * 392
            csz = min(392, T - c0)
            x1 = p.tile([128, NK, 392], BF16, tag="x1p2")
            nc.sync.dma_start(
                out=x1[:, :, :csz],
                in_=x1_d[:, :, c0 : c0 + csz].rearrange("k p c -> p k c"))
            l2 = p.tile([128, NK, 392], BF16, tag="l2")
            srow = pp1.tile([1, 512], F32, tag="srow")
            ssq = pp1.tile([1, 512], F32, tag="ssq")
            for m in range(NK):
                sqm = ps.tile([128, 392], BF16, bufs=2, tag="sqm")
                nc.scalar.activation(out=sqm, in_=x1[:, m, :], func=AF.Square)
                nc.tensor.matmul(srow[:, :csz], lhsT=onesb[:, 0:1],
                                 rhs=x1[:, m, :csz],
                                 start=(m == 0), stop=(m == NK - 1),
                                 skip_group_check=True)
                nc.tensor.matmul(ssq[:, :csz], lhsT=onesb[:, 0:1],
                                 rhs=sqm[:, :csz],
                                 start=(m == 0), stop=(m == NK - 1),
                                 skip_group_check=True)
            rs_b, nm_b = _ln_rows(nc, ps, srow, ssq, csz, DIM, eps5[0:1, :],
                                  "l2")
            pb1 = pp1.tile([128, 512], F32, tag="pb1")
            nc.tensor.matmul(pb1[:, :csz], lhsT=onesb[0:1, :],
                             rhs=rs_b[:, :csz], start=True, stop=True)
            pb2 = pp1.tile([128, 512], F32, tag="pb2")
            nc.tensor.matmul(pb2[:, :csz], lhsT=onesb[0:1, :],
                             rhs=nm_b[:, :csz], start=True, stop=True)
            for m in range(NK):
                t1 = ps.tile([128, 392], F32, tag="t1", bufs=2)
                nc.vector.tensor_mul(out=t1[:, :csz], in0=x1[:, m, :csz],
                                     in1=pb1[:, :csz])
                nc.vector.tensor_add(out=t1[:, :csz], in0=t1[:, :csz],
                                     in1=pb2[:, :csz])
                if APPLY_AFFINE:
                    nc.vector.tensor_scalar(
                        out=l2[:, m, :csz], in0=t1[:, :csz],
                        scalar1=n2wb[:, m, 0:1], scalar2=n2wb[:, m, 1:2],
                        op0=ALU.mult, op1=ALU.add)
                else:
                    nc.vector.tensor_copy(out=l2[:, m, :csz], in_=t1[:, :csz])
            hb = hp.tile([128, 24, 392], BF16, tag="hb")
            for m24 in range(24):
                pf = pp.tile([128, 392], F32, tag="mm5", bufs=3)
                for k in range(NK):
                    nc.tensor.matmul(
                        pf[:, :csz],
                        lhsT=f1[:, k, m24 * 128 : (m24 + 1) * 128],
                        rhs=l2[:, k, :csz], start=(k == 0), stop=(k == NK - 1))
                nc.scalar.activation(out=hb[:, m24, :csz], in_=pf[:, :csz],
                                     func=AF.Gelu)
            x2 = p.tile([128, NK, 392], BF16, tag="x2")
            for m in range(NK):
                pg = pp.tile([128, 392], F32, tag="mm5", bufs=3)
                for k24 in range(24):
                    nc.tensor.matmul(
                        pg[:, :csz],
                        lhsT=f2[:, k24, m * 128 : (m + 1) * 128],
                        rhs=hb[:, k24, :csz], start=(k24 == 0),
                        stop=(k24 == 23))
                nc.vector.tensor_add(out=x2[:, m, :csz], in0=pg[:, :csz],
                                     in1=x1[:, m, :csz])
            srcv = x2.rearrange("p k (a i j) -> p k a i j", a=2, i=WS)
            for wi in range(2):
                w = nt * 2 + wi
                if w >= NW or wi * NTOK >= csz:
                    continue
                wr, wc, ph, pw = wdims(w)
                for k in range(NK):
                    dst = x2p[k].rearrange("p (r c) -> p r c", r=PAD)[
                        :, 1 + WS * wr : 1 + WS * wr + ph,
                        1 + WS * wc : 1 + WS * wc + pw]
                    if k % 2:
                        nc.scalar.copy(out=dst, in_=srcv[:, k, wi, :ph, :pw])
                    else:
                        nc.vector.tensor_copy(out=dst,
                                              in_=srcv[:, k, wi, :ph, :pw])


# ---------------------------------------------------------------- stage 6
def _s6(nc, tc, w1c_d, w2t_d, w3c_d, lnwb_d, onesb, ident_f,
        ident_b, x2p, y, eps6):
    with tc.tile_pool(name="s6w", bufs=1) as wp, \
         tc.tile_pool(name="s6b", bufs=1) as bigp, \
         tc.tile_pool(name="s6", bufs=2) as p, \
         tc.tile_pool(name="s6s", bufs=3) as ps, \
         tc.tile_pool(name="s6ps", bufs=2, space="PSUM") as pp, \
         tc.tile_pool(name="s6ps1", bufs=1, space="PSUM") as pp1:
        w1s = wp.tile([128, NK, BOT], BF16, tag="w1s")
        nc.sync.dma_start(out=w1s, in_=w1c_d[:, :, :])
        w2s = wp.tile([128, 3, 9, BOT], BF16, tag="w2s")
        nc.sync.dma_start(out=w2s, in_=w2t_d[:, :, :, :])
        w3s = wp.tile([128, 3, DIM], BF16, tag="w3s")
        nc.sync.dma_start(out=w3s, in_=w3c_d[:, :, :])
        lnwb = wp.tile([128, 12, 2], F32, tag="lnwb")
        nc.sync.dma_start(out=lnwb, in_=lnwb_d[:, :, :])

        b1p = [bigp.tile([128, PAD * PAD], BF16, tag=f"b1p{k}", name=f"b1p{k}")
               for k in range(3)]
        b2 = [bigp.tile([128, IMG * IMG], BF16, tag=f"b2_{k}", name=f"b2_{k}")
              for k in range(3)]
        for k in range(3):
            nc.gpsimd.memset(b1p[k], 0.0)
        def convln(rhs_fn, wsel, nk_in, nm_out, taps, eps_ap, nt8):
            seq = [(tap, k) for tap in taps for k in range(nk_in)]
            cms = []
            srow = pp1.tile([1, 512], F32, tag="c_srow")
            ssq = pp1.tile([1, 512], F32, tag="c_ssq")
            for m in range(nm_out):
                pc = pp.tile([128, 512], F32, tag="pc", bufs=3)
                for idx, (tap, k) in enumerate(seq):
                    nc.tensor.matmul(
                        pc.rearrange("p (r c) -> p r c", r=8),
                        lhsT=wsel(k, tap, m),
                        rhs=rhs_fn(k, tap, nt8),
                        start=(idx == 0), stop=(idx == len(seq) - 1))
                cm = ps.tile([128, 512], BF16, bufs=7, tag="cm")
                nc.scalar.copy(out=cm, in_=pc)
                sq = ps.tile([128, 512], BF16, bufs=2, tag="csq")
                nc.scalar.activation(out=sq, in_=pc, func=AF.Square)
                cms.append((cm, sq))
                nc.tensor.matmul(srow, lhsT=onesb[:, 0:1], rhs=cm,
                                 start=(m == 0), stop=(m == nm_out - 1),
                                 skip_group_check=True)
                nc.tensor.matmul(ssq, lhsT=onesb[:, 0:1], rhs=sq,
                                 start=(m == 0), stop=(m == nm_out - 1),
                                 skip_group_check=True)
            rs_b, nm_b = _ln_rows(nc, ps, srow, ssq, 512, nm_out * 128,
                                  eps_ap, "c")
            qb1 = pp1.tile([128, 512], F32, tag="qb1")
            nc.tensor.matmul(qb1, lhsT=onesb[0:1, :], rhs=rs_b, start=True,
                             stop=True)
            qb2 = pp1.tile([128, 512], F32, tag="qb2")
            nc.tensor.matmul(qb2, lhsT=onesb[0:1, :], rhs=nm_b, start=True,
                             stop=True)
            outs = []
            for m in range(nm_out):
                t1 = ps.tile([128, 512], F32, bufs=7, tag="c_t1")
                nc.vector.tensor_mul(out=t1, in0=cms[m][0], in1=qb1)
                nc.vector.tensor_add(out=t1, in0=t1, in1=qb2)
                outs.append(t1)
            return outs

        def x2p_rhs(k, tap, nt8):
            return x2p[k].rearrange("p (r c) -> p r c", r=PAD)[
                :, 1 + nt8 * 8 : 1 + nt8 * 8 + 8, 1 : 1 + IMG]

        def b1p_rhs(k, tap, nt8):
            dy, dx = tap
            return b1p[k].rearrange("p (r c) -> p r c", r=PAD)[
                :, nt8 * 8 + dy : nt8 * 8 + dy + 8, dx : dx + IMG]

        def b2_rhs(k, tap, nt8):
            return b2[k].rearrange("p (r c) -> p r c", r=IMG)[
                :, nt8 * 8 : nt8 * 8 + 8, :]

        for nt8 in range(8):
            outs = convln(
                x2p_rhs,
                lambda k, tap, m: w1s[:, k, m * 128 : (m + 1) * 128],
                NK, 3, [None], eps6[0:1, :], nt8)
            for m in range(3):
                nc.scalar.activation(
                    out=b1p[m].rearrange("p (r c) -> p r c", r=PAD)[
                        :, 1 + nt8 * 8 : 1 + nt8 * 8 + 8, 1 : 1 + IMG],
                    in_=outs[m].rearrange("p (r c) -> p r c", r=8),
                    func=AF.Gelu, scale=lnwb[:, m, 0:1], bias=lnwb[:, m, 1:2])
        taps2 = [(dy, dx) for dy in range(3) for dx in range(3)]
        for nt8 in range(8):
            outs = convln(
                b1p_rhs,
                lambda k, tap, m: w2s[:, k, tap[0] * 3 + tap[1],
                                      m * 128 : (m + 1) * 128],
                3, 3, taps2, eps6[0:1, :], nt8)
            for m in range(3):
                nc.scalar.activation(
                    out=b2[m].rearrange("p (r c) -> p r c", r=IMG)[
                        :, nt8 * 8 : nt8 * 8 + 8, :],
                    in_=outs[m].rearrange("p (r c) -> p r c", r=8),
                    func=AF.Gelu, scale=lnwb[:, 3 + m, 0:1],
                    bias=lnwb[:, 3 + m, 1:2])
        for nt8 in range(8):
            outs = convln(
                b2_rhs,
                lambda k, tap, m: w3s[:, k, m * 128 : (m + 1) * 128],
                3, NK, [None], eps6[0:1, :], nt8)
            om = []
            for m in range(NK):
                fin = ps.tile([128, 512], BF16, bufs=7, tag="c_fin")
                nc.scalar.activation(
                    out=fin, in_=outs[m], func=AF.Identity,
                    scale=lnwb[:, 6 + m, 0:1], bias=lnwb[:, 6 + m, 1:2])
                nc.gpsimd.tensor_add(
                    out=fin.rearrange("p (r c) -> p r c", r=8),
                    in0=fin.rearrange("p (r c) -> p r c", r=8),
                    in1=x2p[m].rearrange("p (r c) -> p r c", r=PAD)[
                        :, 1 + nt8 * 8 : 1 + nt8 * 8 + 8, 1 : 1 + IMG])
                om.append(fin)
            for s in range(4):
                outt = p.tile([128, DIM], BF16, tag="outt")
                for m in range(NK):
                    ptp = pp.tile([128, 128], BF16, tag="ptp", bufs=1)
                    nc.tensor.transpose(ptp, om[m][:, s * 128 : (s + 1) * 128],
                                        ident_b)
                    if m % 2:
                        nc.scalar.copy(
                            out=outt[:, m * 128 : (m + 1) * 128], in_=ptp)
                    else:
                        nc.vector.tensor_copy(
                            out=outt[:, m * 128 : (m + 1) * 128], in_=ptp)
                nc.sync.dma_start(
                    out=y[nt8 * 512 + s * 128 : nt8 * 512 + (s + 1) * 128, :],
                    in_=outt)


# ---------------------------------------------------------------- host side
def _chunk(w, nchunk):
    """[nchunk*128, N] -> [128, nchunk, N] (row r = chunk*128 + p)."""
    n = w.shape[1]
    return np.ascontiguousarray(w.reshape(nchunk, 128, n).transpose(1, 0, 2))


def _prep_weights(inputs):
    bf = ml_dtypes.bfloat16
    f8 = ml_dtypes.float8_e4m3
    qkv_w = np.array(inputs["qkv_w"], np.float32)
    qkv_w[:, :DIM] *= HDIM ** -0.5  # fold score scale into Wq
    # [128, (qk, k), 768]: chunk (qk*NK + k) holds input-rows k*128..+128 of
    # the q/k/v block
    wq = np.stack([qkv_w[k * 128 : (k + 1) * 128, qk * DIM : (qk + 1) * DIM]
                   for qk in range(3) for k in range(NK)], axis=1)
    rel_h = np.array(inputs["rel_pos_h"], np.float32)
    rel_w = np.array(inputs["rel_pos_w"], np.float32)
    coords = np.arange(WS)[:, None] - np.arange(WS)[None, :] + WS - 1
    Rh = rel_h[coords]  # [q, k, c]
    Rw = rel_w[coords]
    # lhsT[c, kh] per qh; x sqrt(HDIM) undoes the q-scale fold
    rh_all = np.concatenate([Rh[i].T * (HDIM ** 0.5) for i in range(WS)],
                            axis=1)
    rw_all = np.concatenate([Rw[i].T * (HDIM ** 0.5) for i in range(WS)],
                            axis=1)
    kh = np.arange(NTOK) // WS
    kw = np.arange(NTOK) % WS
    kon = np.zeros((2 * WS, NTOK), np.float32)
    kon[kh, np.arange(NTOK)] = 1.0
    kon[WS + kw, np.arange(NTOK)] = 1.0
    konst = np.tile(kon, (1, NW))
    e2 = np.zeros((HEADS, NK * 128), np.float32)
    for g in range(HEADS):
        k, half = divmod(g, 2)
        e2[g, k * 128 + half * 64 : k * 128 + half * 64 + 64] = 1.0
    conv2_w = np.array(inputs["conv2_w"], np.float32)
    # [128, (kchunk), (tap), 384]
    w2t = np.stack([conv2_w[:, :, dy, dx].T for dy in range(3)
                    for dx in range(3)])  # [9, 384in, 384out]
    w2t = np.ascontiguousarray(
        w2t.reshape(9, 3, 128, BOT).transpose(2, 1, 0, 3))
    lnw = np.concatenate([np.array(inputs[k], np.float32)
                          for k in ("ln1_w", "ln2_w", "ln3_w")])
    lnb = np.concatenate([np.array(inputs[k], np.float32)
                          for k in ("ln1_b", "ln2_b", "ln3_b")])
    lnwb = np.stack([lnw, lnb], axis=1)  # [1536, 2]
    n2wb = np.stack([np.array(inputs["norm2_w"], np.float32),
                     np.array(inputs["norm2_b"], np.float32)], axis=1)
    d = dict(
        wqkv=np.ascontiguousarray(wq).astype(bf),
        wproj=_chunk(np.array(inputs["proj_w"], np.float32), NK).astype(bf),
        wfc1=_chunk(np.array(inputs["fc1_w"], np.float32), NK).astype(bf),
        wfc2=_chunk(np.array(inputs["fc2_w"], np.float32), 24).astype(bf),
        w1c=_chunk(np.array(inputs["conv1_w"], np.float32)[:, :, 0, 0].T,
                   NK).astype(bf),
        w2t=w2t.astype(bf),
        w3c=_chunk(np.array(inputs["conv3_w"], np.float32)[:, :, 0, 0].T,
                   3).astype(bf),
        rh_all=np.ascontiguousarray(rh_all).astype(bf),
        rw_all=np.ascontiguousarray(rw_all).astype(bf),
        konst=np.ascontiguousarray(konst).astype(bf),
        e2=e2.astype(bf),
        idf=np.eye(128, dtype=np.float32),
        idb=np.eye(128).astype(bf),
        n1wb=_chunk(np.stack([np.array(inputs["norm1_w"], np.float32),
                              np.array(inputs["norm1_b"], np.float32)],
                             axis=1), NK),
        n1wbi=_chunk(np.stack(
            [1.0 / np.array(inputs["norm1_w"], np.float32),
             -np.array(inputs["norm1_b"], np.float32)
             / np.array(inputs["norm1_w"], np.float32)], axis=1), NK),
        n2wb=_chunk(n2wb, NK),
        lnwb=_chunk(lnwb, 12),
    )
    return d


def kernel(**inputs):
    nc = build()
    split_multi_waits(nc)
    wts = _prep_weights(inputs)
    x = np.asarray(inputs["x"], np.float32)
    in_maps = [dict(wts, x=np.ascontiguousarray(x[c])) for c in range(N_CORES)]
    res = run_bass_kernel_spmd(nc, in_maps, core_ids=list(range(N_CORES)))
    return np.stack([np.asarray(res.results[c]["y"], np.float32)
                       .reshape(IMG, IMG, DIM)
                     for c in range(N_CORES)])



# revision 93
# speedup vs baseline: 1.0142x; 1.0142x over previous
"""Trainium2 Bass kernel for the ViTDet-style transformer block
(windowed 14x14 attention w/ decomposed rel-pos, MLP, bottleneck convs).

Sharding: data-parallel over batch — 8 images, one per NeuronCore; no
collectives (windows/MLP/convs are intra-image).

On-chip layout is D-major (channels on partitions, tokens on free dim).
The decomposed rel-pos bias is folded into the score matmul by augmenting
the contraction dim: q rows carry [q (64) | q.Rh (14) | q.Rw (14)] and k
rows carry [k (64) | onehot(kh) | onehot(kw)], padded with zero rows to
K=128. Compute engines cannot cross partitions, so the odd head of each
pair lives at partitions 64..127 with its aug rows at 0..13 / 32..45,
letting every psum->sbuf copy stay partition-aligned. Softmax runs in the
S^T orientation (exp on ACT; key-sums via ones-matmul; 1/sum broadcast by
a K=1 matmul). 3x3 conv = 9 shifted accumulating matmuls over a
zero-padded 66x66 layout. PE transposes only at entry/exit.
"""

import sys

sys.path.insert(0, "/opt/trn_rl_repo")

import numpy as np
import ml_dtypes

import concourse.bass as bass
import concourse.mybir as mybir
from concourse.tile import TileContext
from concourse.bass_utils import run_bass_kernel_spmd

F32 = mybir.dt.float32
BF16 = mybir.dt.bfloat16
F8 = mybir.dt.float8e4
F8SC = 32.0  # fp8 weight pre-scale (undone at activation/copy time)
DR = mybir.MatmulPerfMode.DoubleRow
AF = mybir.ActivationFunctionType
ALU = mybir.AluOpType

DIM = 768
HEADS = 12
HDIM = 64
WS = 14
MLPD = 3072
BOT = 384
IMG = 64
GRID = 5
NW = GRID * GRID  # 25
NTOK = WS * WS  # 196
T = NW * NTOK  # 4900
C0, C1 = 126, 70  # window-token chunks (rows 0..8 / 9..13)
NK = DIM // 128  # 6
NT45 = 13  # token tiles of 392 (last one 196)
N_CORES = 8
PAD = IMG + 2  # 66

APPLY_AFFINE = True

# per-head-parity row layouts inside the [128, T] qa/ka tiles
QK_BASE = (0, 64)  # q/k content rows
RH_BASE = (64, 0)  # q.Rh / onehot(kh) rows (14)
RW_BASE = (96, 32)  # q.Rw / onehot(kw) rows (14)
Z_ROWS = (((78, 96), (110, 128)), ((14, 32), (46, 64)))  # zeroed rows


def wdims(w):
    wr, wc = divmod(w, GRID)
    ph = WS if wr < GRID - 1 else IMG - WS * (GRID - 1)
    pw = WS if wc < GRID - 1 else IMG - WS * (GRID - 1)
    return wr, wc, ph, pw


def split_multi_waits(nc):
    """walrus CoreV3 encodes at most one sync-wait per CTRL-class op; move
    extra waits onto preceding same-engine NoOps."""
    n = 0
    for f in nc.m.functions:
        for bb in f.blocks:
            out = []
            for inst in bb.instructions:
                si = getattr(inst, "sync_info", None)
                waits = list(si.on_wait) if si is not None else []
                if len(waits) > 1:
                    for i, wt in enumerate(waits[:-1]):
                        nop = mybir.InstNoOp(
                            name=f"{inst.name}-wsplit{i}", ins=[], outs=[]
                        )
                        nop.engine = inst.engine
                        nop.sync_info = mybir.SyncInfo(on_wait=[wt], on_update=[])
                        out.append(nop)
                        n += 1
                    inst.sync_info = mybir.SyncInfo(
                        on_wait=[waits[-1]], on_update=list(si.on_update)
                    )
                out.append(inst)
            bb.instructions = out
    return n


def _bcast_part(ap, n):
    return bass.AP(tensor=ap.tensor, offset=ap.offset, ap=[[0, n]] + list(ap.ap))


def _ln_rows(nc, ps, srow, ssq, csz, nch, eps_ap, tag):
    """psum row-sums -> bf16 rows (rstd, -mean*rstd), each [1, csz]."""
    mrow = ps.tile([1, 512], F32, bufs=1, tag=f"{tag}mr")
    nc.scalar.mul(out=mrow[:, :csz], in_=srow[:, :csz], mul=1.0 / nch)
    vrow = ps.tile([1, 512], F32, bufs=1, tag=f"{tag}vr")
    nc.scalar.mul(out=vrow[:, :csz], in_=ssq[:, :csz], mul=1.0 / nch)
    m2 = ps.tile([1, 512], F32, bufs=1, tag=f"{tag}m2")
    nc.vector.tensor_mul(out=m2[:, :csz], in0=mrow[:, :csz], in1=mrow[:, :csz])
    nc.vector.tensor_sub(out=vrow[:, :csz], in0=vrow[:, :csz], in1=m2[:, :csz])
    sdr = ps.tile([1, 512], F32, bufs=1, tag=f"{tag}sd")
    nc.scalar.activation(out=sdr[:, :csz], in_=vrow[:, :csz], func=AF.Sqrt,
                         bias=eps_ap)
    rs = ps.tile([1, 512], F32, bufs=1, tag=f"{tag}rs")
    nc.vector.reciprocal(out=rs[:, :csz], in_=sdr[:, :csz])
    nmr = ps.tile([1, 512], F32, bufs=1, tag=f"{tag}nm")
    nc.vector.tensor_mul(out=nmr[:, :csz], in0=mrow[:, :csz], in1=rs[:, :csz])
    nc.vector.tensor_scalar_mul(out=nmr[:, :csz], in0=nmr[:, :csz],
                                scalar1=-1.0)
    rs_b = ps.tile([1, 512], BF16, bufs=1, tag=f"{tag}rsb")
    nc.vector.tensor_copy(out=rs_b[:, :csz], in_=rs[:, :csz])
    nm_b = ps.tile([1, 512], BF16, bufs=1, tag=f"{tag}nmb")
    nc.vector.tensor_copy(out=nm_b[:, :csz], in_=nmr[:, :csz])
    return rs_b, nm_b


def build(stages=(1, 2, 3, 45, 6)):
    nc = bass.Bass("TRN2", target_bir_lowering=False)

    x = nc.dram_tensor("x", [IMG, IMG, DIM], F32, kind="ExternalInput")
    y = nc.dram_tensor("y", [IMG * IMG, DIM], BF16, kind="ExternalOutput")

    # weights pre-chunked on host to the SBUF layout [128, nchunk, cols]
    wqkv_d = nc.dram_tensor("wqkv", [128, 3 * NK, DIM], BF16,
                            kind="ExternalInput")
    wproj_d = nc.dram_tensor("wproj", [128, NK, DIM], BF16,
                             kind="ExternalInput")
    wfc1_d = nc.dram_tensor("wfc1", [128, NK, MLPD], BF16,
                            kind="ExternalInput")
    wfc2_d = nc.dram_tensor("wfc2", [128, 24, DIM], BF16,
                            kind="ExternalInput")
    w1c_d = nc.dram_tensor("w1c", [128, NK, BOT], BF16, kind="ExternalInput")
    w2t_d = nc.dram_tensor("w2t", [128, 3, 9, BOT], BF16,
                           kind="ExternalInput")
    w3c_d = nc.dram_tensor("w3c", [128, 3, DIM], BF16, kind="ExternalInput")
    rh_d = nc.dram_tensor("rh_all", [HDIM, WS * WS], BF16,
                          kind="ExternalInput")
    rw_d = nc.dram_tensor("rw_all", [HDIM, WS * WS], BF16,
                          kind="ExternalInput")
    kon_d = nc.dram_tensor("konst", [2 * WS, T], BF16, kind="ExternalInput")
    e2_d = nc.dram_tensor("e2", [HEADS, NK * 128], BF16, kind="ExternalInput")
    idf_d = nc.dram_tensor("idf", [128, 128], F32, kind="ExternalInput")
    idb_d = nc.dram_tensor("idb", [128, 128], BF16, kind="ExternalInput")
    n1wb_d = nc.dram_tensor("n1wb", [128, NK, 2], F32, kind="ExternalInput")
    n1wbi_d = nc.dram_tensor("n1wbi", [128, NK, 2], F32,
                             kind="ExternalInput")
    n2wb_d = nc.dram_tensor("n2wb", [128, NK, 2], F32, kind="ExternalInput")
    lnwb_d = nc.dram_tensor("lnwb", [128, 12, 2], F32, kind="ExternalInput")

    attn_d = nc.dram_tensor("attn_d", [DIM, T], BF16)

    with TileContext(nc) as tc:
        with tc.tile_pool(name="const", bufs=1) as constp:
            ident_f = constp.tile([128, 128], F32, tag="idf")
            nc.sync.dma_start(out=ident_f, in_=idf_d[:, :])
            ident_b = constp.tile([128, 128], BF16, tag="idb")
            nc.sync.dma_start(out=ident_b, in_=idb_d[:, :])
            onesb = constp.tile([128, 128], BF16, tag="onesb")
            nc.vector.memset(onesb, 1.0)
            eps5 = constp.tile([128, 1], F32, tag="eps5")
            nc.vector.memset(eps5, 1e-5)
            eps6 = constp.tile([128, 1], F32, tag="eps6")
            nc.vector.memset(eps6, 1e-6)

            with tc.tile_pool(name="srp", bufs=1) as srp:
                srows = srp.tile([HEADS, T], BF16, tag="srows")
                smrows = srp.tile([33, T], BF16, tag="smrows")
                x1_d = nc.dram_tensor("x1_d", [NK, 128, T], BF16)
                with tc.tile_pool(name="attw", bufs=1) as awp:
                    wqkv = awp.tile([128, 3 * NK, DIM], BF16, tag="wqkv")
                    nc.sync.dma_start(out=wqkv, in_=wqkv_d[:, :, :])
                    # rel-pos weights duplicated on partitions 0..63 and
                    # 64..127 so lhsT base matches the rhs base of either
                    # head parity
                    rh_s = awp.tile([128, WS * WS], BF16, tag="rh")
                    rw_s = awp.tile([128, WS * WS], BF16, tag="rw")
                    for b in (0, 64):
                        nc.sync.dma_start(out=rh_s[b : b + HDIM, :],
                                          in_=rh_d[:, :])
                        nc.sync.dma_start(out=rw_s[b : b + HDIM, :],
                                          in_=rw_d[:, :])
                    with tc.tile_pool(name="ln1t", bufs=1) as lp:
                        ln1t = [lp.tile([128, T], BF16, tag=f"ln1t{k}",
                                        name=f"ln1t{k}")
                                for k in range(NK)]
                        with tc.tile_pool(name="qa", bufs=1) as qap:
                            qa2 = [[qap.tile([128, T], BF16, tag=f"qa{h}{s}",
                                             name=f"qa{h}{s}")
                                    for h in range(2)] for s in range(2)]
                            ka2 = [[qap.tile([128, T], BF16, tag=f"ka{h}{s}",
                                             name=f"ka{h}{s}")
                                    for h in range(2)] for s in range(2)]
                            for s in range(2):
                                for h in range(2):
                                    zb = 64 - QK_BASE[h]
                                    nc.vector.memset(
                                        qa2[s][h][zb : zb + 64, :], 0.0)
                                    nc.vector.memset(
                                        ka2[s][h][zb : zb + 64, :], 0.0)
                                    nc.sync.dma_start(
                                        out=ka2[s][h][RH_BASE[h] :
                                                      RH_BASE[h] + WS, :],
                                        in_=kon_d[0:WS, :])
                                    nc.sync.dma_start(
                                        out=ka2[s][h][RW_BASE[h] :
                                                      RW_BASE[h] + WS, :],
                                        in_=kon_d[WS : 2 * WS, :])
                            if 1 in stages:
                                _s1(nc, tc, x, ident_b, ident_f, n1wb_d,
                                    ln1t, eps5, smrows, qa2, ka2, wqkv)
                            if 3 in stages:
                                _s3(nc, tc, ln1t, wqkv, rh_s, rw_s, qa2, ka2,
                                    onesb, attn_d, srows)
                        if 45 in stages:
                            _s4(nc, tc, wproj_d, n1wbi_d, onesb, ln1t,
                                attn_d, x1_d, srows, smrows, e2_d)
            with tc.tile_pool(name="xp", bufs=1) as xpp:
                x2p = [xpp.tile([128, PAD * PAD], BF16, tag=f"x2p{k}",
                                name=f"x2p{k}")
                       for k in range(NK)]
                for k in range(NK):
                    nc.gpsimd.memset(x2p[k], 0.0)
                if 45 in stages:
                    _s5(nc, tc, wfc1_d, wfc2_d, n2wb_d, onesb, x1_d, x2p,
                        eps5)
                if 6 in stages:
                    _s6(nc, tc, w1c_d, w2t_d, w3c_d, lnwb_d, onesb, ident_f,
                        ident_b, x2p, y, eps6)
    return nc


# ---------------------------------------------------------------- stage 1
def _s1(nc, tc, x, ident_b, ident_f, n1wb_d, ln1t, eps5, smrows,
        qa2, ka2, wqkv):
    with tc.tile_pool(name="s1c", bufs=1) as cp, \
         tc.tile_pool(name="s1", bufs=2) as p, \
         tc.tile_pool(name="s1s", bufs=4) as ps, \
         tc.tile_pool(name="s1ps", bufs=4, space="PSUM") as pp:
        n1wb = cp.tile([128, NK, 2], F32, tag="n1wb")
        nc.sync.dma_start(out=n1wb, in_=n1wb_d[:, :, :])
        for w in range(NW):
            wr, wc, ph, pw = wdims(w)
            xw0 = p.tile([C0, DIM], F32, tag="xw0", bufs=3)
            xw1 = p.tile([C1, DIM], F32, tag="xw1", bufs=3)
            if ph == WS and pw == WS:
                nc.gpsimd.dma_start(
                    out=xw0, in_=x[WS * wr : WS * wr + 9,
                                   WS * wc : WS * wc + WS, :])
                nc.gpsimd.dma_start(
                    out=xw1, in_=x[WS * wr + 9 : WS * wr + WS,
                                   WS * wc : WS * wc + WS, :])
            else:
                nc.vector.memset(xw0, 0.0)
                nc.vector.memset(xw1, 0.0)
                for i in range(ph):
                    src = x[WS * wr + i, WS * wc : WS * wc + pw, :]
                    t0 = i * WS
                    if t0 + pw <= C0:
                        nc.gpsimd.dma_start(out=xw0[t0 : t0 + pw, :], in_=src)
                    elif t0 >= C0:
                        nc.gpsimd.dma_start(
                            out=xw1[t0 - C0 : t0 - C0 + pw, :], in_=src)
                    else:
                        sp = C0 - t0
                        nc.gpsimd.dma_start(out=xw0[t0:C0, :],
                                            in_=src[:sp, :])
                        nc.gpsimd.dma_start(out=xw1[0 : pw - sp, :],
                                            in_=src[sp:pw, :])
            for ci, (xw, csz, coff) in enumerate(((xw0, C0, 0), (xw1, C1, C0))):
                stats = ps.tile([csz, 3, 6], F32, tag=f"st{ci}")
                xv = xw.rearrange("p (s d) -> p s d", s=3)
                for s in range(3):
                    nc.vector.bn_stats(out=stats[:, s, :], in_=xv[:, s, :])
                mv = ps.tile([csz, 2], F32, tag=f"mv{ci}")
                nc.vector.bn_aggr(out=mv, in_=stats)
                # per-token (std, mean) -> rows of smrows for the stage-4
                # x reconstruction
                sm = ps.tile([csz, 33], F32, tag=f"sm{ci}")
                nc.scalar.activation(out=sm[:, 0:1], in_=mv[:, 1:2],
                                     func=AF.Sqrt, bias=eps5[:csz, :])
                nc.vector.tensor_copy(out=sm[:, 32:33], in_=mv[:, 0:1])
                pst = pp.tile([33, C0], F32, tag="pst", bufs=2)
                nc.tensor.transpose(pst[:, :csz], sm, ident_f[:csz, :csz])
                nc.vector.tensor_copy(
                    out=smrows[0:1, w * NTOK + coff : w * NTOK + coff + csz],
                    in_=pst[0:1, :csz])
                nc.vector.tensor_copy(
                    out=smrows[32:33, w * NTOK + coff : w * NTOK + coff + csz],
                    in_=pst[32:33, :csz])
                rstd = ps.tile([csz, 1], F32, tag=f"rstd{ci}")
                nc.vector.reciprocal(out=rstd, in_=sm[:, 0:1])
                nmr = ps.tile([csz, 1], F32, tag=f"nmr{ci}")
                nc.vector.tensor_mul(out=nmr, in0=mv[:, 0:1], in1=rstd)
                nc.vector.tensor_scalar_mul(out=nmr, in0=nmr, scalar1=-1.0)
                lnw_t = p.tile([csz, DIM], BF16, tag=f"lnw{ci}",
                               bufs=2 if ci == 0 else 1)
                nc.scalar.activation(out=lnw_t, in_=xw, func=AF.Identity,
                                     scale=rstd, bias=nmr)
                for k in range(NK):
                    ptb = pp.tile([128, C0], BF16, tag="ptb")
                    nc.tensor.transpose(ptb[:, :csz],
                                        lnw_t[:, k * 128 : (k + 1) * 128],
                                        ident_b[:csz, :csz])
                    dst = ln1t[k][:, w * NTOK + coff :
                                  w * NTOK + coff + csz]
                    if k % 2:
                        nc.scalar.activation(
                            out=dst, in_=ptb[:, :csz], func=AF.Identity,
                            scale=n1wb[:, k, 0:1], bias=n1wb[:, k, 1:2])
                    else:
                        nc.vector.tensor_scalar(
                            out=dst, in0=ptb[:, :csz],
                            scalar1=n1wb[:, k, 0:1], scalar2=n1wb[:, k, 1:2],
                            op0=ALU.mult, op1=ALU.add)
            if w % 2 == 1 or w == NW - 1:
                nt = w // 2
                c0 = nt * 392
                csz = min(392, T - c0)
                for pair in range(2):
                    for qk, dst in ((0, qa2[pair]), (1, ka2[pair])):
                        pqk = pp.tile([128, 392], F32, tag="pqk", bufs=2)
                        for k in range(NK):
                            nc.tensor.matmul(
                                pqk[:, :csz],
                                lhsT=wqkv[:, qk * NK + k,
                                          pair * 128 : (pair + 1) * 128],
                                rhs=ln1t[k][:, c0 : c0 + csz],
                                start=(k == 0), stop=(k == NK - 1))
                        for h in range(2):
                            b = QK_BASE[h]
                            if h == 0:
                                nc.vector.tensor_copy(
                                    out=dst[h][b : b + HDIM, c0 : c0 + csz],
                                    in_=pqk[b : b + HDIM, :csz])
                            else:
                                nc.scalar.copy(
                                    out=dst[h][b : b + HDIM, c0 : c0 + csz],
                                    in_=pqk[b : b + HDIM, :csz])


# ---------------------------------------------------------------- stage 3
def _s3(nc, tc, ln1t, wqkv, rh_s, rw_s, qa2, ka2, onesb, attn_d, srows):
    with tc.tile_pool(name="s3", bufs=3) as p, \
         tc.tile_pool(name="s3o", bufs=2) as po_p, \
         tc.tile_pool(name="s3ps", bufs=2, space="PSUM") as pp, \
         tc.tile_pool(name="s3ps1", bufs=1, space="PSUM") as pp1:
        HB = HDIM + 1
        for pair in range(HEADS // 2):
            qa = qa2[pair % 2]
            ka = ka2[pair % 2]
            for nt in range(NT45 if pair >= 2 else 0):
                c0 = nt * 392
                csz = min(392, T - c0)
                for qk, dst in ((0, qa), (1, ka)):
                    pqk = pp.tile([128, 392], F32, tag="ps3a", bufs=3)
                    for k in range(NK):
                        nc.tensor.matmul(
                            pqk[:, :csz],
                            lhsT=wqkv[:, qk * NK + k,
                                      pair * 128 : (pair + 1) * 128],
                            rhs=ln1t[k][:, c0 : c0 + csz],
                            start=(k == 0), stop=(k == NK - 1))
                    for h in range(2):
                        b = QK_BASE[h]
                        nc.vector.tensor_copy(
                            out=dst[h][b : b + HDIM, c0 : c0 + csz],
                            in_=pqk[b : b + HDIM, :csz])
            for h in range(2):
                qb = QK_BASE[h]
                qa_v = qa[h].rearrange("p (w a b) -> p w a b", a=WS, b=WS)
                for qh in range(WS):
                    pr = pp.tile([WS, NW * WS], F32, tag="ps3a", bufs=3, name="pr")
                    nc.tensor.matmul(
                        pr.rearrange("p (w b) -> p w b", w=NW),
                        lhsT=rh_s[qb : qb + HDIM, qh * WS : (qh + 1) * WS],
                        rhs=qa_v[qb : qb + HDIM, :, qh, :],
                        start=True, stop=True)
                    if h == 0:
                        nc.vector.tensor_copy(
                            out=qa_v[RH_BASE[h] : RH_BASE[h] + WS, :, qh, :],
                            in_=pr.rearrange("p (w b) -> p w b", w=NW))
                    else:
                        nc.scalar.copy(
                            out=qa_v[RH_BASE[h] : RH_BASE[h] + WS, :, qh, :],
                            in_=pr.rearrange("p (w b) -> p w b", w=NW))
                for qw in range(WS):
                    pr2 = pp.tile([WS, NW * WS], F32, tag="ps3a", bufs=3, name="pr2")
                    nc.tensor.matmul(
                        pr2.rearrange("p (w a) -> p w a", w=NW),
                        lhsT=rw_s[qb : qb + HDIM, qw * WS : (qw + 1) * WS],
                        rhs=qa_v[qb : qb + HDIM, :, :, qw],
                        start=True, stop=True)
                    if h == 0:
                        nc.vector.tensor_copy(
                            out=qa_v[RW_BASE[h] : RW_BASE[h] + WS, :, :, qw],
                            in_=pr2.rearrange("p (w a) -> p w a", w=NW))
                    else:
                        nc.scalar.copy(
                            out=qa_v[RW_BASE[h] : RW_BASE[h] + WS, :, :, qw],
                            in_=pr2.rearrange("p (w a) -> p w a", w=NW))

            srh = po_p.tile([33, T], BF16, tag="srh", bufs=1)
            ao = None
            for w in range(NW):
                if w % 13 == 0:
                    ao = po_p.tile([128, 2548], BF16, tag="ao")
                    w0 = w
                vws = []
                for coff in (0, 98):
                    pv = pp.tile([98, 128], F32, tag="ps3a", bufs=3, name="pv")
                    for k in range(NK):
                        nc.tensor.matmul(
                            pv,
                            lhsT=ln1t[k][:, w * NTOK + coff :
                                         w * NTOK + coff + 98],
                            rhs=wqkv[:, 2 * NK + k,
                                     pair * 128 : (pair + 1) * 128],
                            start=(k == 0), stop=(k == NK - 1))
                    # [v_h0 (64) | 1 | v_h1 (64) | 1] so lhsT slices stay
                    # contiguous and carry the softmax-sums ones column
                    vw = p.tile([98, 2 * HB], BF16,
                                tag=f"vw{coff > 0}", name="vw")
                    vwv = vw.rearrange("p (h c) -> p h c", c=HB)
                    nc.vector.tensor_copy(
                        out=vwv[:, :, 0:HDIM],
                        in_=pv.rearrange("p (h c) -> p h c", c=HDIM))
                    nc.vector.memset(vwv[:, :, HDIM : HDIM + 1], 1.0)
                    vws.append(vw)
                vw0, vw1 = vws
                for h in range(2):
                    st = pp1.tile([98, 2, NTOK], F32, tag="st0", bufs=2)
                    nc.tensor.matmul(
                        st[:, 0, :], lhsT=ka[h][:, w * NTOK : w * NTOK + 98],
                        rhs=qa[h][:, w * NTOK : (w + 1) * NTOK],
                        start=True, stop=True)
                    nc.tensor.matmul(
                        st[:, 1, :],
                        lhsT=ka[h][:, w * NTOK + 98 : w * NTOK + 196],
                        rhs=qa[h][:, w * NTOK : (w + 1) * NTOK],
                        start=True, stop=True)
                    et = p.tile([98, 2, NTOK], BF16, tag="et0")
                    nc.scalar.activation(out=et, in_=st, func=AF.Exp)
                    # O plus a sums row via the ones column of v (row 64)
                    pO = pp1.tile([HB, NTOK], F32, tag="pO", bufs=3)
                    nc.tensor.matmul(pO,
                                     lhsT=vw0[:, h * HB : (h + 1) * HB],
                                     rhs=et[:, 0, :], start=True, stop=False)
                    nc.tensor.matmul(pO,
                                     lhsT=vw1[:, h * HB : (h + 1) * HB],
                                     rhs=et[:, 1, :], start=False, stop=True)
                    cof = (w - w0) * NTOK
                    if h == 0:
                        nc.vector.tensor_copy(
                            out=srh[0:1, w * NTOK : (w + 1) * NTOK],
                            in_=pO[HDIM : HDIM + 1, :])
                        nc.scalar.copy(
                            out=ao[0:HDIM, cof : cof + NTOK],
                            in_=pO[0:HDIM, :])
                    else:
                        nc.scalar.copy(
                            out=srh[32:33, w * NTOK : (w + 1) * NTOK],
                            in_=pO[HDIM : HDIM + 1, :])
                        nc.vector.tensor_copy(
                            out=ao[HDIM:128, cof : cof + NTOK],
                            in_=pO[0:HDIM, :])
                if w in (12, NW - 1):
                    nsz = (w - w0 + 1) * NTOK
                    nc.sync.dma_start(
                        out=attn_d[pair * 128 : (pair + 1) * 128,
                                   w0 * NTOK : w0 * NTOK + nsz],
                        in_=ao[:, :nsz])
            for h in range(2):
                nc.sync.dma_start(
                    out=srows[pair * 2 + h : pair * 2 + h + 1, :],
                    in_=srh[h * 32 : h * 32 + 1, :])


# ---------------------------------------------------------------- stage 4+5
def _s4(nc, tc, wproj_d, n1wbi_d, onesb, ln1t, attn_d, x1_d, srows, smrows,
        e2_d):
    # phase 1 (windowed tokens): proj + residual -> x1_d (window order).
    # x is reconstructed from ln1t via per-token (std, mean) rows.
    with tc.tile_pool(name="s4w", bufs=1) as wp, \
         tc.tile_pool(name="s4", bufs=2) as p, \
         tc.tile_pool(name="s4ps", bufs=2, space="PSUM") as pp, \
         tc.tile_pool(name="s4ps1", bufs=1, space="PSUM") as pp1:
        wproj = wp.tile([128, NK, DIM], BF16, tag="wproj")
        nc.sync.dma_start(out=wproj, in_=wproj_d[:, :, :])
        e2 = wp.tile([HEADS, NK * 128], BF16, tag="e2")
        nc.sync.dma_start(out=e2, in_=e2_d[:, :])
        n1wbi = wp.tile([128, NK, 2], F32, tag="n1wbi")
        nc.sync.dma_start(out=n1wbi, in_=n1wbi_d[:, :, :])
        for nt in range(NT45):
            c0 = nt * 392
            csz = min(392, T - c0)
            at = [p.tile([128, 392], BF16, tag=f"at{k}", name=f"at{k}")
                  for k in range(NK)]
            recb = p.tile([HEADS, 392], BF16, tag="recb")
            with nc.allow_low_precision(reason="softmax denominators"):
                nc.vector.reciprocal(out=recb[:, :csz],
                                     in_=srows[:, c0 : c0 + csz])
            pbs_p = pp1.tile([128, 392], F32, tag="pbs")
            nc.tensor.matmul(pbs_p[:, :csz], lhsT=onesb[0:1, :],
                             rhs=smrows[0:1, c0 : c0 + csz],
                             start=True, stop=True)


            for k in range(NK):
                nc.gpsimd.dma_start(
                    out=at[k][:, :csz],
                    in_=attn_d[k * 128 : (k + 1) * 128, c0 : c0 + csz])
                bck = pp.tile([128, 392], F32, tag="bck")
                nc.tensor.matmul(bck[:, :csz],
                                 lhsT=e2[:, k * 128 : (k + 1) * 128],
                                 rhs=recb[:, :csz], start=True, stop=True)
                nc.vector.tensor_mul(out=at[k][:, :csz], in0=at[k][:, :csz],
                                     in1=bck[:, :csz])
            x1 = p.tile([128, NK, 392], BF16, tag="x1")
            for m in range(NK):
                xs = p.tile([128, 392], F32, tag="xs")
                nc.scalar.activation(
                    out=xs[:, :csz], in_=ln1t[m][:, c0 : c0 + csz],
                    func=AF.Identity,
                    scale=n1wbi[:, m, 0:1], bias=n1wbi[:, m, 1:2])
                xsb = p.tile([128, 392], F32, tag="xsb")
                nc.vector.tensor_mul(out=xsb[:, :csz], in0=xs[:, :csz],
                                     in1=pbs_p[:, :csz])
                pj = pp.tile([128, 392], F32, tag="pj", bufs=3)
                nc.tensor.matmul(pj[:, :csz], lhsT=onesb[32:33, :],
                                 rhs=smrows[32:33, c0 : c0 + csz],
                                 start=True, stop=False)
                for k in range(NK):
                    nc.tensor.matmul(
                        pj[:, :csz],
                        lhsT=wproj[:, k, m * 128 : (m + 1) * 128],
                        rhs=at[k][:, :csz], start=False, stop=(k == NK - 1))
                nc.vector.tensor_add(out=x1[:, m, :csz], in0=pj[:, :csz],
                                     in1=xsb[:, :csz])
            nc.sync.dma_start(
                out=x1_d[:, :, c0 : c0 + csz].rearrange("k p c -> p k c"),
                in_=x1[:, :, :csz])


def _s5(nc, tc, wfc1_d, wfc2_d, n2wb_d, onesb, x1_d, x2p, eps5):
    # phase 2 (window order, 13 tiles of 392): LN2 + MLP + residual -> x2_d
    with tc.tile_pool(name="s5w", bufs=1) as wp, \
         tc.tile_pool(name="s5", bufs=2) as p, \
         tc.tile_pool(name="s5h", bufs=1) as hp, \
         tc.tile_pool(name="s5s", bufs=3) as ps, \
         tc.tile_pool(name="s5ps", bufs=2, space="PSUM") as pp, \
         tc.tile_pool(name="s5ps1", bufs=1, space="PSUM") as pp1:
        f1 = wp.tile([128, NK, MLPD], BF16, tag="f1")
        nc.sync.dma_start(out=f1, in_=wfc1_d[:, :, :])
        f2 = wp.tile([128, 24, DIM], BF16, tag="f2")
        nc.sync.dma_start(out=f2, in_=wfc2_d[:, :, :])
        n2wb = wp.tile([128, NK, 2], F32, tag="n2wb")
        nc.sync.dma_start(out=n2wb, in_=n2wb_d[:, :, :])
        for nt in range(NT45):
            c0 = nt * 392
            csz = min(392, T - c0)
            x1 = p.tile([128, NK, 392], BF16, tag="x1p2")
            nc.sync.dma_start(
                out=x1[:, :, :csz],
                in_=x1_d[:, :, c0 : c0 + csz].rearrange("k p c -> p k c"))
            l2 = p.tile([128, NK, 392], BF16, tag="l2")
            srow = pp1.tile([1, 512], F32, tag="srow")
            ssq = pp1.tile([1, 512], F32, tag="ssq")
            for m in range(NK):
                sqm = ps.tile([128, 392], BF16, bufs=2, tag="sqm")
                nc.scalar.activation(out=sqm, in_=x1[:, m, :], func=AF.Square)
                nc.tensor.matmul(srow[:, :csz], lhsT=onesb[:, 0:1],
                                 rhs=x1[:, m, :csz],
                                 start=(m == 0), stop=(m == NK - 1),
                                 skip_group_check=True)
                nc.tensor.matmul(ssq[:, :csz], lhsT=onesb[:, 0:1],
                                 rhs=sqm[:, :csz],
                                 start=(m == 0), stop=(m == NK - 1),
                                 skip_group_check=True)
            rs_b, nm_b = _ln_rows(nc, ps, srow, ssq, csz, DIM, eps5[0:1, :],
                                  "l2")
            pb1 = pp1.tile([128, 512], F32, tag="pb1")
            nc.tensor.matmul(pb1[:, :csz], lhsT=onesb[0:1, :],
                             rhs=rs_b[:, :csz], start=True, stop=True)
            pb2 = pp1.tile([128, 512], F32, tag="pb2")
            nc.tensor.matmul(pb2[:, :csz], lhsT=onesb[0:1, :],
                             rhs=nm_b[:, :csz], start=True, stop=True)
            for m in range(NK):
                t1 = ps.tile([128, 392], F32, tag="t1")
                nc.vector.tensor_mul(out=t1[:, :csz], in0=x1[:, m, :csz],
                                     in1=pb1[:, :csz])
                nc.vector.tensor_add(out=t1[:, :csz], in0=t1[:, :csz],
                                     in1=pb2[:, :csz])
                if APPLY_AFFINE:
                    nc.vector.tensor_scalar(
                        out=l2[:, m, :csz], in0=t1[:, :csz],
                        scalar1=n2wb[:, m, 0:1], scalar2=n2wb[:, m, 1:2],
                        op0=ALU.mult, op1=ALU.add)
                else:
                    nc.vector.tensor_copy(out=l2[:, m, :csz], in_=t1[:, :csz])
            hb = hp.tile([128, 24, 392], BF16, tag="hb")
            for m24 in range(24):
                pf = pp.tile([128, 392], F32, tag="mm5", bufs=3)
                for k in range(NK):
                    nc.tensor.matmul(
                        pf[:, :csz],
                        lhsT=f1[:, k, m24 * 128 : (m24 + 1) * 128],
                        rhs=l2[:, k, :csz], start=(k == 0), stop=(k == NK - 1))
                nc.scalar.activation(out=hb[:, m24, :csz], in_=pf[:, :csz],
                                     func=AF.Gelu)
            x2 = p.tile([128, NK, 392], BF16, tag="x2")
            for m in range(NK):
                pg = pp.tile([128, 392], F32, tag="mm5", bufs=3)
                for k24 in range(24):
                    nc.tensor.matmul(
                        pg[:, :csz],
                        lhsT=f2[:, k24, m * 128 : (m + 1) * 128],
                        rhs=hb[:, k24, :csz], start=(k24 == 0),
                        stop=(k24 == 23))
                nc.vector.tensor_add(out=x2[:, m, :csz], in0=pg[:, :csz],
                                     in1=x1[:, m, :csz])
            srcv = x2.rearrange("p k (a i j) -> p k a i j", a=2, i=WS)
            for wi in range(2):
                w = nt * 2 + wi
                if w >= NW or wi * NTOK >= csz:
                    continue
                wr, wc, ph, pw = wdims(w)
                for k in range(NK):
                    dst = x2p[k].rearrange("p (r c) -> p r c", r=PAD)[
                        :, 1 + WS * wr : 1 + WS * wr + ph,
                        1 + WS * wc : 1 + WS * wc + pw]
                    if k % 2:
                        nc.scalar.copy(out=dst, in_=srcv[:, k, wi, :ph, :pw])
                    else:
                        nc.vector.tensor_copy(out=dst,
                                              in_=srcv[:, k, wi, :ph, :pw])


# ---------------------------------------------------------------- stage 6
def _s6(nc, tc, w1c_d, w2t_d, w3c_d, lnwb_d, onesb, ident_f,
        ident_b, x2p, y, eps6):
    with tc.tile_pool(name="s6w", bufs=1) as wp, \
         tc.tile_pool(name="s6b", bufs=1) as bigp, \
         tc.tile_pool(name="s6", bufs=2) as p, \
         tc.tile_pool(name="s6s", bufs=3) as ps, \
         tc.tile_pool(name="s6ps", bufs=2, space="PSUM") as pp, \
         tc.tile_pool(name="s6ps1", bufs=1, space="PSUM") as pp1:
        w1s = wp.tile([128, NK, BOT], BF16, tag="w1s")
        nc.sync.dma_start(out=w1s, in_=w1c_d[:, :, :])
        w2s = wp.tile([128, 3, 9, BOT], BF16, tag="w2s")
        nc.sync.dma_start(out=w2s, in_=w2t_d[:, :, :, :])
        w3s = wp.tile([128, 3, DIM], BF16, tag="w3s")
        nc.sync.dma_start(out=w3s, in_=w3c_d[:, :, :])
        lnwb = wp.tile([128, 12, 2], F32, tag="lnwb")
        nc.sync.dma_start(out=lnwb, in_=lnwb_d[:, :, :])

        b1p = [bigp.tile([128, PAD * PAD], BF16, tag=f"b1p{k}", name=f"b1p{k}")
               for k in range(3)]
        b2 = [bigp.tile([128, IMG * IMG], BF16, tag=f"b2_{k}", name=f"b2_{k}")
              for k in range(3)]
        for k in range(3):
            nc.gpsimd.memset(b1p[k], 0.0)
        def convln(rhs_fn, wsel, nk_in, nm_out, taps, eps_ap, nt8):
            seq = [(tap, k) for tap in taps for k in range(nk_in)]
            cms = []
            srow = pp1.tile([1, 512], F32, tag="c_srow")
            ssq = pp1.tile([1, 512], F32, tag="c_ssq")
            for m in range(nm_out):
                pc = pp.tile([128, 512], F32, tag="pc", bufs=3)
                for idx, (tap, k) in enumerate(seq):
                    nc.tensor.matmul(
                        pc.rearrange("p (r c) -> p r c", r=8),
                        lhsT=wsel(k, tap, m),
                        rhs=rhs_fn(k, tap, nt8),
                        start=(idx == 0), stop=(idx == len(seq) - 1))
                cm = ps.tile([128, 512], BF16, bufs=7, tag="cm")
                nc.scalar.copy(out=cm, in_=pc)
                sq = ps.tile([128, 512], BF16, bufs=2, tag="csq")
                nc.scalar.activation(out=sq, in_=pc, func=AF.Square)
                cms.append((cm, sq))
                nc.tensor.matmul(srow, lhsT=onesb[:, 0:1], rhs=cm,
                                 start=(m == 0), stop=(m == nm_out - 1),
                                 skip_group_check=True)
                nc.tensor.matmul(ssq, lhsT=onesb[:, 0:1], rhs=sq,
                                 start=(m == 0), stop=(m == nm_out - 1),
                                 skip_group_check=True)
            rs_b, nm_b = _ln_rows(nc, ps, srow, ssq, 512, nm_out * 128,
                                  eps_ap, "c")
            qb1 = pp1.tile([128, 512], F32, tag="qb1")
            nc.tensor.matmul(qb1, lhsT=onesb[0:1, :], rhs=rs_b, start=True,
                             stop=True)
            qb2 = pp1.tile([128, 512], F32, tag="qb2")
            nc.tensor.matmul(qb2, lhsT=onesb[0:1, :], rhs=nm_b, start=True,
                             stop=True)
            outs = []
            for m in range(nm_out):
                t1 = ps.tile([128, 512], F32, bufs=7, tag="c_t1")
                nc.vector.tensor_mul(out=t1, in0=cms[m][0], in1=qb1)
                nc.vector.tensor_add(out=t1, in0=t1, in1=qb2)
                outs.append(t1)
            return outs

        def x2p_rhs(k, tap, nt8):
            return x2p[k].rearrange("p (r c) -> p r c", r=PAD)[
                :, 1 + nt8 * 8 : 1 + nt8 * 8 + 8, 1 : 1 + IMG]

        def b1p_rhs(k, tap, nt8):
            dy, dx = tap
            return b1p[k].rearrange("p (r c) -> p r c", r=PAD)[
                :, nt8 * 8 + dy : nt8 * 8 + dy + 8, dx : dx + IMG]

        def b2_rhs(k, tap, nt8):
            return b2[k].rearrange("p (r c) -> p r c", r=IMG)[
                :, nt8 * 8 : nt8 * 8 + 8, :]

        for nt8 in range(8):
            outs = convln(
                x2p_rhs,
                lambda k, tap, m: w1s[:, k, m * 128 : (m + 1) * 128],
                NK, 3, [None], eps6[0:1, :], nt8)
            for m in range(3):
                nc.scalar.activation(
                    out=b1p[m].rearrange("p (r c) -> p r c", r=PAD)[
                        :, 1 + nt8 * 8 : 1 + nt8 * 8 + 8, 1 : 1 + IMG],
                    in_=outs[m].rearrange("p (r c) -> p r c", r=8),
                    func=AF.Gelu, scale=lnwb[:, m, 0:1], bias=lnwb[:, m, 1:2])
        taps2 = [(dy, dx) for dy in range(3) for dx in range(3)]
        for nt8 in range(8):
            outs = convln(
                b1p_rhs,
                lambda k, tap, m: w2s[:, k, tap[0] * 3 + tap[1],
                                      m * 128 : (m + 1) * 128],
                3, 3, taps2, eps6[0:1, :], nt8)
            for m in range(3):
                nc.scalar.activation(
                    out=b2[m].rearrange("p (r c) -> p r c", r=IMG)[
                        :, nt8 * 8 : nt8 * 8 + 8, :],
                    in_=outs[m].rearrange("p (r c) -> p r c", r=8),
                    func=AF.Gelu, scale=lnwb[:, 3 + m, 0:1],
                    bias=lnwb[:, 3 + m, 1:2])
        for nt8 in range(8):
            outs = convln(
                b2_rhs,
                lambda k, tap, m: w3s[:, k, m * 128 : (m + 1) * 128],
                3, NK, [None], eps6[0:1, :], nt8)
            om = []
            for m in range(NK):
                fin = ps.tile([128, 512], BF16, bufs=7, tag="c_fin")
                nc.scalar.activation(
                    out=fin, in_=outs[m], func=AF.Identity,
                    scale=lnwb[:, 6 + m, 0:1], bias=lnwb[:, 6 + m, 1:2])
                nc.gpsimd.tensor_add(
                    out=fin.rearrange("p (r c) -> p r c", r=8),
                    in0=fin.rearrange("p (r c) -> p r c", r=8),
                    in1=x2p[m].rearrange("p (r c) -> p r c", r=PAD)[
                        :, 1 + nt8 * 8 : 1 + nt8 * 8 + 8, 1 : 1 + IMG])
                om.append(fin)
            for s in range(4):
                outt = p.tile([128, DIM], BF16, tag="outt")
                for m in range(NK):
                    ptp = pp.tile([128, 128], BF16, tag="ptp", bufs=1)
                    nc.tensor.transpose(ptp, om[m][:, s * 128 : (s + 1) * 128],
                                        ident_b)
                    if m % 2:
                        nc.scalar.copy(
                            out=outt[:, m * 128 : (m + 1) * 128], in_=ptp)
                    else:
                        nc.vector.tensor_copy(
                            out=outt[:, m * 128 : (m + 1) * 128], in_=ptp)
                nc.sync.dma_start(
                    out=y[nt8 * 512 + s * 128 : nt8 * 512 + (s + 1) * 128, :],
                    in_=outt)


# ---------------------------------------------------------------- host side
def _chunk(w, nchunk):
    """[nchunk*128, N] -> [128, nchunk, N] (row r = chunk*128 + p)."""
    n = w.shape[1]
    return np.ascontiguousarray(w.reshape(nchunk, 128, n).transpose(1, 0, 2))


def _prep_weights(inputs):
    bf = ml_dtypes.bfloat16
    f8 = ml_dtypes.float8_e4m3
    qkv_w = np.array(inputs["qkv_w"], np.float32)
    qkv_w[:, :DIM] *= HDIM ** -0.5  # fold score scale into Wq
    # [128, (qk, k), 768]: chunk (qk*NK + k) holds input-rows k*128..+128 of
    # the q/k/v block
    wq = np.stack([qkv_w[k * 128 : (k + 1) * 128, qk * DIM : (qk + 1) * DIM]
                   for qk in range(3) for k in range(NK)], axis=1)
    rel_h = np.array(inputs["rel_pos_h"], np.float32)
    rel_w = np.array(inputs["rel_pos_w"], np.float32)
    coords = np.arange(WS)[:, None] - np.arange(WS)[None, :] + WS - 1
    Rh = rel_h[coords]  # [q, k, c]
    Rw = rel_w[coords]
    # lhsT[c, kh] per qh; x sqrt(HDIM) undoes the q-scale fold
    rh_all = np.concatenate([Rh[i].T * (HDIM ** 0.5) for i in range(WS)],
                            axis=1)
    rw_all = np.concatenate([Rw[i].T * (HDIM ** 0.5) for i in range(WS)],
                            axis=1)
    kh = np.arange(NTOK) // WS
    kw = np.arange(NTOK) % WS
    kon = np.zeros((2 * WS, NTOK), np.float32)
    kon[kh, np.arange(NTOK)] = 1.0
    kon[WS + kw, np.arange(NTOK)] = 1.0
    konst = np.tile(kon, (1, NW))
    e2 = np.zeros((HEADS, NK * 128), np.float32)
    for g in range(HEADS):
        k, half = divmod(g, 2)
        e2[g, k * 128 + half * 64 : k * 128 + half * 64 + 64] = 1.0
    conv2_w = np.array(inputs["conv2_w"], np.float32)
    # [128, (kchunk), (tap), 384]
    w2t = np.stack([conv2_w[:, :, dy, dx].T for dy in range(3)
                    for dx in range(3)])  # [9, 384in, 384out]
    w2t = np.ascontiguousarray(
        w2t.reshape(9, 3, 128, BOT).transpose(2, 1, 0, 3))
    lnw = np.concatenate([np.array(inputs[k], np.float32)
                          for k in ("ln1_w", "ln2_w", "ln3_w")])
    lnb = np.concatenate([np.array(inputs[k], np.float32)
                          for k in ("ln1_b", "ln2_b", "ln3_b")])
    lnwb = np.stack([lnw, lnb], axis=1)  # [1536, 2]
    n2wb = np.stack([np.array(inputs["norm2_w"], np.float32),
                     np.array(inputs["norm2_b"], np.float32)], axis=1)
    d = dict(
        wqkv=np.ascontiguousarray(wq).astype(bf),
        wproj=_chunk(np.array(inputs["proj_w"], np.float32), NK).astype(bf),
        wfc1=_chunk(np.array(inputs["fc1_w"], np.float32), NK).astype(bf),
        wfc2=_chunk(np.array(inputs["fc2_w"], np.float32), 24).astype(bf),
        w1c=_chunk(np.array(inputs["conv1_w"], np.float32)[:, :, 0, 0].T,
                   NK).astype(bf),
        w2t=w2t.astype(bf),
        w3c=_chunk(np.array(inputs["conv3_w"], np.float32)[:, :, 0, 0].T,
                   3).astype(bf),
        rh_all=np.ascontiguousarray(rh_all).astype(bf),
        rw_all=np.ascontiguousarray(rw_all).astype(bf),
        konst=np.ascontiguousarray(konst).astype(bf),
        e2=e2.astype(bf),
        idf=np.eye(128, dtype=np.float32),
        idb=np.eye(128).astype(bf),
        n1wb=_chunk(np.stack([np.array(inputs["norm1_w"], np.float32),
                              np.array(inputs["norm1_b"], np.float32)],
                             axis=1), NK),
        n1wbi=_chunk(np.stack(
            [1.0 / np.array(inputs["norm1_w"], np.float32),
             -np.array(inputs["norm1_b"], np.float32)
             / np.array(inputs["norm1_w"], np.float32)], axis=1), NK),
        n2wb=_chunk(n2wb, NK),
        lnwb=_chunk(lnwb, 12),
    )
    return d


def kernel(**inputs):
    nc = build()
    split_multi_waits(nc)
    wts = _prep_weights(inputs)
    x = np.asarray(inputs["x"], np.float32)
    in_maps = [dict(wts, x=np.ascontiguousarray(x[c])) for c in range(N_CORES)]
    res = run_bass_kernel_spmd(nc, in_maps, core_ids=list(range(N_CORES)))
    return np.stack([np.asarray(res.results[c]["y"], np.float32)
                       .reshape(IMG, IMG, DIM)
                     for c in range(N_CORES)])



# revision 94
# speedup vs baseline: 1.0142x; 1.0000x over previous
"""Trainium2 Bass kernel for the ViTDet-style transformer block
(windowed 14x14 attention w/ decomposed rel-pos, MLP, bottleneck convs).

Sharding: data-parallel over batch — 8 images, one per NeuronCore; no
collectives (windows/MLP/convs are intra-image).

On-chip layout is D-major (channels on partitions, tokens on free dim).
The decomposed rel-pos bias is folded into the score matmul by augmenting
the contraction dim: q rows carry [q (64) | q.Rh (14) | q.Rw (14)] and k
rows carry [k (64) | onehot(kh) | onehot(kw)], padded with zero rows to
K=128. Compute engines cannot cross partitions, so the odd head of each
pair lives at partitions 64..127 with its aug rows at 0..13 / 32..45,
letting every psum->sbuf copy stay partition-aligned. Softmax runs in the
S^T orientation (exp on ACT; key-sums via ones-matmul; 1/sum broadcast by
a K=1 matmul). 3x3 conv = 9 shifted accumulating matmuls over a
zero-padded 66x66 layout. PE transposes only at entry/exit.
"""

import sys

sys.path.insert(0, "/opt/trn_rl_repo")

import numpy as np
import ml_dtypes

import concourse.bass as bass
import concourse.mybir as mybir
from concourse.tile import TileContext
from concourse.bass_utils import run_bass_kernel_spmd

F32 = mybir.dt.float32
BF16 = mybir.dt.bfloat16
F8 = mybir.dt.float8e4
F8SC = 32.0  # fp8 weight pre-scale (undone at activation/copy time)
DR = mybir.MatmulPerfMode.DoubleRow
AF = mybir.ActivationFunctionType
ALU = mybir.AluOpType

DIM = 768
HEADS = 12
HDIM = 64
WS = 14
MLPD = 3072
BOT = 384
IMG = 64
GRID = 5
NW = GRID * GRID  # 25
NTOK = WS * WS  # 196
T = NW * NTOK  # 4900
C0, C1 = 126, 70  # window-token chunks (rows 0..8 / 9..13)
NK = DIM // 128  # 6
NT45 = 13  # token tiles of 392 (last one 196)
N_CORES = 8
PAD = IMG + 2  # 66

APPLY_AFFINE = True

# per-head-parity row layouts inside the [128, T] qa/ka tiles
QK_BASE = (0, 64)  # q/k content rows
RH_BASE = (64, 0)  # q.Rh / onehot(kh) rows (14)
RW_BASE = (96, 32)  # q.Rw / onehot(kw) rows (14)
Z_ROWS = (((78, 96), (110, 128)), ((14, 32), (46, 64)))  # zeroed rows


def wdims(w):
    wr, wc = divmod(w, GRID)
    ph = WS if wr < GRID - 1 else IMG - WS * (GRID - 1)
    pw = WS if wc < GRID - 1 else IMG - WS * (GRID - 1)
    return wr, wc, ph, pw


def split_multi_waits(nc):
    """walrus CoreV3 encodes at most one sync-wait per CTRL-class op; move
    extra waits onto preceding same-engine NoOps."""
    n = 0
    for f in nc.m.functions:
        for bb in f.blocks:
            out = []
            for inst in bb.instructions:
                si = getattr(inst, "sync_info", None)
                waits = list(si.on_wait) if si is not None else []
                if len(waits) > 1:
                    for i, wt in enumerate(waits[:-1]):
                        nop = mybir.InstNoOp(
                            name=f"{inst.name}-wsplit{i}", ins=[], outs=[]
                        )
                        nop.engine = inst.engine
                        nop.sync_info = mybir.SyncInfo(on_wait=[wt], on_update=[])
                        out.append(nop)
                        n += 1
                    inst.sync_info = mybir.SyncInfo(
                        on_wait=[waits[-1]], on_update=list(si.on_update)
                    )
                out.append(inst)
            bb.instructions = out
    return n


def _bcast_part(ap, n):
    return bass.AP(tensor=ap.tensor, offset=ap.offset, ap=[[0, n]] + list(ap.ap))


def _ln_rows(nc, ps, srow, ssq, csz, nch, eps_ap, tag):
    """psum row-sums -> bf16 rows (rstd, -mean*rstd), each [1, csz]."""
    mrow = ps.tile([1, 512], F32, bufs=1, tag=f"{tag}mr")
    nc.scalar.mul(out=mrow[:, :csz], in_=srow[:, :csz], mul=1.0 / nch)
    vrow = ps.tile([1, 512], F32, bufs=1, tag=f"{tag}vr")
    nc.scalar.mul(out=vrow[:, :csz], in_=ssq[:, :csz], mul=1.0 / nch)
    m2 = ps.tile([1, 512], F32, bufs=1, tag=f"{tag}m2")
    nc.vector.tensor_mul(out=m2[:, :csz], in0=mrow[:, :csz], in1=mrow[:, :csz])
    nc.vector.tensor_sub(out=vrow[:, :csz], in0=vrow[:, :csz], in1=m2[:, :csz])
    sdr = ps.tile([1, 512], F32, bufs=1, tag=f"{tag}sd")
    nc.scalar.activation(out=sdr[:, :csz], in_=vrow[:, :csz], func=AF.Sqrt,
                         bias=eps_ap)
    rs = ps.tile([1, 512], F32, bufs=1, tag=f"{tag}rs")
    nc.vector.reciprocal(out=rs[:, :csz], in_=sdr[:, :csz])
    nmr = ps.tile([1, 512], F32, bufs=1, tag=f"{tag}nm")
    nc.vector.tensor_mul(out=nmr[:, :csz], in0=mrow[:, :csz], in1=rs[:, :csz])
    nc.vector.tensor_scalar_mul(out=nmr[:, :csz], in0=nmr[:, :csz],
                                scalar1=-1.0)
    rs_b = ps.tile([1, 512], BF16, bufs=1, tag=f"{tag}rsb")
    nc.vector.tensor_copy(out=rs_b[:, :csz], in_=rs[:, :csz])
    nm_b = ps.tile([1, 512], BF16, bufs=1, tag=f"{tag}nmb")
    nc.vector.tensor_copy(out=nm_b[:, :csz], in_=nmr[:, :csz])
    return rs_b, nm_b


def build(stages=(1, 2, 3, 45, 6)):
    nc = bass.Bass("TRN2", target_bir_lowering=False)

    x = nc.dram_tensor("x", [IMG, IMG, DIM], F32, kind="ExternalInput")
    y = nc.dram_tensor("y", [IMG * IMG, DIM], BF16, kind="ExternalOutput")

    # weights pre-chunked on host to the SBUF layout [128, nchunk, cols]
    wqkv_d = nc.dram_tensor("wqkv", [128, 3 * NK, DIM], BF16,
                            kind="ExternalInput")
    wproj_d = nc.dram_tensor("wproj", [128, NK, DIM], BF16,
                             kind="ExternalInput")
    wfc1_d = nc.dram_tensor("wfc1", [128, NK, MLPD], BF16,
                            kind="ExternalInput")
    wfc2_d = nc.dram_tensor("wfc2", [128, 24, DIM], BF16,
                            kind="ExternalInput")
    w1c_d = nc.dram_tensor("w1c", [128, NK, BOT], BF16, kind="ExternalInput")
    w2t_d = nc.dram_tensor("w2t", [128, 3, 9, BOT], BF16,
                           kind="ExternalInput")
    w3c_d = nc.dram_tensor("w3c", [128, 3, DIM], BF16, kind="ExternalInput")
    rh_d = nc.dram_tensor("rh_all", [HDIM, WS * WS], BF16,
                          kind="ExternalInput")
    rw_d = nc.dram_tensor("rw_all", [HDIM, WS * WS], BF16,
                          kind="ExternalInput")
    kon_d = nc.dram_tensor("konst", [2 * WS, T], BF16, kind="ExternalInput")
    e2_d = nc.dram_tensor("e2", [HEADS, NK * 128], BF16, kind="ExternalInput")
    idf_d = nc.dram_tensor("idf", [128, 128], F32, kind="ExternalInput")
    idb_d = nc.dram_tensor("idb", [128, 128], BF16, kind="ExternalInput")
    n1wb_d = nc.dram_tensor("n1wb", [128, NK, 2], F32, kind="ExternalInput")
    n1wbi_d = nc.dram_tensor("n1wbi", [128, NK, 2], F32,
                             kind="ExternalInput")
    n2wb_d = nc.dram_tensor("n2wb", [128, NK, 2], F32, kind="ExternalInput")
    lnwb_d = nc.dram_tensor("lnwb", [128, 12, 2], F32, kind="ExternalInput")

    attn_d = nc.dram_tensor("attn_d", [DIM, T], BF16)

    with TileContext(nc) as tc:
        with tc.tile_pool(name="const", bufs=1) as constp:
            ident_f = constp.tile([128, 128], F32, tag="idf")
            nc.sync.dma_start(out=ident_f, in_=idf_d[:, :])
            ident_b = constp.tile([128, 128], BF16, tag="idb")
            nc.sync.dma_start(out=ident_b, in_=idb_d[:, :])
            onesb = constp.tile([128, 128], BF16, tag="onesb")
            nc.vector.memset(onesb, 1.0)
            eps5 = constp.tile([128, 1], F32, tag="eps5")
            nc.vector.memset(eps5, 1e-5)
            eps6 = constp.tile([128, 1], F32, tag="eps6")
            nc.vector.memset(eps6, 1e-6)

            with tc.tile_pool(name="srp", bufs=1) as srp:
                srows = srp.tile([HEADS, T], BF16, tag="srows")
                smrows = srp.tile([33, T], BF16, tag="smrows")
                x1_d = nc.dram_tensor("x1_d", [NK, 128, T], BF16)
                with tc.tile_pool(name="attw", bufs=1) as awp:
                    wqkv = awp.tile([128, 3 * NK, DIM], BF16, tag="wqkv")
                    nc.sync.dma_start(out=wqkv, in_=wqkv_d[:, :, :])
                    # rel-pos weights duplicated on partitions 0..63 and
                    # 64..127 so lhsT base matches the rhs base of either
                    # head parity
                    rh_s = awp.tile([128, WS * WS], BF16, tag="rh")
                    rw_s = awp.tile([128, WS * WS], BF16, tag="rw")
                    for b in (0, 64):
                        nc.sync.dma_start(out=rh_s[b : b + HDIM, :],
                                          in_=rh_d[:, :])
                        nc.sync.dma_start(out=rw_s[b : b + HDIM, :],
                                          in_=rw_d[:, :])
                    with tc.tile_pool(name="ln1t", bufs=1) as lp:
                        ln1t = [lp.tile([128, T], BF16, tag=f"ln1t{k}",
                                        name=f"ln1t{k}")
                                for k in range(NK)]
                        with tc.tile_pool(name="qa", bufs=1) as qap:
                            qa2 = [[qap.tile([128, T], BF16, tag=f"qa{h}{s}",
                                             name=f"qa{h}{s}")
                                    for h in range(2)] for s in range(2)]
                            ka2 = [[qap.tile([128, T], BF16, tag=f"ka{h}{s}",
                                             name=f"ka{h}{s}")
                                    for h in range(2)] for s in range(2)]
                            for s in range(2):
                                for h in range(2):
                                    zb = 64 - QK_BASE[h]
                                    nc.vector.memset(
                                        qa2[s][h][zb : zb + 64, :], 0.0)
                                    nc.vector.memset(
                                        ka2[s][h][zb : zb + 64, :], 0.0)
                                    nc.sync.dma_start(
                                        out=ka2[s][h][RH_BASE[h] :
                                                      RH_BASE[h] + WS, :],
                                        in_=kon_d[0:WS, :])
                                    nc.sync.dma_start(
                                        out=ka2[s][h][RW_BASE[h] :
                                                      RW_BASE[h] + WS, :],
                                        in_=kon_d[WS : 2 * WS, :])
                            if 1 in stages:
                                _s1(nc, tc, x, ident_b, ident_f, n1wb_d,
                                    ln1t, eps5, smrows, qa2, ka2, wqkv)
                            if 3 in stages:
                                _s3(nc, tc, ln1t, wqkv, rh_s, rw_s, qa2, ka2,
                                    onesb, attn_d, srows)
                        if 45 in stages:
                            _s4(nc, tc, wproj_d, n1wbi_d, onesb, ln1t,
                                attn_d, x1_d, srows, smrows, e2_d)
            with tc.tile_pool(name="xp", bufs=1) as xpp:
                x2p = [xpp.tile([128, PAD * PAD], BF16, tag=f"x2p{k}",
                                name=f"x2p{k}")
                       for k in range(NK)]
                for k in range(NK):
                    nc.gpsimd.memset(x2p[k], 0.0)
                if 45 in stages:
                    _s5(nc, tc, wfc1_d, wfc2_d, n2wb_d, onesb, x1_d, x2p,
                        eps5)
                if 6 in stages:
                    _s6(nc, tc, w1c_d, w2t_d, w3c_d, lnwb_d, onesb, ident_f,
                        ident_b, x2p, y, eps6)
    return nc


# ---------------------------------------------------------------- stage 1
def _s1(nc, tc, x, ident_b, ident_f, n1wb_d, ln1t, eps5, smrows,
        qa2, ka2, wqkv):
    with tc.tile_pool(name="s1c", bufs=1) as cp, \
         tc.tile_pool(name="s1", bufs=2) as p, \
         tc.tile_pool(name="s1s", bufs=4) as ps, \
         tc.tile_pool(name="s1ps", bufs=4, space="PSUM") as pp:
        n1wb = cp.tile([128, NK, 2], F32, tag="n1wb")
        nc.sync.dma_start(out=n1wb, in_=n1wb_d[:, :, :])
        for w in range(NW):
            wr, wc, ph, pw = wdims(w)
            xw0 = p.tile([C0, DIM], F32, tag="xw0", bufs=3)
            xw1 = p.tile([C1, DIM], F32, tag="xw1", bufs=3)
            if ph == WS and pw == WS:
                nc.gpsimd.dma_start(
                    out=xw0, in_=x[WS * wr : WS * wr + 9,
                                   WS * wc : WS * wc + WS, :])
                nc.gpsimd.dma_start(
                    out=xw1, in_=x[WS * wr + 9 : WS * wr + WS,
                                   WS * wc : WS * wc + WS, :])
            else:
                nc.vector.memset(xw0, 0.0)
                nc.vector.memset(xw1, 0.0)
                for i in range(ph):
                    src = x[WS * wr + i, WS * wc : WS * wc + pw, :]
                    t0 = i * WS
                    if t0 + pw <= C0:
                        nc.gpsimd.dma_start(out=xw0[t0 : t0 + pw, :], in_=src)
                    elif t0 >= C0:
                        nc.gpsimd.dma_start(
                            out=xw1[t0 - C0 : t0 - C0 + pw, :], in_=src)
                    else:
                        sp = C0 - t0
                        nc.gpsimd.dma_start(out=xw0[t0:C0, :],
                                            in_=src[:sp, :])
                        nc.gpsimd.dma_start(out=xw1[0 : pw - sp, :],
                                            in_=src[sp:pw, :])
            for ci, (xw, csz, coff) in enumerate(((xw0, C0, 0), (xw1, C1, C0))):
                stats = ps.tile([csz, 3, 6], F32, tag=f"st{ci}")
                xv = xw.rearrange("p (s d) -> p s d", s=3)
                for s in range(3):
                    nc.vector.bn_stats(out=stats[:, s, :], in_=xv[:, s, :])
                mv = ps.tile([csz, 2], F32, tag=f"mv{ci}")
                nc.vector.bn_aggr(out=mv, in_=stats)
                # per-token (std, mean) -> rows of smrows for the stage-4
                # x reconstruction
                sm = ps.tile([csz, 33], F32, tag=f"sm{ci}")
                nc.scalar.activation(out=sm[:, 0:1], in_=mv[:, 1:2],
                                     func=AF.Sqrt, bias=eps5[:csz, :])
                nc.vector.tensor_copy(out=sm[:, 32:33], in_=mv[:, 0:1])
                pst = pp.tile([33, C0], F32, tag="pst", bufs=2)
                nc.tensor.transpose(pst[:, :csz], sm, ident_f[:csz, :csz])
                nc.vector.tensor_copy(
                    out=smrows[0:1, w * NTOK + coff : w * NTOK + coff + csz],
                    in_=pst[0:1, :csz])
                nc.vector.tensor_copy(
                    out=smrows[32:33, w * NTOK + coff : w * NTOK + coff + csz],
                    in_=pst[32:33, :csz])
                rstd = ps.tile([csz, 1], F32, tag=f"rstd{ci}")
                nc.vector.reciprocal(out=rstd, in_=sm[:, 0:1])
                nmr = ps.tile([csz, 1], F32, tag=f"nmr{ci}")
                nc.vector.tensor_mul(out=nmr, in0=mv[:, 0:1], in1=rstd)
                nc.vector.tensor_scalar_mul(out=nmr, in0=nmr, scalar1=-1.0)
                lnw_t = p.tile([csz, DIM], BF16, tag=f"lnw{ci}",
                               bufs=2 if ci == 0 else 1)
                nc.scalar.activation(out=lnw_t, in_=xw, func=AF.Identity,
                                     scale=rstd, bias=nmr)
                for k in range(NK):
                    ptb = pp.tile([128, C0], BF16, tag="ptb", bufs=3)
                    nc.tensor.transpose(ptb[:, :csz],
                                        lnw_t[:, k * 128 : (k + 1) * 128],
                                        ident_b[:csz, :csz])
                    dst = ln1t[k][:, w * NTOK + coff :
                                  w * NTOK + coff + csz]
                    if k % 2:
                        nc.scalar.activation(
                            out=dst, in_=ptb[:, :csz], func=AF.Identity,
                            scale=n1wb[:, k, 0:1], bias=n1wb[:, k, 1:2])
                    else:
                        nc.vector.tensor_scalar(
                            out=dst, in0=ptb[:, :csz],
                            scalar1=n1wb[:, k, 0:1], scalar2=n1wb[:, k, 1:2],
                            op0=ALU.mult, op1=ALU.add)
            if w % 2 == 1 or w == NW - 1:
                nt = w // 2
                c0 = nt * 392
                csz = min(392, T - c0)
                for pair in range(2):
                    for qk, dst in ((0, qa2[pair]), (1, ka2[pair])):
                        pqk = pp.tile([128, 392], F32, tag="pqk", bufs=3)
                        for k in range(NK):
                            nc.tensor.matmul(
                                pqk[:, :csz],
                                lhsT=wqkv[:, qk * NK + k,
                                          pair * 128 : (pair + 1) * 128],
                                rhs=ln1t[k][:, c0 : c0 + csz],
                                start=(k == 0), stop=(k == NK - 1))
                        for h in range(2):
                            b = QK_BASE[h]
                            if h == 0:
                                nc.vector.tensor_copy(
                                    out=dst[h][b : b + HDIM, c0 : c0 + csz],
                                    in_=pqk[b : b + HDIM, :csz])
                            else:
                                nc.scalar.copy(
                                    out=dst[h][b : b + HDIM, c0 : c0 + csz],
                                    in_=pqk[b : b + HDIM, :csz])


# ---------------------------------------------------------------- stage 3
def _s3(nc, tc, ln1t, wqkv, rh_s, rw_s, qa2, ka2, onesb, attn_d, srows):
    with tc.tile_pool(name="s3", bufs=3) as p, \
         tc.tile_pool(name="s3o", bufs=2) as po_p, \
         tc.tile_pool(name="s3ps", bufs=2, space="PSUM") as pp, \
         tc.tile_pool(name="s3ps1", bufs=1, space="PSUM") as pp1:
        HB = HDIM + 1
        for pair in range(HEADS // 2):
            qa = qa2[pair % 2]
            ka = ka2[pair % 2]
            for nt in range(NT45 if pair >= 2 else 0):
                c0 = nt * 392
                csz = min(392, T - c0)
                for qk, dst in ((0, qa), (1, ka)):
                    pqk = pp.tile([128, 392], F32, tag="ps3a", bufs=3)
                    for k in range(NK):
                        nc.tensor.matmul(
                            pqk[:, :csz],
                            lhsT=wqkv[:, qk * NK + k,
                                      pair * 128 : (pair + 1) * 128],
                            rhs=ln1t[k][:, c0 : c0 + csz],
                            start=(k == 0), stop=(k == NK - 1))
                    for h in range(2):
                        b = QK_BASE[h]
                        nc.vector.tensor_copy(
                            out=dst[h][b : b + HDIM, c0 : c0 + csz],
                            in_=pqk[b : b + HDIM, :csz])
            for h in range(2):
                qb = QK_BASE[h]
                qa_v = qa[h].rearrange("p (w a b) -> p w a b", a=WS, b=WS)
                for qh in range(WS):
                    pr = pp.tile([WS, NW * WS], F32, tag="ps3a", bufs=3, name="pr")
                    nc.tensor.matmul(
                        pr.rearrange("p (w b) -> p w b", w=NW),
                        lhsT=rh_s[qb : qb + HDIM, qh * WS : (qh + 1) * WS],
                        rhs=qa_v[qb : qb + HDIM, :, qh, :],
                        start=True, stop=True)
                    if h == 0:
                        nc.vector.tensor_copy(
                            out=qa_v[RH_BASE[h] : RH_BASE[h] + WS, :, qh, :],
                            in_=pr.rearrange("p (w b) -> p w b", w=NW))
                    else:
                        nc.scalar.copy(
                            out=qa_v[RH_BASE[h] : RH_BASE[h] + WS, :, qh, :],
                            in_=pr.rearrange("p (w b) -> p w b", w=NW))
                for qw in range(WS):
                    pr2 = pp.tile([WS, NW * WS], F32, tag="ps3a", bufs=3, name="pr2")
                    nc.tensor.matmul(
                        pr2.rearrange("p (w a) -> p w a", w=NW),
                        lhsT=rw_s[qb : qb + HDIM, qw * WS : (qw + 1) * WS],
                        rhs=qa_v[qb : qb + HDIM, :, :, qw],
                        start=True, stop=True)
                    if h == 0:
                        nc.vector.tensor_copy(
                            out=qa_v[RW_BASE[h] : RW_BASE[h] + WS, :, :, qw],
                            in_=pr2.rearrange("p (w a) -> p w a", w=NW))
                    else:
                        nc.scalar.copy(
                            out=qa_v[RW_BASE[h] : RW_BASE[h] + WS, :, :, qw],
                            in_=pr2.rearrange("p (w a) -> p w a", w=NW))

            srh = po_p.tile([33, T], BF16, tag="srh", bufs=1)
            ao = None
            for w in range(NW):
                if w % 13 == 0:
                    ao = po_p.tile([128, 2548], BF16, tag="ao")
                    w0 = w
                vws = []
                for coff in (0, 98):
                    pv = pp.tile([98, 128], F32, tag="ps3a", bufs=3, name="pv")
                    for k in range(NK):
                        nc.tensor.matmul(
                            pv,
                            lhsT=ln1t[k][:, w * NTOK + coff :
                                         w * NTOK + coff + 98],
                            rhs=wqkv[:, 2 * NK + k,
                                     pair * 128 : (pair + 1) * 128],
                            start=(k == 0), stop=(k == NK - 1))
                    # [v_h0 (64) | 1 | v_h1 (64) | 1] so lhsT slices stay
                    # contiguous and carry the softmax-sums ones column
                    vw = p.tile([98, 2 * HB], BF16,
                                tag=f"vw{coff > 0}", name="vw")
                    vwv = vw.rearrange("p (h c) -> p h c", c=HB)
                    nc.vector.tensor_copy(
                        out=vwv[:, :, 0:HDIM],
                        in_=pv.rearrange("p (h c) -> p h c", c=HDIM))
                    nc.vector.memset(vwv[:, :, HDIM : HDIM + 1], 1.0)
                    vws.append(vw)
                vw0, vw1 = vws
                for h in range(2):
                    st = pp1.tile([98, 2, NTOK], F32, tag="st0", bufs=2)
                    nc.tensor.matmul(
                        st[:, 0, :], lhsT=ka[h][:, w * NTOK : w * NTOK + 98],
                        rhs=qa[h][:, w * NTOK : (w + 1) * NTOK],
                        start=True, stop=True)
                    nc.tensor.matmul(
                        st[:, 1, :],
                        lhsT=ka[h][:, w * NTOK + 98 : w * NTOK + 196],
                        rhs=qa[h][:, w * NTOK : (w + 1) * NTOK],
                        start=True, stop=True)
                    et = p.tile([98, 2, NTOK], BF16, tag="et0", bufs=4)
                    nc.scalar.activation(out=et, in_=st, func=AF.Exp)
                    # O plus a sums row via the ones column of v (row 64)
                    pO = pp1.tile([HB, NTOK], F32, tag="pO", bufs=3)
                    nc.tensor.matmul(pO,
                                     lhsT=vw0[:, h * HB : (h + 1) * HB],
                                     rhs=et[:, 0, :], start=True, stop=False)
                    nc.tensor.matmul(pO,
                                     lhsT=vw1[:, h * HB : (h + 1) * HB],
                                     rhs=et[:, 1, :], start=False, stop=True)
                    cof = (w - w0) * NTOK
                    if h == 0:
                        nc.vector.tensor_copy(
                            out=srh[0:1, w * NTOK : (w + 1) * NTOK],
                            in_=pO[HDIM : HDIM + 1, :])
                        nc.scalar.copy(
                            out=ao[0:HDIM, cof : cof + NTOK],
                            in_=pO[0:HDIM, :])
                    else:
                        nc.scalar.copy(
                            out=srh[32:33, w * NTOK : (w + 1) * NTOK],
                            in_=pO[HDIM : HDIM + 1, :])
                        nc.vector.tensor_copy(
                            out=ao[HDIM:128, cof : cof + NTOK],
                            in_=pO[0:HDIM, :])
                if w in (12, NW - 1):
                    nsz = (w - w0 + 1) * NTOK
                    nc.sync.dma_start(
                        out=attn_d[pair * 128 : (pair + 1) * 128,
                                   w0 * NTOK : w0 * NTOK + nsz],
                        in_=ao[:, :nsz])
            for h in range(2):
                nc.sync.dma_start(
                    out=srows[pair * 2 + h : pair * 2 + h + 1, :],
                    in_=srh[h * 32 : h * 32 + 1, :])


# ---------------------------------------------------------------- stage 4+5
def _s4(nc, tc, wproj_d, n1wbi_d, onesb, ln1t, attn_d, x1_d, srows, smrows,
        e2_d):
    # phase 1 (windowed tokens): proj + residual -> x1_d (window order).
    # x is reconstructed from ln1t via per-token (std, mean) rows.
    with tc.tile_pool(name="s4w", bufs=1) as wp, \
         tc.tile_pool(name="s4", bufs=2) as p, \
         tc.tile_pool(name="s4ps", bufs=2, space="PSUM") as pp, \
         tc.tile_pool(name="s4ps1", bufs=1, space="PSUM") as pp1:
        wproj = wp.tile([128, NK, DIM], BF16, tag="wproj")
        nc.sync.dma_start(out=wproj, in_=wproj_d[:, :, :])
        e2 = wp.tile([HEADS, NK * 128], BF16, tag="e2")
        nc.sync.dma_start(out=e2, in_=e2_d[:, :])
        n1wbi = wp.tile([128, NK, 2], F32, tag="n1wbi")
        nc.sync.dma_start(out=n1wbi, in_=n1wbi_d[:, :, :])
        for nt in range(NT45):
            c0 = nt * 392
            csz = min(392, T - c0)
            at = [p.tile([128, 392], BF16, tag=f"at{k}", name=f"at{k}")
                  for k in range(NK)]
            recb = p.tile([HEADS, 392], BF16, tag="recb")
            with nc.allow_low_precision(reason="softmax denominators"):
                nc.vector.reciprocal(out=recb[:, :csz],
                                     in_=srows[:, c0 : c0 + csz])
            pbs_p = pp1.tile([128, 392], F32, tag="pbs")
            nc.tensor.matmul(pbs_p[:, :csz], lhsT=onesb[0:1, :],
                             rhs=smrows[0:1, c0 : c0 + csz],
                             start=True, stop=True)


            for k in range(NK):
                nc.gpsimd.dma_start(
                    out=at[k][:, :csz],
                    in_=attn_d[k * 128 : (k + 1) * 128, c0 : c0 + csz])
                bck = pp.tile([128, 392], F32, tag="bck")
                nc.tensor.matmul(bck[:, :csz],
                                 lhsT=e2[:, k * 128 : (k + 1) * 128],
                                 rhs=recb[:, :csz], start=True, stop=True)
                nc.vector.tensor_mul(out=at[k][:, :csz], in0=at[k][:, :csz],
                                     in1=bck[:, :csz])
            x1 = p.tile([128, NK, 392], BF16, tag="x1")
            for m in range(NK):
                xs = p.tile([128, 392], F32, tag="xs")
                nc.scalar.activation(
                    out=xs[:, :csz], in_=ln1t[m][:, c0 : c0 + csz],
                    func=AF.Identity,
                    scale=n1wbi[:, m, 0:1], bias=n1wbi[:, m, 1:2])
                xsb = p.tile([128, 392], F32, tag="xsb")
                nc.vector.tensor_mul(out=xsb[:, :csz], in0=xs[:, :csz],
                                     in1=pbs_p[:, :csz])
                pj = pp.tile([128, 392], F32, tag="pj", bufs=3)
                nc.tensor.matmul(pj[:, :csz], lhsT=onesb[32:33, :],
                                 rhs=smrows[32:33, c0 : c0 + csz],
                                 start=True, stop=False)
                for k in range(NK):
                    nc.tensor.matmul(
                        pj[:, :csz],
                        lhsT=wproj[:, k, m * 128 : (m + 1) * 128],
                        rhs=at[k][:, :csz], start=False, stop=(k == NK - 1))
                nc.vector.tensor_add(out=x1[:, m, :csz], in0=pj[:, :csz],
                                     in1=xsb[:, :csz])
            nc.sync.dma_start(
                out=x1_d[:, :, c0 : c0 + csz].rearrange("k p c -> p k c"),
                in_=x1[:, :, :csz])


def _s5(nc, tc, wfc1_d, wfc2_d, n2wb_d, onesb, x1_d, x2p, eps5):
    # phase 2 (window order, 13 tiles of 392): LN2 + MLP + residual -> x2_d
    with tc.tile_pool(name="s5w", bufs=1) as wp, \
         tc.tile_pool(name="s5", bufs=2) as p, \
         tc.tile_pool(name="s5h", bufs=1) as hp, \
         tc.tile_pool(name="s5s", bufs=3) as ps, \
         tc.tile_pool(name="s5ps", bufs=2, space="PSUM") as pp, \
         tc.tile_pool(name="s5ps1", bufs=1, space="PSUM") as pp1:
        f1 = wp.tile([128, NK, MLPD], BF16, tag="f1")
        nc.sync.dma_start(out=f1, in_=wfc1_d[:, :, :])
        f2 = wp.tile([128, 24, DIM], BF16, tag="f2")
        nc.sync.dma_start(out=f2, in_=wfc2_d[:, :, :])
        n2wb = wp.tile([128, NK, 2], F32, tag="n2wb")
        nc.sync.dma_start(out=n2wb, in_=n2wb_d[:, :, :])
        for nt in range(NT45):
            c0 = nt * 392
            csz = min(392, T - c0)
            x1 = p.tile([128, NK, 392], BF16, tag="x1p2")
            nc.sync.dma_start(
                out=x1[:, :, :csz],
                in_=x1_d[:, :, c0 : c0 + csz].rearrange("k p c -> p k c"))
            l2 = p.tile([128, NK, 392], BF16, tag="l2")
            srow = pp1.tile([1, 512], F32, tag="srow")
            ssq = pp1.tile([1, 512], F32, tag="ssq")
            for m in range(NK):
                sqm = ps.tile([128, 392], BF16, bufs=2, tag="sqm")
                nc.scalar.activation(out=sqm, in_=x1[:, m, :], func=AF.Square)
                nc.tensor.matmul(srow[:, :csz], lhsT=onesb[:, 0:1],
                                 rhs=x1[:, m, :csz],
                                 start=(m == 0), stop=(m == NK - 1),
                                 skip_group_check=True)
                nc.tensor.matmul(ssq[:, :csz], lhsT=onesb[:, 0:1],
                                 rhs=sqm[:, :csz],
                                 start=(m == 0), stop=(m == NK - 1),
                                 skip_group_check=True)
            rs_b, nm_b = _ln_rows(nc, ps, srow, ssq, csz, DIM, eps5[0:1, :],
                                  "l2")
            pb1 = pp1.tile([128, 512], F32, tag="pb1")
            nc.tensor.matmul(pb1[:, :csz], lhsT=onesb[0:1, :],
                             rhs=rs_b[:, :csz], start=True, stop=True)
            pb2 = pp1.tile([128, 512], F32, tag="pb2")
            nc.tensor.matmul(pb2[:, :csz], lhsT=onesb[0:1, :],
                             rhs=nm_b[:, :csz], start=True, stop=True)
            for m in range(NK):
                t1 = ps.tile([128, 392], F32, tag="t1")
                nc.vector.tensor_mul(out=t1[:, :csz], in0=x1[:, m, :csz],
                                     in1=pb1[:, :csz])
                nc.vector.tensor_add(out=t1[:, :csz], in0=t1[:, :csz],
                                     in1=pb2[:, :csz])
                if APPLY_AFFINE:
                    nc.vector.tensor_scalar(
                        out=l2[:, m, :csz], in0=t1[:, :csz],
                        scalar1=n2wb[:, m, 0:1], scalar2=n2wb[:, m, 1:2],
                        op0=ALU.mult, op1=ALU.add)
                else:
                    nc.vector.tensor_copy(out=l2[:, m, :csz], in_=t1[:, :csz])
            hb = hp.tile([128, 24, 392], BF16, tag="hb")
            for m24 in range(24):
                pf = pp.tile([128, 392], F32, tag="mm5", bufs=3)
                for k in range(NK):
                    nc.tensor.matmul(
                        pf[:, :csz],
                        lhsT=f1[:, k, m24 * 128 : (m24 + 1) * 128],
                        rhs=l2[:, k, :csz], start=(k == 0), stop=(k == NK - 1))
                nc.scalar.activation(out=hb[:, m24, :csz], in_=pf[:, :csz],
                                     func=AF.Gelu)
            x2 = p.tile([128, NK, 392], BF16, tag="x2")
            for m in range(NK):
                pg = pp.tile([128, 392], F32, tag="mm5", bufs=3)
                for k24 in range(24):
                    nc.tensor.matmul(
                        pg[:, :csz],
                        lhsT=f2[:, k24, m * 128 : (m + 1) * 128],
                        rhs=hb[:, k24, :csz], start=(k24 == 0),
                        stop=(k24 == 23))
                nc.vector.tensor_add(out=x2[:, m, :csz], in0=pg[:, :csz],
                                     in1=x1[:, m, :csz])
            srcv = x2.rearrange("p k (a i j) -> p k a i j", a=2, i=WS)
            for wi in range(2):
                w = nt * 2 + wi
                if w >= NW or wi * NTOK >= csz:
                    continue
                wr, wc, ph, pw = wdims(w)
                for k in range(NK):
                    dst = x2p[k].rearrange("p (r c) -> p r c", r=PAD)[
                        :, 1 + WS * wr : 1 + WS * wr + ph,
                        1 + WS * wc : 1 + WS * wc + pw]
                    if k % 2:
                        nc.scalar.copy(out=dst, in_=srcv[:, k, wi, :ph, :pw])
                    else:
                        nc.vector.tensor_copy(out=dst,
                                              in_=srcv[:, k, wi, :ph, :pw])


# ---------------------------------------------------------------- stage 6
def _s6(nc, tc, w1c_d, w2t_d, w3c_d, lnwb_d, onesb, ident_f,
        ident_b, x2p, y, eps6):
    with tc.tile_pool(name="s6w", bufs=1) as wp, \
         tc.tile_pool(name="s6b", bufs=1) as bigp, \
         tc.tile_pool(name="s6", bufs=2) as p, \
         tc.tile_pool(name="s6s", bufs=3) as ps, \
         tc.tile_pool(name="s6ps", bufs=2, space="PSUM") as pp, \
         tc.tile_pool(name="s6ps1", bufs=1, space="PSUM") as pp1:
        w1s = wp.tile([128, NK, BOT], BF16, tag="w1s")
        nc.sync.dma_start(out=w1s, in_=w1c_d[:, :, :])
        w2s = wp.tile([128, 3, 9, BOT], BF16, tag="w2s")
        nc.sync.dma_start(out=w2s, in_=w2t_d[:, :, :, :])
        w3s = wp.tile([128, 3, DIM], BF16, tag="w3s")
        nc.sync.dma_start(out=w3s, in_=w3c_d[:, :, :])
        lnwb = wp.tile([128, 12, 2], F32, tag="lnwb")
        nc.sync.dma_start(out=lnwb, in_=lnwb_d[:, :, :])

        b1p = [bigp.tile([128, PAD * PAD], BF16, tag=f"b1p{k}", name=f"b1p{k}")
               for k in range(3)]
        b2 = [bigp.tile([128, IMG * IMG], BF16, tag=f"b2_{k}", name=f"b2_{k}")
              for k in range(3)]
        for k in range(3):
            nc.gpsimd.memset(b1p[k], 0.0)
        def convln(rhs_fn, wsel, nk_in, nm_out, taps, eps_ap, nt8):
            seq = [(tap, k) for tap in taps for k in range(nk_in)]
            cms = []
            srow = pp1.tile([1, 512], F32, tag="c_srow")
            ssq = pp1.tile([1, 512], F32, tag="c_ssq")
            for m in range(nm_out):
                pc = pp.tile([128, 512], F32, tag="pc", bufs=3)
                for idx, (tap, k) in enumerate(seq):
                    nc.tensor.matmul(
                        pc.rearrange("p (r c) -> p r c", r=8),
                        lhsT=wsel(k, tap, m),
                        rhs=rhs_fn(k, tap, nt8),
                        start=(idx == 0), stop=(idx == len(seq) - 1))
                cm = ps.tile([128, 512], BF16, bufs=7, tag="cm")
                nc.scalar.copy(out=cm, in_=pc)
                sq = ps.tile([128, 512], BF16, bufs=2, tag="csq")
                nc.scalar.activation(out=sq, in_=pc, func=AF.Square)
                cms.append((cm, sq))
                nc.tensor.matmul(srow, lhsT=onesb[:, 0:1], rhs=cm,
                                 start=(m == 0), stop=(m == nm_out - 1),
                                 skip_group_check=True)
                nc.tensor.matmul(ssq, lhsT=onesb[:, 0:1], rhs=sq,
                                 start=(m == 0), stop=(m == nm_out - 1),
                                 skip_group_check=True)
            rs_b, nm_b = _ln_rows(nc, ps, srow, ssq, 512, nm_out * 128,
                                  eps_ap, "c")
            qb1 = pp1.tile([128, 512], F32, tag="qb1")
            nc.tensor.matmul(qb1, lhsT=onesb[0:1, :], rhs=rs_b, start=True,
                             stop=True)
            qb2 = pp1.tile([128, 512], F32, tag="qb2")
            nc.tensor.matmul(qb2, lhsT=onesb[0:1, :], rhs=nm_b, start=True,
                             stop=True)
            outs = []
            for m in range(nm_out):
                t1 = ps.tile([128, 512], F32, bufs=7, tag="c_t1")
                nc.vector.tensor_mul(out=t1, in0=cms[m][0], in1=qb1)
                nc.vector.tensor_add(out=t1, in0=t1, in1=qb2)
                outs.append(t1)
            return outs

        def x2p_rhs(k, tap, nt8):
            return x2p[k].rearrange("p (r c) -> p r c", r=PAD)[
                :, 1 + nt8 * 8 : 1 + nt8 * 8 + 8, 1 : 1 + IMG]

        def b1p_rhs(k, tap, nt8):
            dy, dx = tap
            return b1p[k].rearrange("p (r c) -> p r c", r=PAD)[
                :, nt8 * 8 + dy : nt8 * 8 + dy + 8, dx : dx + IMG]

        def b2_rhs(k, tap, nt8):
            return b2[k].rearrange("p (r c) -> p r c", r=IMG)[
                :, nt8 * 8 : nt8 * 8 + 8, :]

        for nt8 in range(8):
            outs = convln(
                x2p_rhs,
                lambda k, tap, m: w1s[:, k, m * 128 : (m + 1) * 128],
                NK, 3, [None], eps6[0:1, :], nt8)
            for m in range(3):
                nc.scalar.activation(
                    out=b1p[m].rearrange("p (r c) -> p r c", r=PAD)[
                        :, 1 + nt8 * 8 : 1 + nt8 * 8 + 8, 1 : 1 + IMG],
                    in_=outs[m].rearrange("p (r c) -> p r c", r=8),
                    func=AF.Gelu, scale=lnwb[:, m, 0:1], bias=lnwb[:, m, 1:2])
        taps2 = [(dy, dx) for dy in range(3) for dx in range(3)]
        for nt8 in range(8):
            outs = convln(
                b1p_rhs,
                lambda k, tap, m: w2s[:, k, tap[0] * 3 + tap[1],
                                      m * 128 : (m + 1) * 128],
                3, 3, taps2, eps6[0:1, :], nt8)
            for m in range(3):
                nc.scalar.activation(
                    out=b2[m].rearrange("p (r c) -> p r c", r=IMG)[
                        :, nt8 * 8 : nt8 * 8 + 8, :],
                    in_=outs[m].rearrange("p (r c) -> p r c", r=8),
                    func=AF.Gelu, scale=lnwb[:, 3 + m, 0:1],
                    bias=lnwb[:, 3 + m, 1:2])
        for nt8 in range(8):
            outs = convln(
                b2_rhs,
                lambda k, tap, m: w3s[:, k, m * 128 : (m + 1) * 128],
                3, NK, [None], eps6[0:1, :], nt8)
            om = []
            for m in range(NK):
                fin = ps.tile([128, 512], BF16, bufs=7, tag="c_fin")
                nc.scalar.activation(
                    out=fin, in_=outs[m], func=AF.Identity,
                    scale=lnwb[:, 6 + m, 0:1], bias=lnwb[:, 6 + m, 1:2])
                nc.gpsimd.tensor_add(
                    out=fin.rearrange("p (r c) -> p r c", r=8),
                    in0=fin.rearrange("p (r c) -> p r c", r=8),
                    in1=x2p[m].rearrange("p (r c) -> p r c", r=PAD)[
                        :, 1 + nt8 * 8 : 1 + nt8 * 8 + 8, 1 : 1 + IMG])
                om.append(fin)
            for s in range(4):
                outt = p.tile([128, DIM], BF16, tag="outt")
                for m in range(NK):
                    ptp = pp.tile([128, 128], BF16, tag="ptp", bufs=1)
                    nc.tensor.transpose(ptp, om[m][:, s * 128 : (s + 1) * 128],
                                        ident_b)
                    if m % 2:
                        nc.scalar.copy(
                            out=outt[:, m * 128 : (m + 1) * 128], in_=ptp)
                    else:
                        nc.vector.tensor_copy(
                            out=outt[:, m * 128 : (m + 1) * 128], in_=ptp)
                nc.sync.dma_start(
                    out=y[nt8 * 512 + s * 128 : nt8 * 512 + (s + 1) * 128, :],
                    in_=outt)


# ---------------------------------------------------------------- host side
def _chunk(w, nchunk):
    """[nchunk*128, N] -> [128, nchunk, N] (row r = chunk*128 + p)."""
    n = w.shape[1]
    return np.ascontiguousarray(w.reshape(nchunk, 128, n).transpose(1, 0, 2))


def _prep_weights(inputs):
    bf = ml_dtypes.bfloat16
    f8 = ml_dtypes.float8_e4m3
    qkv_w = np.array(inputs["qkv_w"], np.float32)
    qkv_w[:, :DIM] *= HDIM ** -0.5  # fold score scale into Wq
    # [128, (qk, k), 768]: chunk (qk*NK + k) holds input-rows k*128..+128 of
    # the q/k/v block
    wq = np.stack([qkv_w[k * 128 : (k + 1) * 128, qk * DIM : (qk + 1) * DIM]
                   for qk in range(3) for k in range(NK)], axis=1)
    rel_h = np.array(inputs["rel_pos_h"], np.float32)
    rel_w = np.array(inputs["rel_pos_w"], np.float32)
    coords = np.arange(WS)[:, None] - np.arange(WS)[None, :] + WS - 1
    Rh = rel_h[coords]  # [q, k, c]
    Rw = rel_w[coords]
    # lhsT[c, kh] per qh; x sqrt(HDIM) undoes the q-scale fold
    rh_all = np.concatenate([Rh[i].T * (HDIM ** 0.5) for i in range(WS)],
                            axis=1)
    rw_all = np.concatenate([Rw[i].T * (HDIM ** 0.5) for i in range(WS)],
                            axis=1)
    kh = np.arange(NTOK) // WS
    kw = np.arange(NTOK) % WS
    kon = np.zeros((2 * WS, NTOK), np.float32)
    kon[kh, np.arange(NTOK)] = 1.0
    kon[WS + kw, np.arange(NTOK)] = 1.0
    konst = np.tile(kon, (1, NW))
    e2 = np.zeros((HEADS, NK * 128), np.float32)
    for g in range(HEADS):
        k, half = divmod(g, 2)
        e2[g, k * 128 + half * 64 : k * 128 + half * 64 + 64] = 1.0
    conv2_w = np.array(inputs["conv2_w"], np.float32)
    # [128, (kchunk), (tap), 384]
    w2t = np.stack([conv2_w[:, :, dy, dx].T for dy in range(3)
                    for dx in range(3)])  # [9, 384in, 384out]
    w2t = np.ascontiguousarray(
        w2t.reshape(9, 3, 128, BOT).transpose(2, 1, 0, 3))
    lnw = np.concatenate([np.array(inputs[k], np.float32)
                          for k in ("ln1_w", "ln2_w", "ln3_w")])
    lnb = np.concatenate([np.array(inputs[k], np.float32)
                          for k in ("ln1_b", "ln2_b", "ln3_b")])
    lnwb = np.stack([lnw, lnb], axis=1)  # [1536, 2]
    n2wb = np.stack([np.array(inputs["norm2_w"], np.float32),
                     np.array(inputs["norm2_b"], np.float32)], axis=1)
    d = dict(
        wqkv=np.ascontiguousarray(wq).astype(bf),
        wproj=_chunk(np.array(inputs["proj_w"], np.float32), NK).astype(bf),
        wfc1=_chunk(np.array(inputs["fc1_w"], np.float32), NK).astype(bf),
        wfc2=_chunk(np.array(inputs["fc2_w"], np.float32), 24).astype(bf),
        w1c=_chunk(np.array(inputs["conv1_w"], np.float32)[:, :, 0, 0].T,
                   NK).astype(bf),
        w2t=w2t.astype(bf),
        w3c=_chunk(np.array(inputs["conv3_w"], np.float32)[:, :, 0, 0].T,
                   3).astype(bf),
        rh_all=np.ascontiguousarray(rh_all).astype(bf),
        rw_all=np.ascontiguousarray(rw_all).astype(bf),
        konst=np.ascontiguousarray(konst).astype(bf),
        e2=e2.astype(bf),
        idf=np.eye(128, dtype=np.float32),
        idb=np.eye(128).astype(bf),
        n1wb=_chunk(np.stack([np.array(inputs["norm1_w"], np.float32),
                              np.array(inputs["norm1_b"], np.float32)],
                             axis=1), NK),
        n1wbi=_chunk(np.stack(
            [1.0 / np.array(inputs["norm1_w"], np.float32),
             -np.array(inputs["norm1_b"], np.float32)
             / np.array(inputs["norm1_w"], np.float32)], axis=1), NK),
        n2wb=_chunk(n2wb, NK),
        lnwb=_chunk(lnwb, 12),
    )
    return d


def kernel(**inputs):
    nc = build()
    split_multi_waits(nc)
    wts = _prep_weights(inputs)
    x = np.asarray(inputs["x"], np.float32)
    in_maps = [dict(wts, x=np.ascontiguousarray(x[c])) for c in range(N_CORES)]
    res = run_bass_kernel_spmd(nc, in_maps, core_ids=list(range(N_CORES)))
    return np.stack([np.asarray(res.results[c]["y"], np.float32)
                       .reshape(IMG, IMG, DIM)
                     for c in range(N_CORES)])



# revision 96
# speedup vs baseline: 1.0143x; 1.0001x over previous
"""Trainium2 Bass kernel for the ViTDet-style transformer block
(windowed 14x14 attention w/ decomposed rel-pos, MLP, bottleneck convs).

Sharding: data-parallel over batch — 8 images, one per NeuronCore; no
collectives (windows/MLP/convs are intra-image).

On-chip layout is D-major (channels on partitions, tokens on free dim).
The decomposed rel-pos bias is folded into the score matmul by augmenting
the contraction dim: q rows carry [q (64) | q.Rh (14) | q.Rw (14)] and k
rows carry [k (64) | onehot(kh) | onehot(kw)], padded with zero rows to
K=128. Compute engines cannot cross partitions, so the odd head of each
pair lives at partitions 64..127 with its aug rows at 0..13 / 32..45,
letting every psum->sbuf copy stay partition-aligned. Softmax runs in the
S^T orientation (exp on ACT; key-sums via ones-matmul; 1/sum broadcast by
a K=1 matmul). 3x3 conv = 9 shifted accumulating matmuls over a
zero-padded 66x66 layout. PE transposes only at entry/exit.
"""

import sys

sys.path.insert(0, "/opt/trn_rl_repo")

import numpy as np
import ml_dtypes

import concourse.bass as bass
import concourse.mybir as mybir
from concourse.tile import TileContext
from concourse.bass_utils import run_bass_kernel_spmd

F32 = mybir.dt.float32
BF16 = mybir.dt.bfloat16
F8 = mybir.dt.float8e4
F8SC = 32.0  # fp8 weight pre-scale (undone at activation/copy time)
DR = mybir.MatmulPerfMode.DoubleRow
AF = mybir.ActivationFunctionType
ALU = mybir.AluOpType

DIM = 768
HEADS = 12
HDIM = 64
WS = 14
MLPD = 3072
BOT = 384
IMG = 64
GRID = 5
NW = GRID * GRID  # 25
NTOK = WS * WS  # 196
T = NW * NTOK  # 4900
C0, C1 = 126, 70  # window-token chunks (rows 0..8 / 9..13)
NK = DIM // 128  # 6
NT45 = 13  # token tiles of 392 (last one 196)
N_CORES = 8
PAD = IMG + 2  # 66

APPLY_AFFINE = True

# per-head-parity row layouts inside the [128, T] qa/ka tiles
QK_BASE = (0, 64)  # q/k content rows
RH_BASE = (64, 0)  # q.Rh / onehot(kh) rows (14)
RW_BASE = (96, 32)  # q.Rw / onehot(kw) rows (14)
Z_ROWS = (((78, 96), (110, 128)), ((14, 32), (46, 64)))  # zeroed rows


def wdims(w):
    wr, wc = divmod(w, GRID)
    ph = WS if wr < GRID - 1 else IMG - WS * (GRID - 1)
    pw = WS if wc < GRID - 1 else IMG - WS * (GRID - 1)
    return wr, wc, ph, pw


def split_multi_waits(nc):
    """walrus CoreV3 encodes at most one sync-wait per CTRL-class op; move
    extra waits onto preceding same-engine NoOps."""
    n = 0
    for f in nc.m.functions:
        for bb in f.blocks:
            out = []
            for inst in bb.instructions:
                si = getattr(inst, "sync_info", None)
                waits = list(si.on_wait) if si is not None else []
                if len(waits) > 1:
                    for i, wt in enumerate(waits[:-1]):
                        nop = mybir.InstNoOp(
                            name=f"{inst.name}-wsplit{i}", ins=[], outs=[]
                        )
                        nop.engine = inst.engine
                        nop.sync_info = mybir.SyncInfo(on_wait=[wt], on_update=[])
                        out.append(nop)
                        n += 1
                    inst.sync_info = mybir.SyncInfo(
                        on_wait=[waits[-1]], on_update=list(si.on_update)
                    )
                out.append(inst)
            bb.instructions = out
    return n


def _bcast_part(ap, n):
    return bass.AP(tensor=ap.tensor, offset=ap.offset, ap=[[0, n]] + list(ap.ap))


def _ln_rows(nc, ps, srow, ssq, csz, nch, eps_ap, tag):
    """psum row-sums -> bf16 rows (rstd, -mean*rstd), each [1, csz]."""
    mrow = ps.tile([1, 512], F32, bufs=1, tag=f"{tag}mr")
    nc.scalar.mul(out=mrow[:, :csz], in_=srow[:, :csz], mul=1.0 / nch)
    vrow = ps.tile([1, 512], F32, bufs=1, tag=f"{tag}vr")
    nc.scalar.mul(out=vrow[:, :csz], in_=ssq[:, :csz], mul=1.0 / nch)
    m2 = ps.tile([1, 512], F32, bufs=1, tag=f"{tag}m2")
    nc.vector.tensor_mul(out=m2[:, :csz], in0=mrow[:, :csz], in1=mrow[:, :csz])
    nc.vector.tensor_sub(out=vrow[:, :csz], in0=vrow[:, :csz], in1=m2[:, :csz])
    sdr = ps.tile([1, 512], F32, bufs=1, tag=f"{tag}sd")
    nc.scalar.activation(out=sdr[:, :csz], in_=vrow[:, :csz], func=AF.Sqrt,
                         bias=eps_ap)
    rs = ps.tile([1, 512], F32, bufs=1, tag=f"{tag}rs")
    nc.vector.reciprocal(out=rs[:, :csz], in_=sdr[:, :csz])
    nmr = ps.tile([1, 512], F32, bufs=1, tag=f"{tag}nm")
    nc.vector.tensor_mul(out=nmr[:, :csz], in0=mrow[:, :csz], in1=rs[:, :csz])
    nc.vector.tensor_scalar_mul(out=nmr[:, :csz], in0=nmr[:, :csz],
                                scalar1=-1.0)
    rs_b = ps.tile([1, 512], BF16, bufs=1, tag=f"{tag}rsb")
    nc.vector.tensor_copy(out=rs_b[:, :csz], in_=rs[:, :csz])
    nm_b = ps.tile([1, 512], BF16, bufs=1, tag=f"{tag}nmb")
    nc.vector.tensor_copy(out=nm_b[:, :csz], in_=nmr[:, :csz])
    return rs_b, nm_b


def build(stages=(1, 2, 3, 45, 6)):
    nc = bass.Bass("TRN2", target_bir_lowering=False)

    x = nc.dram_tensor("x", [IMG, IMG, DIM], F32, kind="ExternalInput")
    y = nc.dram_tensor("y", [IMG * IMG, DIM], BF16, kind="ExternalOutput")

    # weights pre-chunked on host to the SBUF layout [128, nchunk, cols]
    wqkv_d = nc.dram_tensor("wqkv", [128, 3 * NK, DIM], BF16,
                            kind="ExternalInput")
    wproj_d = nc.dram_tensor("wproj", [128, NK, DIM], BF16,
                             kind="ExternalInput")
    wfc1_d = nc.dram_tensor("wfc1", [128, NK, MLPD], BF16,
                            kind="ExternalInput")
    wfc2_d = nc.dram_tensor("wfc2", [128, 24, DIM], BF16,
                            kind="ExternalInput")
    w1c_d = nc.dram_tensor("w1c", [128, NK, BOT], BF16, kind="ExternalInput")
    w2t_d = nc.dram_tensor("w2t", [128, 3, 9, BOT], BF16,
                           kind="ExternalInput")
    w3c_d = nc.dram_tensor("w3c", [128, 3, DIM], BF16, kind="ExternalInput")
    rh_d = nc.dram_tensor("rh_all", [HDIM, WS * WS], BF16,
                          kind="ExternalInput")
    rw_d = nc.dram_tensor("rw_all", [HDIM, WS * WS], BF16,
                          kind="ExternalInput")
    kon_d = nc.dram_tensor("konst", [2 * WS, T], BF16, kind="ExternalInput")
    e2_d = nc.dram_tensor("e2", [HEADS, NK * 128], BF16, kind="ExternalInput")
    idf_d = nc.dram_tensor("idf", [128, 128], F32, kind="ExternalInput")
    idb_d = nc.dram_tensor("idb", [128, 128], BF16, kind="ExternalInput")
    n1wb_d = nc.dram_tensor("n1wb", [128, NK, 2], F32, kind="ExternalInput")
    n1wbi_d = nc.dram_tensor("n1wbi", [128, NK, 2], F32,
                             kind="ExternalInput")
    n2wb_d = nc.dram_tensor("n2wb", [128, NK, 2], F32, kind="ExternalInput")
    lnwb_d = nc.dram_tensor("lnwb", [128, 12, 2], F32, kind="ExternalInput")

    attn_d = nc.dram_tensor("attn_d", [DIM, T], BF16)

    with TileContext(nc) as tc:
        with tc.tile_pool(name="const", bufs=1) as constp:
            ident_f = constp.tile([128, 128], F32, tag="idf")
            nc.sync.dma_start(out=ident_f, in_=idf_d[:, :])
            ident_b = constp.tile([128, 128], BF16, tag="idb")
            nc.sync.dma_start(out=ident_b, in_=idb_d[:, :])
            onesb = constp.tile([128, 128], BF16, tag="onesb")
            nc.vector.memset(onesb, 1.0)
            eps5 = constp.tile([128, 1], F32, tag="eps5")
            nc.vector.memset(eps5, 1e-5)
            eps6 = constp.tile([128, 1], F32, tag="eps6")
            nc.vector.memset(eps6, 1e-6)

            with tc.tile_pool(name="srp", bufs=1) as srp:
                srows = srp.tile([HEADS, T], BF16, tag="srows")
                smrows = srp.tile([33, T], BF16, tag="smrows")
                x1_d = nc.dram_tensor("x1_d", [NK, 128, T], BF16)
                with tc.tile_pool(name="attw", bufs=1) as awp:
                    wqkv = awp.tile([128, 3 * NK, DIM], BF16, tag="wqkv")
                    nc.sync.dma_start(out=wqkv, in_=wqkv_d[:, :, :])
                    # rel-pos weights duplicated on partitions 0..63 and
                    # 64..127 so lhsT base matches the rhs base of either
                    # head parity
                    rh_s = awp.tile([128, WS * WS], BF16, tag="rh")
                    rw_s = awp.tile([128, WS * WS], BF16, tag="rw")
                    for b in (0, 64):
                        nc.sync.dma_start(out=rh_s[b : b + HDIM, :],
                                          in_=rh_d[:, :])
                        nc.sync.dma_start(out=rw_s[b : b + HDIM, :],
                                          in_=rw_d[:, :])
                    with tc.tile_pool(name="ln1t", bufs=1) as lp:
                        ln1t = [lp.tile([128, T], BF16, tag=f"ln1t{k}",
                                        name=f"ln1t{k}")
                                for k in range(NK)]
                        with tc.tile_pool(name="qa", bufs=1) as qap:
                            qa2 = [[qap.tile([128, T], BF16, tag=f"qa{h}{s}",
                                             name=f"qa{h}{s}")
                                    for h in range(2)] for s in range(2)]
                            ka2 = [[qap.tile([128, T], BF16, tag=f"ka{h}{s}",
                                             name=f"ka{h}{s}")
                                    for h in range(2)] for s in range(2)]
                            for s in range(2):
                                for h in range(2):
                                    zb = 64 - QK_BASE[h]
                                    nc.vector.memset(
                                        qa2[s][h][zb : zb + 64, :], 0.0)
                                    nc.vector.memset(
                                        ka2[s][h][zb : zb + 64, :], 0.0)
                                    nc.sync.dma_start(
                                        out=ka2[s][h][RH_BASE[h] :
                                                      RH_BASE[h] + WS, :],
                                        in_=kon_d[0:WS, :])
                                    nc.sync.dma_start(
                                        out=ka2[s][h][RW_BASE[h] :
                                                      RW_BASE[h] + WS, :],
                                        in_=kon_d[WS : 2 * WS, :])
                            if 1 in stages:
                                _s1(nc, tc, x, ident_b, ident_f, n1wb_d,
                                    ln1t, eps5, smrows, qa2, ka2, wqkv)
                            if 3 in stages:
                                _s3(nc, tc, ln1t, wqkv, rh_s, rw_s, qa2, ka2,
                                    onesb, attn_d, srows)
                        if 45 in stages:
                            _s4(nc, tc, wproj_d, n1wbi_d, onesb, ln1t,
                                attn_d, x1_d, srows, smrows, e2_d)
            with tc.tile_pool(name="xp", bufs=1) as xpp:
                x2p = [xpp.tile([128, PAD * PAD], BF16, tag=f"x2p{k}",
                                name=f"x2p{k}")
                       for k in range(NK)]
                for k in range(NK):
                    nc.gpsimd.memset(x2p[k], 0.0)
                if 45 in stages:
                    _s5(nc, tc, wfc1_d, wfc2_d, n2wb_d, onesb, x1_d, x2p,
                        eps5)
                if 6 in stages:
                    _s6(nc, tc, w1c_d, w2t_d, w3c_d, lnwb_d, onesb, ident_f,
                        ident_b, x2p, y, eps6)
    return nc


# ---------------------------------------------------------------- stage 1
def _s1(nc, tc, x, ident_b, ident_f, n1wb_d, ln1t, eps5, smrows,
        qa2, ka2, wqkv):
    with tc.tile_pool(name="s1c", bufs=1) as cp, \
         tc.tile_pool(name="s1", bufs=2) as p, \
         tc.tile_pool(name="s1s", bufs=4) as ps, \
         tc.tile_pool(name="s1ps", bufs=4, space="PSUM") as pp:
        n1wb = cp.tile([128, NK, 2], F32, tag="n1wb")
        nc.sync.dma_start(out=n1wb, in_=n1wb_d[:, :, :])
        for w in range(NW):
            wr, wc, ph, pw = wdims(w)
            xw0 = p.tile([C0, DIM], F32, tag="xw0", bufs=3)
            xw1 = p.tile([C1, DIM], F32, tag="xw1", bufs=3)
            if ph == WS and pw == WS:
                nc.gpsimd.dma_start(
                    out=xw0, in_=x[WS * wr : WS * wr + 9,
                                   WS * wc : WS * wc + WS, :])
                nc.gpsimd.dma_start(
                    out=xw1, in_=x[WS * wr + 9 : WS * wr + WS,
                                   WS * wc : WS * wc + WS, :])
            else:
                nc.vector.memset(xw0, 0.0)
                nc.vector.memset(xw1, 0.0)
                for i in range(ph):
                    src = x[WS * wr + i, WS * wc : WS * wc + pw, :]
                    t0 = i * WS
                    if t0 + pw <= C0:
                        nc.gpsimd.dma_start(out=xw0[t0 : t0 + pw, :], in_=src)
                    elif t0 >= C0:
                        nc.gpsimd.dma_start(
                            out=xw1[t0 - C0 : t0 - C0 + pw, :], in_=src)
                    else:
                        sp = C0 - t0
                        nc.gpsimd.dma_start(out=xw0[t0:C0, :],
                                            in_=src[:sp, :])
                        nc.gpsimd.dma_start(out=xw1[0 : pw - sp, :],
                                            in_=src[sp:pw, :])
            for ci, (xw, csz, coff) in enumerate(((xw0, C0, 0), (xw1, C1, C0))):
                stats = ps.tile([csz, 3, 6], F32, tag=f"st{ci}")
                xv = xw.rearrange("p (s d) -> p s d", s=3)
                for s in range(3):
                    nc.vector.bn_stats(out=stats[:, s, :], in_=xv[:, s, :])
                mv = ps.tile([csz, 2], F32, tag=f"mv{ci}")
                nc.vector.bn_aggr(out=mv, in_=stats)
                # per-token (std, mean) -> rows of smrows for the stage-4
                # x reconstruction
                sm = ps.tile([csz, 33], F32, tag=f"sm{ci}")
                nc.scalar.activation(out=sm[:, 0:1], in_=mv[:, 1:2],
                                     func=AF.Sqrt, bias=eps5[:csz, :])
                nc.vector.tensor_copy(out=sm[:, 32:33], in_=mv[:, 0:1])
                pst = pp.tile([33, C0], F32, tag="pst", bufs=2)
                nc.tensor.transpose(pst[:, :csz], sm, ident_f[:csz, :csz])
                nc.vector.tensor_copy(
                    out=smrows[0:1, w * NTOK + coff : w * NTOK + coff + csz],
                    in_=pst[0:1, :csz])
                nc.vector.tensor_copy(
                    out=smrows[32:33, w * NTOK + coff : w * NTOK + coff + csz],
                    in_=pst[32:33, :csz])
                rstd = ps.tile([csz, 1], F32, tag=f"rstd{ci}")
                nc.vector.reciprocal(out=rstd, in_=sm[:, 0:1])
                nmr = ps.tile([csz, 1], F32, tag=f"nmr{ci}")
                nc.vector.tensor_mul(out=nmr, in0=mv[:, 0:1], in1=rstd)
                nc.vector.tensor_scalar_mul(out=nmr, in0=nmr, scalar1=-1.0)
                lnw_t = p.tile([csz, DIM], BF16, tag=f"lnw{ci}",
                               bufs=2 if ci == 0 else 1)
                nc.scalar.activation(out=lnw_t, in_=xw, func=AF.Identity,
                                     scale=rstd, bias=nmr)
                for k in range(NK):
                    ptb = pp.tile([128, C0], BF16, tag="ptb", bufs=3)
                    nc.tensor.transpose(ptb[:, :csz],
                                        lnw_t[:, k * 128 : (k + 1) * 128],
                                        ident_b[:csz, :csz])
                    dst = ln1t[k][:, w * NTOK + coff :
                                  w * NTOK + coff + csz]
                    if k % 2:
                        nc.scalar.activation(
                            out=dst, in_=ptb[:, :csz], func=AF.Identity,
                            scale=n1wb[:, k, 0:1], bias=n1wb[:, k, 1:2])
                    else:
                        nc.vector.tensor_scalar(
                            out=dst, in0=ptb[:, :csz],
                            scalar1=n1wb[:, k, 0:1], scalar2=n1wb[:, k, 1:2],
                            op0=ALU.mult, op1=ALU.add)
            if w % 2 == 1 or w == NW - 1:
                nt = w // 2
                c0 = nt * 392
                csz = min(392, T - c0)
                for pair in range(2):
                    for qk, dst in ((0, qa2[pair]), (1, ka2[pair])):
                        pqk = pp.tile([128, 392], F32, tag="pqk", bufs=3)
                        for k in range(NK):
                            nc.tensor.matmul(
                                pqk[:, :csz],
                                lhsT=wqkv[:, qk * NK + k,
                                          pair * 128 : (pair + 1) * 128],
                                rhs=ln1t[k][:, c0 : c0 + csz],
                                start=(k == 0), stop=(k == NK - 1))
                        for h in range(2):
                            b = QK_BASE[h]
                            if h == 0:
                                nc.vector.tensor_copy(
                                    out=dst[h][b : b + HDIM, c0 : c0 + csz],
                                    in_=pqk[b : b + HDIM, :csz])
                            else:
                                nc.scalar.copy(
                                    out=dst[h][b : b + HDIM, c0 : c0 + csz],
                                    in_=pqk[b : b + HDIM, :csz])


# ---------------------------------------------------------------- stage 3
def _s3(nc, tc, ln1t, wqkv, rh_s, rw_s, qa2, ka2, onesb, attn_d, srows):
    with tc.tile_pool(name="s3", bufs=3) as p, \
         tc.tile_pool(name="s3o", bufs=2) as po_p, \
         tc.tile_pool(name="s3ps", bufs=2, space="PSUM") as pp, \
         tc.tile_pool(name="s3ps1", bufs=1, space="PSUM") as pp1:
        HB = HDIM + 1
        for pair in range(HEADS // 2):
            qa = qa2[pair % 2]
            ka = ka2[pair % 2]
            for nt in range(NT45 if pair >= 2 else 0):
                c0 = nt * 392
                csz = min(392, T - c0)
                for qk, dst in ((0, qa), (1, ka)):
                    pqk = pp.tile([128, 392], F32, tag="ps3a", bufs=3)
                    for k in range(NK):
                        nc.tensor.matmul(
                            pqk[:, :csz],
                            lhsT=wqkv[:, qk * NK + k,
                                      pair * 128 : (pair + 1) * 128],
                            rhs=ln1t[k][:, c0 : c0 + csz],
                            start=(k == 0), stop=(k == NK - 1))
                    for h in range(2):
                        b = QK_BASE[h]
                        nc.vector.tensor_copy(
                            out=dst[h][b : b + HDIM, c0 : c0 + csz],
                            in_=pqk[b : b + HDIM, :csz])
            for h in range(2):
                qb = QK_BASE[h]
                qa_v = qa[h].rearrange("p (w a b) -> p w a b", a=WS, b=WS)
                for qh in range(WS):
                    pr = pp.tile([WS, NW * WS], F32, tag="ps3a", bufs=3, name="pr")
                    nc.tensor.matmul(
                        pr.rearrange("p (w b) -> p w b", w=NW),
                        lhsT=rh_s[qb : qb + HDIM, qh * WS : (qh + 1) * WS],
                        rhs=qa_v[qb : qb + HDIM, :, qh, :],
                        start=True, stop=True)
                    if h == 0:
                        nc.vector.tensor_copy(
                            out=qa_v[RH_BASE[h] : RH_BASE[h] + WS, :, qh, :],
                            in_=pr.rearrange("p (w b) -> p w b", w=NW))
                    else:
                        nc.scalar.copy(
                            out=qa_v[RH_BASE[h] : RH_BASE[h] + WS, :, qh, :],
                            in_=pr.rearrange("p (w b) -> p w b", w=NW))
                for qw in range(WS):
                    pr2 = pp.tile([WS, NW * WS], F32, tag="ps3a", bufs=3, name="pr2")
                    nc.tensor.matmul(
                        pr2.rearrange("p (w a) -> p w a", w=NW),
                        lhsT=rw_s[qb : qb + HDIM, qw * WS : (qw + 1) * WS],
                        rhs=qa_v[qb : qb + HDIM, :, :, qw],
                        start=True, stop=True)
                    if h == 0:
                        nc.vector.tensor_copy(
                            out=qa_v[RW_BASE[h] : RW_BASE[h] + WS, :, :, qw],
                            in_=pr2.rearrange("p (w a) -> p w a", w=NW))
                    else:
                        nc.scalar.copy(
                            out=qa_v[RW_BASE[h] : RW_BASE[h] + WS, :, :, qw],
                            in_=pr2.rearrange("p (w a) -> p w a", w=NW))

            srh = po_p.tile([33, T], BF16, tag="srh", bufs=1)
            ao = None
            for w in range(NW):
                if w % 13 == 0:
                    ao = po_p.tile([128, 2548], BF16, tag="ao")
                    w0 = w
                vws = []
                for coff in (0, 98):
                    pv = pp.tile([98, 128], F32, tag="ps3a", bufs=3, name="pv")
                    for k in range(NK):
                        nc.tensor.matmul(
                            pv,
                            lhsT=ln1t[k][:, w * NTOK + coff :
                                         w * NTOK + coff + 98],
                            rhs=wqkv[:, 2 * NK + k,
                                     pair * 128 : (pair + 1) * 128],
                            start=(k == 0), stop=(k == NK - 1))
                    # [v_h0 (64) | 1 | v_h1 (64) | 1] so lhsT slices stay
                    # contiguous and carry the softmax-sums ones column
                    vw = p.tile([98, 2 * HB], BF16,
                                tag=f"vw{coff > 0}", name="vw")
                    vwv = vw.rearrange("p (h c) -> p h c", c=HB)
                    nc.vector.tensor_copy(
                        out=vwv[:, :, 0:HDIM],
                        in_=pv.rearrange("p (h c) -> p h c", c=HDIM))
                    nc.vector.memset(vwv[:, :, HDIM : HDIM + 1], 1.0)
                    vws.append(vw)
                vw0, vw1 = vws
                for h in range(2):
                    st = pp1.tile([98, 2, NTOK], F32, tag="st0", bufs=2)
                    nc.tensor.matmul(
                        st[:, 0, :], lhsT=ka[h][:, w * NTOK : w * NTOK + 98],
                        rhs=qa[h][:, w * NTOK : (w + 1) * NTOK],
                        start=True, stop=True)
                    nc.tensor.matmul(
                        st[:, 1, :],
                        lhsT=ka[h][:, w * NTOK + 98 : w * NTOK + 196],
                        rhs=qa[h][:, w * NTOK : (w + 1) * NTOK],
                        start=True, stop=True)
                    et = p.tile([98, 2, NTOK], BF16, tag="et0", bufs=4)
                    nc.scalar.activation(out=et, in_=st, func=AF.Exp)
                    # O plus a sums row via the ones column of v (row 64)
                    pO = pp1.tile([HB, NTOK], F32, tag="pO", bufs=3)
                    nc.tensor.matmul(pO,
                                     lhsT=vw0[:, h * HB : (h + 1) * HB],
                                     rhs=et[:, 0, :], start=True, stop=False)
                    nc.tensor.matmul(pO,
                                     lhsT=vw1[:, h * HB : (h + 1) * HB],
                                     rhs=et[:, 1, :], start=False, stop=True)
                    cof = (w - w0) * NTOK
                    if h == 0:
                        nc.vector.tensor_copy(
                            out=srh[0:1, w * NTOK : (w + 1) * NTOK],
                            in_=pO[HDIM : HDIM + 1, :])
                        nc.scalar.copy(
                            out=ao[0:HDIM, cof : cof + NTOK],
                            in_=pO[0:HDIM, :])
                    else:
                        nc.scalar.copy(
                            out=srh[32:33, w * NTOK : (w + 1) * NTOK],
                            in_=pO[HDIM : HDIM + 1, :])
                        nc.vector.tensor_copy(
                            out=ao[HDIM:128, cof : cof + NTOK],
                            in_=pO[0:HDIM, :])
                if w in (12, NW - 1):
                    nsz = (w - w0 + 1) * NTOK
                    nc.sync.dma_start(
                        out=attn_d[pair * 128 : (pair + 1) * 128,
                                   w0 * NTOK : w0 * NTOK + nsz],
                        in_=ao[:, :nsz])
            for h in range(2):
                nc.sync.dma_start(
                    out=srows[pair * 2 + h : pair * 2 + h + 1, :],
                    in_=srh[h * 32 : h * 32 + 1, :])


# ---------------------------------------------------------------- stage 4+5
def _s4(nc, tc, wproj_d, n1wbi_d, onesb, ln1t, attn_d, x1_d, srows, smrows,
        e2_d):
    # phase 1 (windowed tokens): proj + residual -> x1_d (window order).
    # x is reconstructed from ln1t via per-token (std, mean) rows.
    with tc.tile_pool(name="s4w", bufs=1) as wp, \
         tc.tile_pool(name="s4", bufs=3) as p, \
         tc.tile_pool(name="s4ps", bufs=2, space="PSUM") as pp, \
         tc.tile_pool(name="s4ps1", bufs=1, space="PSUM") as pp1:
        wproj = wp.tile([128, NK, DIM], BF16, tag="wproj")
        nc.sync.dma_start(out=wproj, in_=wproj_d[:, :, :])
        e2 = wp.tile([HEADS, NK * 128], BF16, tag="e2")
        nc.sync.dma_start(out=e2, in_=e2_d[:, :])
        n1wbi = wp.tile([128, NK, 2], F32, tag="n1wbi")
        nc.sync.dma_start(out=n1wbi, in_=n1wbi_d[:, :, :])
        for nt in range(NT45):
            c0 = nt * 392
            csz = min(392, T - c0)
            at = [p.tile([128, 392], BF16, tag=f"at{k}", name=f"at{k}")
                  for k in range(NK)]
            recb = p.tile([HEADS, 392], BF16, tag="recb")
            with nc.allow_low_precision(reason="softmax denominators"):
                nc.vector.reciprocal(out=recb[:, :csz],
                                     in_=srows[:, c0 : c0 + csz])
            pbs_p = pp1.tile([128, 392], F32, tag="pbs")
            nc.tensor.matmul(pbs_p[:, :csz], lhsT=onesb[0:1, :],
                             rhs=smrows[0:1, c0 : c0 + csz],
                             start=True, stop=True)


            for k in range(NK):
                nc.gpsimd.dma_start(
                    out=at[k][:, :csz],
                    in_=attn_d[k * 128 : (k + 1) * 128, c0 : c0 + csz])
                bck = pp.tile([128, 392], F32, tag="bck")
                nc.tensor.matmul(bck[:, :csz],
                                 lhsT=e2[:, k * 128 : (k + 1) * 128],
                                 rhs=recb[:, :csz], start=True, stop=True)
                nc.vector.tensor_mul(out=at[k][:, :csz], in0=at[k][:, :csz],
                                     in1=bck[:, :csz])
            x1 = p.tile([128, NK, 392], BF16, tag="x1")
            for m in range(NK):
                xs = p.tile([128, 392], F32, tag="xs")
                nc.scalar.activation(
                    out=xs[:, :csz], in_=ln1t[m][:, c0 : c0 + csz],
                    func=AF.Identity,
                    scale=n1wbi[:, m, 0:1], bias=n1wbi[:, m, 1:2])
                xsb = p.tile([128, 392], F32, tag="xsb")
                nc.vector.tensor_mul(out=xsb[:, :csz], in0=xs[:, :csz],
                                     in1=pbs_p[:, :csz])
                pj = pp.tile([128, 392], F32, tag="pj", bufs=3)
                nc.tensor.matmul(pj[:, :csz], lhsT=onesb[32:33, :],
                                 rhs=smrows[32:33, c0 : c0 + csz],
                                 start=True, stop=False)
                for k in range(NK):
                    nc.tensor.matmul(
                        pj[:, :csz],
                        lhsT=wproj[:, k, m * 128 : (m + 1) * 128],
                        rhs=at[k][:, :csz], start=False, stop=(k == NK - 1))
                nc.vector.tensor_add(out=x1[:, m, :csz], in0=pj[:, :csz],
                                     in1=xsb[:, :csz])
            nc.sync.dma_start(
                out=x1_d[:, :, c0 : c0 + csz].rearrange("k p c -> p k c"),
                in_=x1[:, :, :csz])


def _s5(nc, tc, wfc1_d, wfc2_d, n2wb_d, onesb, x1_d, x2p, eps5):
    # phase 2 (window order, 13 tiles of 392): LN2 + MLP + residual -> x2_d
    with tc.tile_pool(name="s5w", bufs=1) as wp, \
         tc.tile_pool(name="s5", bufs=2) as p, \
         tc.tile_pool(name="s5h", bufs=1) as hp, \
         tc.tile_pool(name="s5s", bufs=3) as ps, \
         tc.tile_pool(name="s5ps", bufs=2, space="PSUM") as pp, \
         tc.tile_pool(name="s5ps1", bufs=1, space="PSUM") as pp1:
        f1 = wp.tile([128, NK, MLPD], BF16, tag="f1")
        nc.sync.dma_start(out=f1, in_=wfc1_d[:, :, :])
        f2 = wp.tile([128, 24, DIM], BF16, tag="f2")
        nc.sync.dma_start(out=f2, in_=wfc2_d[:, :, :])
        n2wb = wp.tile([128, NK, 2], F32, tag="n2wb")
        nc.sync.dma_start(out=n2wb, in_=n2wb_d[:, :, :])
        for nt in range(NT45):
            c0 = nt * 392
            csz = min(392, T - c0)
            x1 = p.tile([128, NK, 392], BF16, tag="x1p2")
            nc.sync.dma_start(
                out=x1[:, :, :csz],
                in_=x1_d[:, :, c0 : c0 + csz].rearrange("k p c -> p k c"))
            l2 = p.tile([128, NK, 392], BF16, tag="l2")
            srow = pp1.tile([1, 512], F32, tag="srow")
            ssq = pp1.tile([1, 512], F32, tag="ssq")
            for m in range(NK):
                sqm = ps.tile([128, 392], BF16, bufs=2, tag="sqm")
                nc.scalar.activation(out=sqm, in_=x1[:, m, :], func=AF.Square)
                nc.tensor.matmul(srow[:, :csz], lhsT=onesb[:, 0:1],
                                 rhs=x1[:, m, :csz],
                                 start=(m == 0), stop=(m == NK - 1),
                                 skip_group_check=True)
                nc.tensor.matmul(ssq[:, :csz], lhsT=onesb[:, 0:1],
                                 rhs=sqm[:, :csz],
                                 start=(m == 0), stop=(m == NK - 1),
                                 skip_group_check=True)
            rs_b, nm_b = _ln_rows(nc, ps, srow, ssq, csz, DIM, eps5[0:1, :],
                                  "l2")
            pb1 = pp1.tile([128, 512], F32, tag="pb1")
            nc.tensor.matmul(pb1[:, :csz], lhsT=onesb[0:1, :],
                             rhs=rs_b[:, :csz], start=True, stop=True)
            pb2 = pp1.tile([128, 512], F32, tag="pb2")
            nc.tensor.matmul(pb2[:, :csz], lhsT=onesb[0:1, :],
                             rhs=nm_b[:, :csz], start=True, stop=True)
            for m in range(NK):
                t1 = ps.tile([128, 392], F32, tag="t1")
                nc.vector.tensor_mul(out=t1[:, :csz], in0=x1[:, m, :csz],
                                     in1=pb1[:, :csz])
                nc.vector.tensor_add(out=t1[:, :csz], in0=t1[:, :csz],
                                     in1=pb2[:, :csz])
                if APPLY_AFFINE:
                    nc.vector.tensor_scalar(
                        out=l2[:, m, :csz], in0=t1[:, :csz],
                        scalar1=n2wb[:, m, 0:1], scalar2=n2wb[:, m, 1:2],
                        op0=ALU.mult, op1=ALU.add)
                else:
                    nc.vector.tensor_copy(out=l2[:, m, :csz], in_=t1[:, :csz])
            hb = hp.tile([128, 24, 392], BF16, tag="hb")
            for m24 in range(24):
                pf = pp.tile([128, 392], F32, tag="mm5", bufs=3)
                for k in range(NK):
                    nc.tensor.matmul(
                        pf[:, :csz],
                        lhsT=f1[:, k, m24 * 128 : (m24 + 1) * 128],
                        rhs=l2[:, k, :csz], start=(k == 0), stop=(k == NK - 1))
                nc.scalar.activation(out=hb[:, m24, :csz], in_=pf[:, :csz],
                                     func=AF.Gelu)
            x2 = p.tile([128, NK, 392], BF16, tag="x2")
            for m in range(NK):
                pg = pp.tile([128, 392], F32, tag="mm5", bufs=3)
                for k24 in range(24):
                    nc.tensor.matmul(
                        pg[:, :csz],
                        lhsT=f2[:, k24, m * 128 : (m + 1) * 128],
                        rhs=hb[:, k24, :csz], start=(k24 == 0),
                        stop=(k24 == 23))
                nc.vector.tensor_add(out=x2[:, m, :csz], in0=pg[:, :csz],
                                     in1=x1[:, m, :csz])
            srcv = x2.rearrange("p k (a i j) -> p k a i j", a=2, i=WS)
            for wi in range(2):
                w = nt * 2 + wi
                if w >= NW or wi * NTOK >= csz:
                    continue
                wr, wc, ph, pw = wdims(w)
                for k in range(NK):
                    dst = x2p[k].rearrange("p (r c) -> p r c", r=PAD)[
                        :, 1 + WS * wr : 1 + WS * wr + ph,
                        1 + WS * wc : 1 + WS * wc + pw]
                    if k % 2:
                        nc.scalar.copy(out=dst, in_=srcv[:, k, wi, :ph, :pw])
                    else:
                        nc.vector.tensor_copy(out=dst,
                                              in_=srcv[:, k, wi, :ph, :pw])


# ---------------------------------------------------------------- stage 6
def _s6(nc, tc, w1c_d, w2t_d, w3c_d, lnwb_d, onesb, ident_f,
        ident_b, x2p, y, eps6):
    with tc.tile_pool(name="s6w", bufs=1) as wp, \
         tc.tile_pool(name="s6b", bufs=1) as bigp, \
         tc.tile_pool(name="s6", bufs=2) as p, \
         tc.tile_pool(name="s6s", bufs=3) as ps, \
         tc.tile_pool(name="s6ps", bufs=2, space="PSUM") as pp, \
         tc.tile_pool(name="s6ps1", bufs=1, space="PSUM") as pp1:
        w1s = wp.tile([128, NK, BOT], BF16, tag="w1s")
        nc.sync.dma_start(out=w1s, in_=w1c_d[:, :, :])
        w2s = wp.tile([128, 3, 9, BOT], BF16, tag="w2s")
        nc.sync.dma_start(out=w2s, in_=w2t_d[:, :, :, :])
        w3s = wp.tile([128, 3, DIM], BF16, tag="w3s")
        nc.sync.dma_start(out=w3s, in_=w3c_d[:, :, :])
        lnwb = wp.tile([128, 12, 2], F32, tag="lnwb")
        nc.sync.dma_start(out=lnwb, in_=lnwb_d[:, :, :])

        b1p = [bigp.tile([128, PAD * PAD], BF16, tag=f"b1p{k}", name=f"b1p{k}")
               for k in range(3)]
        b2 = [bigp.tile([128, IMG * IMG], BF16, tag=f"b2_{k}", name=f"b2_{k}")
              for k in range(3)]
        for k in range(3):
            nc.gpsimd.memset(b1p[k], 0.0)
        def convln(rhs_fn, wsel, nk_in, nm_out, taps, eps_ap, nt8):
            seq = [(tap, k) for tap in taps for k in range(nk_in)]
            cms = []
            srow = pp1.tile([1, 512], F32, tag="c_srow")
            ssq = pp1.tile([1, 512], F32, tag="c_ssq")
            for m in range(nm_out):
                pc = pp.tile([128, 512], F32, tag="pc", bufs=3)
                for idx, (tap, k) in enumerate(seq):
                    nc.tensor.matmul(
                        pc.rearrange("p (r c) -> p r c", r=8),
                        lhsT=wsel(k, tap, m),
                        rhs=rhs_fn(k, tap, nt8),
                        start=(idx == 0), stop=(idx == len(seq) - 1))
                cm = ps.tile([128, 512], BF16, bufs=7, tag="cm")
                nc.scalar.copy(out=cm, in_=pc)
                sq = ps.tile([128, 512], BF16, bufs=2, tag="csq")
                nc.scalar.activation(out=sq, in_=pc, func=AF.Square)
                cms.append((cm, sq))
                nc.tensor.matmul(srow, lhsT=onesb[:, 0:1], rhs=cm,
                                 start=(m == 0), stop=(m == nm_out - 1),
                                 skip_group_check=True)
                nc.tensor.matmul(ssq, lhsT=onesb[:, 0:1], rhs=sq,
                                 start=(m == 0), stop=(m == nm_out - 1),
                                 skip_group_check=True)
            rs_b, nm_b = _ln_rows(nc, ps, srow, ssq, 512, nm_out * 128,
                                  eps_ap, "c")
            qb1 = pp1.tile([128, 512], F32, tag="qb1")
            nc.tensor.matmul(qb1, lhsT=onesb[0:1, :], rhs=rs_b, start=True,
                             stop=True)
            qb2 = pp1.tile([128, 512], F32, tag="qb2")
            nc.tensor.matmul(qb2, lhsT=onesb[0:1, :], rhs=nm_b, start=True,
                             stop=True)
            outs = []
            for m in range(nm_out):
                t1 = ps.tile([128, 512], F32, bufs=7, tag="c_t1")
                nc.vector.tensor_mul(out=t1, in0=cms[m][0], in1=qb1)
                nc.vector.tensor_add(out=t1, in0=t1, in1=qb2)
                outs.append(t1)
            return outs

        def x2p_rhs(k, tap, nt8):
            return x2p[k].rearrange("p (r c) -> p r c", r=PAD)[
                :, 1 + nt8 * 8 : 1 + nt8 * 8 + 8, 1 : 1 + IMG]

        def b1p_rhs(k, tap, nt8):
            dy, dx = tap
            return b1p[k].rearrange("p (r c) -> p r c", r=PAD)[
                :, nt8 * 8 + dy : nt8 * 8 + dy + 8, dx : dx + IMG]

        def b2_rhs(k, tap, nt8):
            return b2[k].rearrange("p (r c) -> p r c", r=IMG)[
                :, nt8 * 8 : nt8 * 8 + 8, :]

        for nt8 in range(8):
            outs = convln(
                x2p_rhs,
                lambda k, tap, m: w1s[:, k, m * 128 : (m + 1) * 128],
                NK, 3, [None], eps6[0:1, :], nt8)
            for m in range(3):
                nc.scalar.activation(
                    out=b1p[m].rearrange("p (r c) -> p r c", r=PAD)[
                        :, 1 + nt8 * 8 : 1 + nt8 * 8 + 8, 1 : 1 + IMG],
                    in_=outs[m].rearrange("p (r c) -> p r c", r=8),
                    func=AF.Gelu, scale=lnwb[:, m, 0:1], bias=lnwb[:, m, 1:2])
        taps2 = [(dy, dx) for dy in range(3) for dx in range(3)]
        for nt8 in range(8):
            outs = convln(
                b1p_rhs,
                lambda k, tap, m: w2s[:, k, tap[0] * 3 + tap[1],
                                      m * 128 : (m + 1) * 128],
                3, 3, taps2, eps6[0:1, :], nt8)
            for m in range(3):
                nc.scalar.activation(
                    out=b2[m].rearrange("p (r c) -> p r c", r=IMG)[
                        :, nt8 * 8 : nt8 * 8 + 8, :],
                    in_=outs[m].rearrange("p (r c) -> p r c", r=8),
                    func=AF.Gelu, scale=lnwb[:, 3 + m, 0:1],
                    bias=lnwb[:, 3 + m, 1:2])
        for nt8 in range(8):
            outs = convln(
                b2_rhs,
                lambda k, tap, m: w3s[:, k, m * 128 : (m + 1) * 128],
                3, NK, [None], eps6[0:1, :], nt8)
            om = []
            for m in range(NK):
                fin = ps.tile([128, 512], BF16, bufs=7, tag="c_fin")
                nc.scalar.activation(
                    out=fin, in_=outs[m], func=AF.Identity,
                    scale=lnwb[:, 6 + m, 0:1], bias=lnwb[:, 6 + m, 1:2])
                nc.gpsimd.tensor_add(
                    out=fin.rearrange("p (r c) -> p r c", r=8),
                    in0=fin.rearrange("p (r c) -> p r c", r=8),
                    in1=x2p[m].rearrange("p (r c) -> p r c", r=PAD)[
                        :, 1 + nt8 * 8 : 1 + nt8 * 8 + 8, 1 : 1 + IMG])
                om.append(fin)
            for s in range(4):
                outt = p.tile([128, DIM], BF16, tag="outt")
                for m in range(NK):
                    ptp = pp.tile([128, 128], BF16, tag="ptp", bufs=1)
                    nc.tensor.transpose(ptp, om[m][:, s * 128 : (s + 1) * 128],
                                        ident_b)
                    if m % 2:
                        nc.scalar.copy(
                            out=outt[:, m * 128 : (m + 1) * 128], in_=ptp)
                    else:
                        nc.vector.tensor_copy(
                            out=outt[:, m * 128 : (m + 1) * 128], in_=ptp)
                nc.sync.dma_start(
                    out=y[nt8 * 512 + s * 128 : nt8 * 512 + (s + 1) * 128, :],
                    in_=outt)


# ---------------------------------------------------------------- host side
def _chunk(w, nchunk):
    """[nchunk*128, N] -> [128, nchunk, N] (row r = chunk*128 + p)."""
    n = w.shape[1]
    return np.ascontiguousarray(w.reshape(nchunk, 128, n).transpose(1, 0, 2))


def _prep_weights(inputs):
    bf = ml_dtypes.bfloat16
    f8 = ml_dtypes.float8_e4m3
    qkv_w = np.array(inputs["qkv_w"], np.float32)
    qkv_w[:, :DIM] *= HDIM ** -0.5  # fold score scale into Wq
    # [128, (qk, k), 768]: chunk (qk*NK + k) holds input-rows k*128..+128 of
    # the q/k/v block
    wq = np.stack([qkv_w[k * 128 : (k + 1) * 128, qk * DIM : (qk + 1) * DIM]
                   for qk in range(3) for k in range(NK)], axis=1)
    rel_h = np.array(inputs["rel_pos_h"], np.float32)
    rel_w = np.array(inputs["rel_pos_w"], np.float32)
    coords = np.arange(WS)[:, None] - np.arange(WS)[None, :] + WS - 1
    Rh = rel_h[coords]  # [q, k, c]
    Rw = rel_w[coords]
    # lhsT[c, kh] per qh; x sqrt(HDIM) undoes the q-scale fold
    rh_all = np.concatenate([Rh[i].T * (HDIM ** 0.5) for i in range(WS)],
                            axis=1)
    rw_all = np.concatenate([Rw[i].T * (HDIM ** 0.5) for i in range(WS)],
                            axis=1)
    kh = np.arange(NTOK) // WS
    kw = np.arange(NTOK) % WS
    kon = np.zeros((2 * WS, NTOK), np.float32)
    kon[kh, np.arange(NTOK)] = 1.0
    kon[WS + kw, np.arange(NTOK)] = 1.0
    konst = np.tile(kon, (1, NW))
    e2 = np.zeros((HEADS, NK * 128), np.float32)
    for g in range(HEADS):
        k, half = divmod(g, 2)
        e2[g, k * 128 + half * 64 : k * 128 + half * 64 + 64] = 1.0
    conv2_w = np.array(inputs["conv2_w"], np.float32)
    # [128, (kchunk), (tap), 384]
    w2t = np.stack([conv2_w[:, :, dy, dx].T for dy in range(3)
                    for dx in range(3)])  # [9, 384in, 384out]
    w2t = np.ascontiguousarray(
        w2t.reshape(9, 3, 128, BOT).transpose(2, 1, 0, 3))
    lnw = np.concatenate([np.array(inputs[k], np.float32)
                          for k in ("ln1_w", "ln2_w", "ln3_w")])
    lnb = np.concatenate([np.array(inputs[k], np.float32)
                          for k in ("ln1_b", "ln2_b", "ln3_b")])
    lnwb = np.stack([lnw, lnb], axis=1)  # [1536, 2]
    n2wb = np.stack([np.array(inputs["norm2_w"], np.float32),
                     np.array(inputs["norm2_b"], np.float32)], axis=1)
    d = dict(
        wqkv=np.ascontiguousarray(wq).astype(bf),
        wproj=_chunk(np.array(inputs["proj_w"], np.float32), NK).astype(bf),
        wfc1=_chunk(np.array(inputs["fc1_w"], np.float32), NK).astype(bf),
        wfc2=_chunk(np.array(inputs["fc2_w"], np.float32), 24).astype(bf),
        w1c=_chunk(np.array(inputs["conv1_w"], np.float32)[:, :, 0, 0].T,
                   NK).astype(bf),
        w2t=w2t.astype(bf),
        w3c=_chunk(np.array(inputs["conv3_w"], np.float32)[:, :, 0, 0].T,
                   3).astype(bf),
        rh_all=np.ascontiguousarray(rh_all).astype(bf),
        rw_all=np.ascontiguousarray(rw_all).astype(bf),
        konst=np.ascontiguousarray(konst).astype(bf),
        e2=e2.astype(bf),
        idf=np.eye(128, dtype=np.float32),
        idb=np.eye(128).astype(bf),
        n1wb=_chunk(np.stack([np.array(inputs["norm1_w"], np.float32),
                              np.array(inputs["norm1_b"], np.float32)],
                             axis=1), NK),
        n1wbi=_chunk(np.stack(
            [1.0 / np.array(inputs["norm1_w"], np.float32),
             -np.array(inputs["norm1_b"], np.float32)
             / np.array(inputs["norm1_w"], np.float32)], axis=1), NK),
        n2wb=_chunk(n2wb, NK),
        lnwb=_chunk(lnwb, 12),
    )
    return d


def kernel(**inputs):
    nc = build()
    split_multi_waits(nc)
    wts = _prep_weights(inputs)
    x = np.asarray(inputs["x"], np.float32)
    in_maps = [dict(wts, x=np.ascontiguousarray(x[c])) for c in range(N_CORES)]
    res = run_bass_kernel_spmd(nc, in_maps, core_ids=list(range(N_CORES)))
    return np.stack([np.asarray(res.results[c]["y"], np.float32)
                       .reshape(IMG, IMG, DIM)
                     for c in range(N_CORES)])



# revision 97
# speedup vs baseline: 1.0202x; 1.0058x over previous
"""Trainium2 Bass kernel for the ViTDet-style transformer block
(windowed 14x14 attention w/ decomposed rel-pos, MLP, bottleneck convs).

Sharding: data-parallel over batch — 8 images, one per NeuronCore; no
collectives (windows/MLP/convs are intra-image).

On-chip layout is D-major (channels on partitions, tokens on free dim).
The decomposed rel-pos bias is folded into the score matmul by augmenting
the contraction dim: q rows carry [q (64) | q.Rh (14) | q.Rw (14)] and k
rows carry [k (64) | onehot(kh) | onehot(kw)], padded with zero rows to
K=128. Compute engines cannot cross partitions, so the odd head of each
pair lives at partitions 64..127 with its aug rows at 0..13 / 32..45,
letting every psum->sbuf copy stay partition-aligned. Softmax runs in the
S^T orientation (exp on ACT; key-sums via ones-matmul; 1/sum broadcast by
a K=1 matmul). 3x3 conv = 9 shifted accumulating matmuls over a
zero-padded 66x66 layout. PE transposes only at entry/exit.
"""

import sys

sys.path.insert(0, "/opt/trn_rl_repo")

import numpy as np
import ml_dtypes

import concourse.bass as bass
import concourse.mybir as mybir
from concourse.tile import TileContext
from concourse.bass_utils import run_bass_kernel_spmd

F32 = mybir.dt.float32
BF16 = mybir.dt.bfloat16
F8 = mybir.dt.float8e4
F8SC = 32.0  # fp8 weight pre-scale (undone at activation/copy time)
DR = mybir.MatmulPerfMode.DoubleRow
AF = mybir.ActivationFunctionType
ALU = mybir.AluOpType

DIM = 768
HEADS = 12
HDIM = 64
WS = 14
MLPD = 3072
BOT = 384
IMG = 64
GRID = 5
NW = GRID * GRID  # 25
NTOK = WS * WS  # 196
T = NW * NTOK  # 4900
C0, C1 = 126, 70  # window-token chunks (rows 0..8 / 9..13)
NK = DIM // 128  # 6
NT45 = 13  # token tiles of 392 (last one 196)
N_CORES = 8
PAD = IMG + 2  # 66

APPLY_AFFINE = True

# per-head-parity row layouts inside the [128, T] qa/ka tiles
QK_BASE = (0, 64)  # q/k content rows
RH_BASE = (64, 0)  # q.Rh / onehot(kh) rows (14)
RW_BASE = (96, 32)  # q.Rw / onehot(kw) rows (14)
Z_ROWS = (((78, 96), (110, 128)), ((14, 32), (46, 64)))  # zeroed rows


def wdims(w):
    wr, wc = divmod(w, GRID)
    ph = WS if wr < GRID - 1 else IMG - WS * (GRID - 1)
    pw = WS if wc < GRID - 1 else IMG - WS * (GRID - 1)
    return wr, wc, ph, pw


def split_multi_waits(nc):
    """walrus CoreV3 encodes at most one sync-wait per CTRL-class op; move
    extra waits onto preceding same-engine NoOps."""
    n = 0
    for f in nc.m.functions:
        for bb in f.blocks:
            out = []
            for inst in bb.instructions:
                si = getattr(inst, "sync_info", None)
                waits = list(si.on_wait) if si is not None else []
                if len(waits) > 1:
                    for i, wt in enumerate(waits[:-1]):
                        nop = mybir.InstNoOp(
                            name=f"{inst.name}-wsplit{i}", ins=[], outs=[]
                        )
                        nop.engine = inst.engine
                        nop.sync_info = mybir.SyncInfo(on_wait=[wt], on_update=[])
                        out.append(nop)
                        n += 1
                    inst.sync_info = mybir.SyncInfo(
                        on_wait=[waits[-1]], on_update=list(si.on_update)
                    )
                out.append(inst)
            bb.instructions = out
    return n


def _bcast_part(ap, n):
    return bass.AP(tensor=ap.tensor, offset=ap.offset, ap=[[0, n]] + list(ap.ap))


def _ln_rows(nc, ps, srow, ssq, csz, nch, eps_ap, tag):
    """psum row-sums -> bf16 rows (rstd, -mean*rstd), each [1, csz]."""
    mrow = ps.tile([1, 512], F32, bufs=1, tag=f"{tag}mr")
    nc.scalar.mul(out=mrow[:, :csz], in_=srow[:, :csz], mul=1.0 / nch)
    vrow = ps.tile([1, 512], F32, bufs=1, tag=f"{tag}vr")
    nc.scalar.mul(out=vrow[:, :csz], in_=ssq[:, :csz], mul=1.0 / nch)
    m2 = ps.tile([1, 512], F32, bufs=1, tag=f"{tag}m2")
    nc.vector.tensor_mul(out=m2[:, :csz], in0=mrow[:, :csz], in1=mrow[:, :csz])
    nc.vector.tensor_sub(out=vrow[:, :csz], in0=vrow[:, :csz], in1=m2[:, :csz])
    sdr = ps.tile([1, 512], F32, bufs=1, tag=f"{tag}sd")
    nc.scalar.activation(out=sdr[:, :csz], in_=vrow[:, :csz], func=AF.Sqrt,
                         bias=eps_ap)
    rs = ps.tile([1, 512], F32, bufs=1, tag=f"{tag}rs")
    nc.vector.reciprocal(out=rs[:, :csz], in_=sdr[:, :csz])
    nmr = ps.tile([1, 512], F32, bufs=1, tag=f"{tag}nm")
    nc.vector.tensor_mul(out=nmr[:, :csz], in0=mrow[:, :csz], in1=rs[:, :csz])
    nc.vector.tensor_scalar_mul(out=nmr[:, :csz], in0=nmr[:, :csz],
                                scalar1=-1.0)
    rs_b = ps.tile([1, 512], BF16, bufs=1, tag=f"{tag}rsb")
    nc.vector.tensor_copy(out=rs_b[:, :csz], in_=rs[:, :csz])
    nm_b = ps.tile([1, 512], BF16, bufs=1, tag=f"{tag}nmb")
    nc.vector.tensor_copy(out=nm_b[:, :csz], in_=nmr[:, :csz])
    return rs_b, nm_b


def build(stages=(1, 2, 3, 45, 6)):
    nc = bass.Bass("TRN2", target_bir_lowering=False)

    x = nc.dram_tensor("x", [IMG, IMG, DIM], F32, kind="ExternalInput")
    y = nc.dram_tensor("y", [IMG * IMG, DIM], BF16, kind="ExternalOutput")

    # weights pre-chunked on host to the SBUF layout [128, nchunk, cols]
    wqkv_d = nc.dram_tensor("wqkv", [128, 3 * NK, DIM], BF16,
                            kind="ExternalInput")
    wproj_d = nc.dram_tensor("wproj", [128, NK, DIM], BF16,
                             kind="ExternalInput")
    wfc1_d = nc.dram_tensor("wfc1", [128, NK, MLPD], BF16,
                            kind="ExternalInput")
    wfc2_d = nc.dram_tensor("wfc2", [128, 24, DIM], BF16,
                            kind="ExternalInput")
    w1c_d = nc.dram_tensor("w1c", [128, NK, BOT], BF16, kind="ExternalInput")
    w2t_d = nc.dram_tensor("w2t", [128, 3, 9, BOT], BF16,
                           kind="ExternalInput")
    w3c_d = nc.dram_tensor("w3c", [128, 3, DIM], BF16, kind="ExternalInput")
    rh_d = nc.dram_tensor("rh_all", [HDIM, WS * WS], BF16,
                          kind="ExternalInput")
    rw_d = nc.dram_tensor("rw_all", [HDIM, WS * WS], BF16,
                          kind="ExternalInput")
    kon_d = nc.dram_tensor("konst", [2 * WS, T], BF16, kind="ExternalInput")
    e2_d = nc.dram_tensor("e2", [HEADS, NK * 128], BF16, kind="ExternalInput")
    idf_d = nc.dram_tensor("idf", [128, 128], F32, kind="ExternalInput")
    idb_d = nc.dram_tensor("idb", [128, 128], BF16, kind="ExternalInput")
    n1wb_d = nc.dram_tensor("n1wb", [128, NK, 2], F32, kind="ExternalInput")
    n1wbi_d = nc.dram_tensor("n1wbi", [128, NK, 2], F32,
                             kind="ExternalInput")
    n2wb_d = nc.dram_tensor("n2wb", [128, NK, 2], F32, kind="ExternalInput")
    lnwb_d = nc.dram_tensor("lnwb", [128, 12, 2], F32, kind="ExternalInput")

    attn_d = nc.dram_tensor("attn_d", [DIM, T], BF16)

    with TileContext(nc) as tc:
        with tc.tile_pool(name="const", bufs=1) as constp:
            ident_f = constp.tile([128, 128], F32, tag="idf")
            nc.sync.dma_start(out=ident_f, in_=idf_d[:, :])
            ident_b = constp.tile([128, 128], BF16, tag="idb")
            nc.sync.dma_start(out=ident_b, in_=idb_d[:, :])
            onesb = constp.tile([128, 128], BF16, tag="onesb")
            nc.vector.memset(onesb, 1.0)
            eps5 = constp.tile([128, 1], F32, tag="eps5")
            nc.vector.memset(eps5, 1e-5)
            eps6 = constp.tile([128, 1], F32, tag="eps6")
            nc.vector.memset(eps6, 1e-6)

            with tc.tile_pool(name="srp", bufs=1) as srp:
                srows = srp.tile([HEADS, T], BF16, tag="srows")
                smrows = srp.tile([33, T], BF16, tag="smrows")
                x1_d = nc.dram_tensor("x1_d", [NK, 128, T], BF16)
                with tc.tile_pool(name="attw", bufs=1) as awp:
                    wqkv = awp.tile([128, 3 * NK, DIM], BF16, tag="wqkv")
                    nc.sync.dma_start(out=wqkv, in_=wqkv_d[:, :, :])
                    # rel-pos weights duplicated on partitions 0..63 and
                    # 64..127 so lhsT base matches the rhs base of either
                    # head parity
                    rh_s = awp.tile([128, WS * WS], BF16, tag="rh")
                    rw_s = awp.tile([128, WS * WS], BF16, tag="rw")
                    for b in (0, 64):
                        nc.sync.dma_start(out=rh_s[b : b + HDIM, :],
                                          in_=rh_d[:, :])
                        nc.sync.dma_start(out=rw_s[b : b + HDIM, :],
                                          in_=rw_d[:, :])
                    with tc.tile_pool(name="ln1t", bufs=1) as lp:
                        ln1t = [lp.tile([128, T], BF16, tag=f"ln1t{k}",
                                        name=f"ln1t{k}")
                                for k in range(NK)]
                        with tc.tile_pool(name="qa", bufs=1) as qap:
                            qa2 = [[qap.tile([128, T], BF16, tag=f"qa{h}{s}",
                                             name=f"qa{h}{s}")
                                    for h in range(2)] for s in range(2)]
                            ka2 = [[qap.tile([128, T], BF16, tag=f"ka{h}{s}",
                                             name=f"ka{h}{s}")
                                    for h in range(2)] for s in range(2)]
                            for s in range(2):
                                for h in range(2):
                                    zb = 64 - QK_BASE[h]
                                    nc.vector.memset(
                                        qa2[s][h][zb : zb + 64, :], 0.0)
                                    nc.vector.memset(
                                        ka2[s][h][zb : zb + 64, :], 0.0)
                                    nc.sync.dma_start(
                                        out=ka2[s][h][RH_BASE[h] :
                                                      RH_BASE[h] + WS, :],
                                        in_=kon_d[0:WS, :])
                                    nc.sync.dma_start(
                                        out=ka2[s][h][RW_BASE[h] :
                                                      RW_BASE[h] + WS, :],
                                        in_=kon_d[WS : 2 * WS, :])
                            if 1 in stages:
                                _s1(nc, tc, x, ident_b, ident_f, n1wb_d,
                                    ln1t, eps5, smrows, qa2, ka2, wqkv)
                            if 3 in stages:
                                _s3(nc, tc, ln1t, wqkv, rh_s, rw_s, qa2, ka2,
                                    onesb, attn_d, srows)
                        if 45 in stages:
                            _s4(nc, tc, wproj_d, n1wbi_d, onesb, ln1t,
                                attn_d, x1_d, srows, smrows, e2_d)
            with tc.tile_pool(name="xp", bufs=1) as xpp:
                x2p = [xpp.tile([128, PAD * PAD], BF16, tag=f"x2p{k}",
                                name=f"x2p{k}")
                       for k in range(NK)]
                for k in range(NK):
                    nc.gpsimd.memset(x2p[k], 0.0)
                if 45 in stages:
                    _s5(nc, tc, wfc1_d, wfc2_d, n2wb_d, onesb, x1_d, x2p,
                        eps5)
                if 6 in stages:
                    _s6(nc, tc, w1c_d, w2t_d, w3c_d, lnwb_d, onesb, ident_f,
                        ident_b, x2p, y, eps6)
    return nc


# ---------------------------------------------------------------- stage 1
def _s1(nc, tc, x, ident_b, ident_f, n1wb_d, ln1t, eps5, smrows,
        qa2, ka2, wqkv):
    with tc.tile_pool(name="s1c", bufs=1) as cp, \
         tc.tile_pool(name="s1", bufs=2) as p, \
         tc.tile_pool(name="s1s", bufs=4) as ps, \
         tc.tile_pool(name="s1ps", bufs=4, space="PSUM") as pp:
        n1wb = cp.tile([128, NK, 2], F32, tag="n1wb")
        nc.sync.dma_start(out=n1wb, in_=n1wb_d[:, :, :])
        for w in range(NW):
            wr, wc, ph, pw = wdims(w)
            xw0 = p.tile([C0, DIM], F32, tag="xw0", bufs=3)
            xw1 = p.tile([C1, DIM], F32, tag="xw1", bufs=3)
            if ph == WS and pw == WS:
                nc.gpsimd.dma_start(
                    out=xw0, in_=x[WS * wr : WS * wr + 9,
                                   WS * wc : WS * wc + WS, :])
                nc.gpsimd.dma_start(
                    out=xw1, in_=x[WS * wr + 9 : WS * wr + WS,
                                   WS * wc : WS * wc + WS, :])
            else:
                nc.gpsimd.memset(xw0, 0.0)
                nc.gpsimd.memset(xw1, 0.0)
                for i in range(ph):
                    src = x[WS * wr + i, WS * wc : WS * wc + pw, :]
                    t0 = i * WS
                    if t0 + pw <= C0:
                        nc.gpsimd.dma_start(out=xw0[t0 : t0 + pw, :], in_=src)
                    elif t0 >= C0:
                        nc.gpsimd.dma_start(
                            out=xw1[t0 - C0 : t0 - C0 + pw, :], in_=src)
                    else:
                        sp = C0 - t0
                        nc.gpsimd.dma_start(out=xw0[t0:C0, :],
                                            in_=src[:sp, :])
                        nc.gpsimd.dma_start(out=xw1[0 : pw - sp, :],
                                            in_=src[sp:pw, :])
            for ci, (xw, csz, coff) in enumerate(((xw0, C0, 0), (xw1, C1, C0))):
                stats = ps.tile([csz, 3, 6], F32, tag=f"st{ci}")
                xv = xw.rearrange("p (s d) -> p s d", s=3)
                for s in range(3):
                    nc.vector.bn_stats(out=stats[:, s, :], in_=xv[:, s, :])
                mv = ps.tile([csz, 2], F32, tag=f"mv{ci}")
                nc.vector.bn_aggr(out=mv, in_=stats)
                # per-token (std, mean) -> rows of smrows for the stage-4
                # x reconstruction
                sm = ps.tile([csz, 33], F32, tag=f"sm{ci}")
                nc.scalar.activation(out=sm[:, 0:1], in_=mv[:, 1:2],
                                     func=AF.Sqrt, bias=eps5[:csz, :])
                nc.scalar.copy(out=sm[:, 32:33], in_=mv[:, 0:1])
                pst = pp.tile([33, C0], F32, tag="pst", bufs=2)
                nc.tensor.transpose(pst[:, :csz], sm, ident_f[:csz, :csz])
                nc.vector.tensor_copy(
                    out=smrows[0:1, w * NTOK + coff : w * NTOK + coff + csz],
                    in_=pst[0:1, :csz])
                nc.vector.tensor_copy(
                    out=smrows[32:33, w * NTOK + coff : w * NTOK + coff + csz],
                    in_=pst[32:33, :csz])
                rstd = ps.tile([csz, 1], F32, tag=f"rstd{ci}")
                nc.vector.reciprocal(out=rstd, in_=sm[:, 0:1])
                nmr = ps.tile([csz, 1], F32, tag=f"nmr{ci}")
                nc.vector.tensor_mul(out=nmr, in0=mv[:, 0:1], in1=rstd)
                nc.vector.tensor_scalar_mul(out=nmr, in0=nmr, scalar1=-1.0)
                lnw_t = p.tile([csz, DIM], BF16, tag=f"lnw{ci}",
                               bufs=2 if ci == 0 else 1)
                nc.scalar.activation(out=lnw_t, in_=xw, func=AF.Identity,
                                     scale=rstd, bias=nmr)
                for k in range(NK):
                    ptb = pp.tile([128, C0], BF16, tag="ptb", bufs=3)
                    nc.tensor.transpose(ptb[:, :csz],
                                        lnw_t[:, k * 128 : (k + 1) * 128],
                                        ident_b[:csz, :csz])
                    dst = ln1t[k][:, w * NTOK + coff :
                                  w * NTOK + coff + csz]
                    if k % 2:
                        nc.scalar.activation(
                            out=dst, in_=ptb[:, :csz], func=AF.Identity,
                            scale=n1wb[:, k, 0:1], bias=n1wb[:, k, 1:2])
                    else:
                        nc.vector.tensor_scalar(
                            out=dst, in0=ptb[:, :csz],
                            scalar1=n1wb[:, k, 0:1], scalar2=n1wb[:, k, 1:2],
                            op0=ALU.mult, op1=ALU.add)
            if w % 2 == 1 or w == NW - 1:
                nt = w // 2
                c0 = nt * 392
                csz = min(392, T - c0)
                for pair in range(2):
                    for qk, dst in ((0, qa2[pair]), (1, ka2[pair])):
                        pqk = pp.tile([128, 392], F32, tag="pqk", bufs=3)
                        for k in range(NK):
                            nc.tensor.matmul(
                                pqk[:, :csz],
                                lhsT=wqkv[:, qk * NK + k,
                                          pair * 128 : (pair + 1) * 128],
                                rhs=ln1t[k][:, c0 : c0 + csz],
                                start=(k == 0), stop=(k == NK - 1))
                        for h in range(2):
                            b = QK_BASE[h]
                            if h == 0:
                                nc.vector.tensor_copy(
                                    out=dst[h][b : b + HDIM, c0 : c0 + csz],
                                    in_=pqk[b : b + HDIM, :csz])
                            else:
                                nc.scalar.copy(
                                    out=dst[h][b : b + HDIM, c0 : c0 + csz],
                                    in_=pqk[b : b + HDIM, :csz])


# ---------------------------------------------------------------- stage 3
def _s3(nc, tc, ln1t, wqkv, rh_s, rw_s, qa2, ka2, onesb, attn_d, srows):
    with tc.tile_pool(name="s3", bufs=3) as p, \
         tc.tile_pool(name="s3o", bufs=2) as po_p, \
         tc.tile_pool(name="s3ps", bufs=2, space="PSUM") as pp, \
         tc.tile_pool(name="s3ps1", bufs=1, space="PSUM") as pp1:
        HB = HDIM + 1
        for pair in range(HEADS // 2):
            qa = qa2[pair % 2]
            ka = ka2[pair % 2]
            for nt in range(NT45 if pair >= 2 else 0):
                c0 = nt * 392
                csz = min(392, T - c0)
                for qk, dst in ((0, qa), (1, ka)):
                    pqk = pp.tile([128, 392], F32, tag="ps3a", bufs=3)
                    for k in range(NK):
                        nc.tensor.matmul(
                            pqk[:, :csz],
                            lhsT=wqkv[:, qk * NK + k,
                                      pair * 128 : (pair + 1) * 128],
                            rhs=ln1t[k][:, c0 : c0 + csz],
                            start=(k == 0), stop=(k == NK - 1))
                    for h in range(2):
                        b = QK_BASE[h]
                        nc.vector.tensor_copy(
                            out=dst[h][b : b + HDIM, c0 : c0 + csz],
                            in_=pqk[b : b + HDIM, :csz])
            for h in range(2):
                qb = QK_BASE[h]
                qa_v = qa[h].rearrange("p (w a b) -> p w a b", a=WS, b=WS)
                for qh in range(WS):
                    pr = pp.tile([WS, NW * WS], F32, tag="ps3a", bufs=3, name="pr")
                    nc.tensor.matmul(
                        pr.rearrange("p (w b) -> p w b", w=NW),
                        lhsT=rh_s[qb : qb + HDIM, qh * WS : (qh + 1) * WS],
                        rhs=qa_v[qb : qb + HDIM, :, qh, :],
                        start=True, stop=True)
                    if h == 0:
                        nc.vector.tensor_copy(
                            out=qa_v[RH_BASE[h] : RH_BASE[h] + WS, :, qh, :],
                            in_=pr.rearrange("p (w b) -> p w b", w=NW))
                    else:
                        nc.scalar.copy(
                            out=qa_v[RH_BASE[h] : RH_BASE[h] + WS, :, qh, :],
                            in_=pr.rearrange("p (w b) -> p w b", w=NW))
                for qw in range(WS):
                    pr2 = pp.tile([WS, NW * WS], F32, tag="ps3a", bufs=3, name="pr2")
                    nc.tensor.matmul(
                        pr2.rearrange("p (w a) -> p w a", w=NW),
                        lhsT=rw_s[qb : qb + HDIM, qw * WS : (qw + 1) * WS],
                        rhs=qa_v[qb : qb + HDIM, :, :, qw],
                        start=True, stop=True)
                    if h == 0:
                        nc.vector.tensor_copy(
                            out=qa_v[RW_BASE[h] : RW_BASE[h] + WS, :, :, qw],
                            in_=pr2.rearrange("p (w a) -> p w a", w=NW))
                    else:
                        nc.scalar.copy(
                            out=qa_v[RW_BASE[h] : RW_BASE[h] + WS, :, :, qw],
                            in_=pr2.rearrange("p (w a) -> p w a", w=NW))

            srh = po_p.tile([33, T], BF16, tag="srh", bufs=1)
            ao = None
            for w in range(NW):
                if w % 13 == 0:
                    ao = po_p.tile([128, 2548], BF16, tag="ao")
                    w0 = w
                vws = []
                for coff in (0, 98):
                    pv = pp.tile([98, 128], F32, tag="ps3a", bufs=3, name="pv")
                    for k in range(NK):
                        nc.tensor.matmul(
                            pv,
                            lhsT=ln1t[k][:, w * NTOK + coff :
                                         w * NTOK + coff + 98],
                            rhs=wqkv[:, 2 * NK + k,
                                     pair * 128 : (pair + 1) * 128],
                            start=(k == 0), stop=(k == NK - 1))
                    # [v_h0 (64) | 1 | v_h1 (64) | 1] so lhsT slices stay
                    # contiguous and carry the softmax-sums ones column
                    vw = p.tile([98, 2 * HB], BF16,
                                tag=f"vw{coff > 0}", name="vw")
                    vwv = vw.rearrange("p (h c) -> p h c", c=HB)
                    nc.vector.tensor_copy(
                        out=vwv[:, :, 0:HDIM],
                        in_=pv.rearrange("p (h c) -> p h c", c=HDIM))
                    nc.vector.memset(vwv[:, :, HDIM : HDIM + 1], 1.0)
                    vws.append(vw)
                vw0, vw1 = vws
                for h in range(2):
                    st = pp1.tile([98, 2, NTOK], F32, tag="st0", bufs=2)
                    nc.tensor.matmul(
                        st[:, 0, :], lhsT=ka[h][:, w * NTOK : w * NTOK + 98],
                        rhs=qa[h][:, w * NTOK : (w + 1) * NTOK],
                        start=True, stop=True)
                    nc.tensor.matmul(
                        st[:, 1, :],
                        lhsT=ka[h][:, w * NTOK + 98 : w * NTOK + 196],
                        rhs=qa[h][:, w * NTOK : (w + 1) * NTOK],
                        start=True, stop=True)
                    et = p.tile([98, 2, NTOK], BF16, tag="et0", bufs=4)
                    nc.scalar.activation(out=et, in_=st, func=AF.Exp)
                    # O plus a sums row via the ones column of v (row 64)
                    pO = pp1.tile([HB, NTOK], F32, tag="pO", bufs=3)
                    nc.tensor.matmul(pO,
                                     lhsT=vw0[:, h * HB : (h + 1) * HB],
                                     rhs=et[:, 0, :], start=True, stop=False)
                    nc.tensor.matmul(pO,
                                     lhsT=vw1[:, h * HB : (h + 1) * HB],
                                     rhs=et[:, 1, :], start=False, stop=True)
                    cof = (w - w0) * NTOK
                    if h == 0:
                        nc.vector.tensor_copy(
                            out=srh[0:1, w * NTOK : (w + 1) * NTOK],
                            in_=pO[HDIM : HDIM + 1, :])
                        nc.scalar.copy(
                            out=ao[0:HDIM, cof : cof + NTOK],
                            in_=pO[0:HDIM, :])
                    else:
                        nc.scalar.copy(
                            out=srh[32:33, w * NTOK : (w + 1) * NTOK],
                            in_=pO[HDIM : HDIM + 1, :])
                        nc.vector.tensor_copy(
                            out=ao[HDIM:128, cof : cof + NTOK],
                            in_=pO[0:HDIM, :])
                if w in (12, NW - 1):
                    nsz = (w - w0 + 1) * NTOK
                    nc.sync.dma_start(
                        out=attn_d[pair * 128 : (pair + 1) * 128,
                                   w0 * NTOK : w0 * NTOK + nsz],
                        in_=ao[:, :nsz])
            for h in range(2):
                nc.sync.dma_start(
                    out=srows[pair * 2 + h : pair * 2 + h + 1, :],
                    in_=srh[h * 32 : h * 32 + 1, :])


# ---------------------------------------------------------------- stage 4+5
def _s4(nc, tc, wproj_d, n1wbi_d, onesb, ln1t, attn_d, x1_d, srows, smrows,
        e2_d):
    # phase 1 (windowed tokens): proj + residual -> x1_d (window order).
    # x is reconstructed from ln1t via per-token (std, mean) rows.
    with tc.tile_pool(name="s4w", bufs=1) as wp, \
         tc.tile_pool(name="s4", bufs=3) as p, \
         tc.tile_pool(name="s4ps", bufs=2, space="PSUM") as pp, \
         tc.tile_pool(name="s4ps1", bufs=1, space="PSUM") as pp1:
        wproj = wp.tile([128, NK, DIM], BF16, tag="wproj")
        nc.sync.dma_start(out=wproj, in_=wproj_d[:, :, :])
        e2 = wp.tile([HEADS, NK * 128], BF16, tag="e2")
        nc.sync.dma_start(out=e2, in_=e2_d[:, :])
        n1wbi = wp.tile([128, NK, 2], F32, tag="n1wbi")
        nc.sync.dma_start(out=n1wbi, in_=n1wbi_d[:, :, :])
        for nt in range(NT45):
            c0 = nt * 392
            csz = min(392, T - c0)
            at = [p.tile([128, 392], BF16, tag=f"at{k}", name=f"at{k}")
                  for k in range(NK)]
            recb = p.tile([HEADS, 392], BF16, tag="recb")
            with nc.allow_low_precision(reason="softmax denominators"):
                nc.vector.reciprocal(out=recb[:, :csz],
                                     in_=srows[:, c0 : c0 + csz])
            pbs_p = pp1.tile([128, 392], F32, tag="pbs")
            nc.tensor.matmul(pbs_p[:, :csz], lhsT=onesb[0:1, :],
                             rhs=smrows[0:1, c0 : c0 + csz],
                             start=True, stop=True)


            for k in range(NK):
                nc.gpsimd.dma_start(
                    out=at[k][:, :csz],
                    in_=attn_d[k * 128 : (k + 1) * 128, c0 : c0 + csz])
                bck = pp.tile([128, 392], F32, tag="bck")
                nc.tensor.matmul(bck[:, :csz],
                                 lhsT=e2[:, k * 128 : (k + 1) * 128],
                                 rhs=recb[:, :csz], start=True, stop=True)
                nc.vector.tensor_mul(out=at[k][:, :csz], in0=at[k][:, :csz],
                                     in1=bck[:, :csz])
            x1 = p.tile([128, NK, 392], BF16, tag="x1")
            for m in range(NK):
                xs = p.tile([128, 392], F32, tag="xs")
                nc.scalar.activation(
                    out=xs[:, :csz], in_=ln1t[m][:, c0 : c0 + csz],
                    func=AF.Identity,
                    scale=n1wbi[:, m, 0:1], bias=n1wbi[:, m, 1:2])
                xsb = p.tile([128, 392], F32, tag="xsb")
                nc.vector.tensor_mul(out=xsb[:, :csz], in0=xs[:, :csz],
                                     in1=pbs_p[:, :csz])
                pj = pp.tile([128, 392], F32, tag="pj", bufs=3)
                nc.tensor.matmul(pj[:, :csz], lhsT=onesb[32:33, :],
                                 rhs=smrows[32:33, c0 : c0 + csz],
                                 start=True, stop=False)
                for k in range(NK):
                    nc.tensor.matmul(
                        pj[:, :csz],
                        lhsT=wproj[:, k, m * 128 : (m + 1) * 128],
                        rhs=at[k][:, :csz], start=False, stop=(k == NK - 1))
                nc.vector.tensor_add(out=x1[:, m, :csz], in0=pj[:, :csz],
                                     in1=xsb[:, :csz])
            nc.sync.dma_start(
                out=x1_d[:, :, c0 : c0 + csz].rearrange("k p c -> p k c"),
                in_=x1[:, :, :csz])


def _s5(nc, tc, wfc1_d, wfc2_d, n2wb_d, onesb, x1_d, x2p, eps5):
    # phase 2 (window order, 13 tiles of 392): LN2 + MLP + residual -> x2_d
    with tc.tile_pool(name="s5w", bufs=1) as wp, \
         tc.tile_pool(name="s5", bufs=2) as p, \
         tc.tile_pool(name="s5h", bufs=1) as hp, \
         tc.tile_pool(name="s5s", bufs=3) as ps, \
         tc.tile_pool(name="s5ps", bufs=2, space="PSUM") as pp, \
         tc.tile_pool(name="s5ps1", bufs=1, space="PSUM") as pp1:
        f1 = wp.tile([128, NK, MLPD], BF16, tag="f1")
        nc.sync.dma_start(out=f1, in_=wfc1_d[:, :, :])
        f2 = wp.tile([128, 24, DIM], BF16, tag="f2")
        nc.sync.dma_start(out=f2, in_=wfc2_d[:, :, :])
        n2wb = wp.tile([128, NK, 2], F32, tag="n2wb")
        nc.sync.dma_start(out=n2wb, in_=n2wb_d[:, :, :])
        for nt in range(NT45):
            c0 = nt * 392
            csz = min(392, T - c0)
            x1 = p.tile([128, NK, 392], BF16, tag="x1p2")
            nc.sync.dma_start(
                out=x1[:, :, :csz],
                in_=x1_d[:, :, c0 : c0 + csz].rearrange("k p c -> p k c"))
            l2 = p.tile([128, NK, 392], BF16, tag="l2")
            srow = pp1.tile([1, 512], F32, tag="srow")
            ssq = pp1.tile([1, 512], F32, tag="ssq")
            for m in range(NK):
                sqm = ps.tile([128, 392], BF16, bufs=2, tag="sqm")
                nc.scalar.activation(out=sqm, in_=x1[:, m, :], func=AF.Square)
                nc.tensor.matmul(srow[:, :csz], lhsT=onesb[:, 0:1],
                                 rhs=x1[:, m, :csz],
                                 start=(m == 0), stop=(m == NK - 1),
                                 skip_group_check=True)
                nc.tensor.matmul(ssq[:, :csz], lhsT=onesb[:, 0:1],
                                 rhs=sqm[:, :csz],
                                 start=(m == 0), stop=(m == NK - 1),
                                 skip_group_check=True)
            rs_b, nm_b = _ln_rows(nc, ps, srow, ssq, csz, DIM, eps5[0:1, :],
                                  "l2")
            pb1 = pp1.tile([128, 512], F32, tag="pb1")
            nc.tensor.matmul(pb1[:, :csz], lhsT=onesb[0:1, :],
                             rhs=rs_b[:, :csz], start=True, stop=True)
            pb2 = pp1.tile([128, 512], F32, tag="pb2")
            nc.tensor.matmul(pb2[:, :csz], lhsT=onesb[0:1, :],
                             rhs=nm_b[:, :csz], start=True, stop=True)
            for m in range(NK):
                t1 = ps.tile([128, 392], F32, tag="t1")
                nc.vector.tensor_mul(out=t1[:, :csz], in0=x1[:, m, :csz],
                                     in1=pb1[:, :csz])
                nc.vector.tensor_add(out=t1[:, :csz], in0=t1[:, :csz],
                                     in1=pb2[:, :csz])
                if APPLY_AFFINE:
                    nc.vector.tensor_scalar(
                        out=l2[:, m, :csz], in0=t1[:, :csz],
                        scalar1=n2wb[:, m, 0:1], scalar2=n2wb[:, m, 1:2],
                        op0=ALU.mult, op1=ALU.add)
                else:
                    nc.vector.tensor_copy(out=l2[:, m, :csz], in_=t1[:, :csz])
            hb = hp.tile([128, 24, 392], BF16, tag="hb")
            for m24 in range(24):
                pf = pp.tile([128, 392], F32, tag="mm5", bufs=3)
                for k in range(NK):
                    nc.tensor.matmul(
                        pf[:, :csz],
                        lhsT=f1[:, k, m24 * 128 : (m24 + 1) * 128],
                        rhs=l2[:, k, :csz], start=(k == 0), stop=(k == NK - 1))
                nc.scalar.activation(out=hb[:, m24, :csz], in_=pf[:, :csz],
                                     func=AF.Gelu)
            x2 = p.tile([128, NK, 392], BF16, tag="x2")
            for m in range(NK):
                pg = pp.tile([128, 392], F32, tag="mm5", bufs=3)
                for k24 in range(24):
                    nc.tensor.matmul(
                        pg[:, :csz],
                        lhsT=f2[:, k24, m * 128 : (m + 1) * 128],
                        rhs=hb[:, k24, :csz], start=(k24 == 0),
                        stop=(k24 == 23))
                nc.vector.tensor_add(out=x2[:, m, :csz], in0=pg[:, :csz],
                                     in1=x1[:, m, :csz])
            srcv = x2.rearrange("p k (a i j) -> p k a i j", a=2, i=WS)
            for wi in range(2):
                w = nt * 2 + wi
                if w >= NW or wi * NTOK >= csz:
                    continue
                wr, wc, ph, pw = wdims(w)
                for k in range(NK):
                    dst = x2p[k].rearrange("p (r c) -> p r c", r=PAD)[
                        :, 1 + WS * wr : 1 + WS * wr + ph,
                        1 + WS * wc : 1 + WS * wc + pw]
                    if k % 2:
                        nc.scalar.copy(out=dst, in_=srcv[:, k, wi, :ph, :pw])
                    else:
                        nc.vector.tensor_copy(out=dst,
                                              in_=srcv[:, k, wi, :ph, :pw])


# ---------------------------------------------------------------- stage 6
def _s6(nc, tc, w1c_d, w2t_d, w3c_d, lnwb_d, onesb, ident_f,
        ident_b, x2p, y, eps6):
    with tc.tile_pool(name="s6w", bufs=1) as wp, \
         tc.tile_pool(name="s6b", bufs=1) as bigp, \
         tc.tile_pool(name="s6", bufs=2) as p, \
         tc.tile_pool(name="s6s", bufs=3) as ps, \
         tc.tile_pool(name="s6ps", bufs=2, space="PSUM") as pp, \
         tc.tile_pool(name="s6ps1", bufs=1, space="PSUM") as pp1:
        w1s = wp.tile([128, NK, BOT], BF16, tag="w1s")
        nc.sync.dma_start(out=w1s, in_=w1c_d[:, :, :])
        w2s = wp.tile([128, 3, 9, BOT], BF16, tag="w2s")
        nc.sync.dma_start(out=w2s, in_=w2t_d[:, :, :, :])
        w3s = wp.tile([128, 3, DIM], BF16, tag="w3s")
        nc.sync.dma_start(out=w3s, in_=w3c_d[:, :, :])
        lnwb = wp.tile([128, 12, 2], F32, tag="lnwb")
        nc.sync.dma_start(out=lnwb, in_=lnwb_d[:, :, :])

        b1p = [bigp.tile([128, PAD * PAD], BF16, tag=f"b1p{k}", name=f"b1p{k}")
               for k in range(3)]
        b2 = [bigp.tile([128, IMG * IMG], BF16, tag=f"b2_{k}", name=f"b2_{k}")
              for k in range(3)]
        for k in range(3):
            nc.gpsimd.memset(b1p[k], 0.0)
        def convln(rhs_fn, wsel, nk_in, nm_out, taps, eps_ap, nt8):
            seq = [(tap, k) for tap in taps for k in range(nk_in)]
            cms = []
            srow = pp1.tile([1, 512], F32, tag="c_srow")
            ssq = pp1.tile([1, 512], F32, tag="c_ssq")
            for m in range(nm_out):
                pc = pp.tile([128, 512], F32, tag="pc", bufs=3)
                for idx, (tap, k) in enumerate(seq):
                    nc.tensor.matmul(
                        pc.rearrange("p (r c) -> p r c", r=8),
                        lhsT=wsel(k, tap, m),
                        rhs=rhs_fn(k, tap, nt8),
                        start=(idx == 0), stop=(idx == len(seq) - 1))
                cm = ps.tile([128, 512], BF16, bufs=7, tag="cm")
                nc.scalar.copy(out=cm, in_=pc)
                sq = ps.tile([128, 512], BF16, bufs=2, tag="csq")
                nc.scalar.activation(out=sq, in_=pc, func=AF.Square)
                cms.append((cm, sq))
                nc.tensor.matmul(srow, lhsT=onesb[:, 0:1], rhs=cm,
                                 start=(m == 0), stop=(m == nm_out - 1),
                                 skip_group_check=True)
                nc.tensor.matmul(ssq, lhsT=onesb[:, 0:1], rhs=sq,
                                 start=(m == 0), stop=(m == nm_out - 1),
                                 skip_group_check=True)
            rs_b, nm_b = _ln_rows(nc, ps, srow, ssq, 512, nm_out * 128,
                                  eps_ap, "c")
            qb1 = pp1.tile([128, 512], F32, tag="qb1")
            nc.tensor.matmul(qb1, lhsT=onesb[0:1, :], rhs=rs_b, start=True,
                             stop=True)
            qb2 = pp1.tile([128, 512], F32, tag="qb2")
            nc.tensor.matmul(qb2, lhsT=onesb[0:1, :], rhs=nm_b, start=True,
                             stop=True)
            outs = []
            for m in range(nm_out):
                t1 = ps.tile([128, 512], F32, bufs=7, tag="c_t1")
                nc.vector.tensor_mul(out=t1, in0=cms[m][0], in1=qb1)
                nc.vector.tensor_add(out=t1, in0=t1, in1=qb2)
                outs.append(t1)
            return outs

        def x2p_rhs(k, tap, nt8):
            return x2p[k].rearrange("p (r c) -> p r c", r=PAD)[
                :, 1 + nt8 * 8 : 1 + nt8 * 8 + 8, 1 : 1 + IMG]

        def b1p_rhs(k, tap, nt8):
            dy, dx = tap
            return b1p[k].rearrange("p (r c) -> p r c", r=PAD)[
                :, nt8 * 8 + dy : nt8 * 8 + dy + 8, dx : dx + IMG]

        def b2_rhs(k, tap, nt8):
            return b2[k].rearrange("p (r c) -> p r c", r=IMG)[
                :, nt8 * 8 : nt8 * 8 + 8, :]

        for nt8 in range(8):
            outs = convln(
                x2p_rhs,
                lambda k, tap, m: w1s[:, k, m * 128 : (m + 1) * 128],
                NK, 3, [None], eps6[0:1, :], nt8)
            for m in range(3):
                nc.scalar.activation(
                    out=b1p[m].rearrange("p (r c) -> p r c", r=PAD)[
                        :, 1 + nt8 * 8 : 1 + nt8 * 8 + 8, 1 : 1 + IMG],
                    in_=outs[m].rearrange("p (r c) -> p r c", r=8),
                    func=AF.Gelu, scale=lnwb[:, m, 0:1], bias=lnwb[:, m, 1:2])
        taps2 = [(dy, dx) for dy in range(3) for dx in range(3)]
        for nt8 in range(8):
            outs = convln(
                b1p_rhs,
                lambda k, tap, m: w2s[:, k, tap[0] * 3 + tap[1],
                                      m * 128 : (m + 1) * 128],
                3, 3, taps2, eps6[0:1, :], nt8)
            for m in range(3):
                nc.scalar.activation(
                    out=b2[m].rearrange("p (r c) -> p r c", r=IMG)[
                        :, nt8 * 8 : nt8 * 8 + 8, :],
                    in_=outs[m].rearrange("p (r c) -> p r c", r=8),
                    func=AF.Gelu, scale=lnwb[:, 3 + m, 0:1],
                    bias=lnwb[:, 3 + m, 1:2])
        for nt8 in range(8):
            outs = convln(
                b2_rhs,
                lambda k, tap, m: w3s[:, k, m * 128 : (m + 1) * 128],
                3, NK, [None], eps6[0:1, :], nt8)
            om = []
            for m in range(NK):
                fin = ps.tile([128, 512], BF16, bufs=7, tag="c_fin")
                nc.scalar.activation(
                    out=fin, in_=outs[m], func=AF.Identity,
                    scale=lnwb[:, 6 + m, 0:1], bias=lnwb[:, 6 + m, 1:2])
                nc.gpsimd.tensor_add(
                    out=fin.rearrange("p (r c) -> p r c", r=8),
                    in0=fin.rearrange("p (r c) -> p r c", r=8),
                    in1=x2p[m].rearrange("p (r c) -> p r c", r=PAD)[
                        :, 1 + nt8 * 8 : 1 + nt8 * 8 + 8, 1 : 1 + IMG])
                om.append(fin)
            for s in range(4):
                outt = p.tile([128, DIM], BF16, tag="outt")
                for m in range(NK):
                    ptp = pp.tile([128, 128], BF16, tag="ptp", bufs=1)
                    nc.tensor.transpose(ptp, om[m][:, s * 128 : (s + 1) * 128],
                                        ident_b)
                    if m % 2:
                        nc.scalar.copy(
                            out=outt[:, m * 128 : (m + 1) * 128], in_=ptp)
                    else:
                        nc.vector.tensor_copy(
                            out=outt[:, m * 128 : (m + 1) * 128], in_=ptp)
                nc.sync.dma_start(
                    out=y[nt8 * 512 + s * 128 : nt8 * 512 + (s + 1) * 128, :],
                    in_=outt)


# ---------------------------------------------------------------- host side
def _chunk(w, nchunk):
    """[nchunk*128, N] -> [128, nchunk, N] (row r = chunk*128 + p)."""
    n = w.shape[1]
    return np.ascontiguousarray(w.reshape(nchunk, 128, n).transpose(1, 0, 2))


def _prep_weights(inputs):
    bf = ml_dtypes.bfloat16
    f8 = ml_dtypes.float8_e4m3
    qkv_w = np.array(inputs["qkv_w"], np.float32)
    qkv_w[:, :DIM] *= HDIM ** -0.5  # fold score scale into Wq
    # [128, (qk, k), 768]: chunk (qk*NK + k) holds input-rows k*128..+128 of
    # the q/k/v block
    wq = np.stack([qkv_w[k * 128 : (k + 1) * 128, qk * DIM : (qk + 1) * DIM]
                   for qk in range(3) for k in range(NK)], axis=1)
    rel_h = np.array(inputs["rel_pos_h"], np.float32)
    rel_w = np.array(inputs["rel_pos_w"], np.float32)
    coords = np.arange(WS)[:, None] - np.arange(WS)[None, :] + WS - 1
    Rh = rel_h[coords]  # [q, k, c]
    Rw = rel_w[coords]
    # lhsT[c, kh] per qh; x sqrt(HDIM) undoes the q-scale fold
    rh_all = np.concatenate([Rh[i].T * (HDIM ** 0.5) for i in range(WS)],
                            axis=1)
    rw_all = np.concatenate([Rw[i].T * (HDIM ** 0.5) for i in range(WS)],
                            axis=1)
    kh = np.arange(NTOK) // WS
    kw = np.arange(NTOK) % WS
    kon = np.zeros((2 * WS, NTOK), np.float32)
    kon[kh, np.arange(NTOK)] = 1.0
    kon[WS + kw, np.arange(NTOK)] = 1.0
    konst = np.tile(kon, (1, NW))
    e2 = np.zeros((HEADS, NK * 128), np.float32)
    for g in range(HEADS):
        k, half = divmod(g, 2)
        e2[g, k * 128 + half * 64 : k * 128 + half * 64 + 64] = 1.0
    conv2_w = np.array(inputs["conv2_w"], np.float32)
    # [128, (kchunk), (tap), 384]
    w2t = np.stack([conv2_w[:, :, dy, dx].T for dy in range(3)
                    for dx in range(3)])  # [9, 384in, 384out]
    w2t = np.ascontiguousarray(
        w2t.reshape(9, 3, 128, BOT).transpose(2, 1, 0, 3))
    lnw = np.concatenate([np.array(inputs[k], np.float32)
                          for k in ("ln1_w", "ln2_w", "ln3_w")])
    lnb = np.concatenate([np.array(inputs[k], np.float32)
                          for k in ("ln1_b", "ln2_b", "ln3_b")])
    lnwb = np.stack([lnw, lnb], axis=1)  # [1536, 2]
    n2wb = np.stack([np.array(inputs["norm2_w"], np.float32),
                     np.array(inputs["norm2_b"], np.float32)], axis=1)
    d = dict(
        wqkv=np.ascontiguousarray(wq).astype(bf),
        wproj=_chunk(np.array(inputs["proj_w"], np.float32), NK).astype(bf),
        wfc1=_chunk(np.array(inputs["fc1_w"], np.float32), NK).astype(bf),
        wfc2=_chunk(np.array(inputs["fc2_w"], np.float32), 24).astype(bf),
        w1c=_chunk(np.array(inputs["conv1_w"], np.float32)[:, :, 0, 0].T,
                   NK).astype(bf),
        w2t=w2t.astype(bf),
        w3c=_chunk(np.array(inputs["conv3_w"], np.float32)[:, :, 0, 0].T,
                   3).astype(bf),
        rh_all=np.ascontiguousarray(rh_all).astype(bf),
        rw_all=np.ascontiguousarray(rw_all).astype(bf),
        konst=np.ascontiguousarray(konst).astype(bf),
        e2=e2.astype(bf),
        idf=np.eye(128, dtype=np.float32),
        idb=np.eye(128).astype(bf),
        n1wb=_chunk(np.stack([np.array(inputs["norm1_w"], np.float32),
                              np.array(inputs["norm1_b"], np.float32)],
                             axis=1), NK),
        n1wbi=_chunk(np.stack(
            [1.0 / np.array(inputs["norm1_w"], np.float32),
             -np.array(inputs["norm1_b"], np.float32)
             / np.array(inputs["norm1_w"], np.float32)], axis=1), NK),
        n2wb=_chunk(n2wb, NK),
        lnwb=_chunk(lnwb, 12),
    )
    return d


def kernel(**inputs):
    nc = build()
    split_multi_waits(nc)
    wts = _prep_weights(inputs)
    x = np.asarray(inputs["x"], np.float32)
    in_maps = [dict(wts, x=np.ascontiguousarray(x[c])) for c in range(N_CORES)]
    res = run_bass_kernel_spmd(nc, in_maps, core_ids=list(range(N_CORES)))
    return np.stack([np.asarray(res.results[c]["y"], np.float32)
                       .reshape(IMG, IMG, DIM)
                     for c in range(N_CORES)])

